# revision 1
# baseline (speedup 1.0000x reference)
"""Multi-head graph attention (rank-2 LeakyReLU-softmax) Trainium2 kernel.

Reference computation (per batch b, head h):
    V = X @ vW + vb                       (N, F)
    q = V @ qW[:,h] + qb[h]               (N,)   per-node scalar
    k = V @ kW[:,h] + kb[h]               (N,)
    A_ij = softmax_j( LeakyReLU(q_i * k_j) )
    out[b,i,h,:] = sum_j A_ij V_j

Key identity used here: with P = max(q,0), M = min(q,0),
alpha = LeakyReLU(k) = max(k, 0.01k), beta = min(k, 0.01k),
    LeakyReLU(q_i * k_j) == alpha_j * P_i + beta_j * M_i      (exactly)
since for each i exactly one of P_i / M_i is nonzero.  So the N x N logit
matrix is a rank-2 outer product, built on the TensorEngine as a K=2
matmul (fp32r), exponentiated on the ScalarEngine straight out of PSUM,
and contracted against [V | 1] without the N x N matrix ever leaving the
chip.  The trailing all-ones column of Vp1 yields the softmax denominator
as row 64 of the same accumulation.

Sharding: core c -> batch b = c//2, heads h0 = 4*(c%2) .. h0+3.
"""

import numpy as np

import concourse.bacc as bacc
import concourse.tile as tile
import concourse.mybir as mybir
from concourse.bass_utils import run_bass_kernel_spmd

B, N, IN, F, H = 4, 2048, 256, 64, 8
NH = H // 2          # heads per core
NT = N // 128        # 16 i-tiles / j-chunks
F32 = mybir.dt.float32
F32R = mybir.dt.float32r
AF = mybir.ActivationFunctionType
ALU = mybir.AluOpType

N_CORES = 8
# packed param tensor columns: ident(128) | vW 2 chunks(128) | vb(1) | qw(4)
# | kw(4) | qb(4) | kb(4)
PRM_COLS = 128 + 128 + 1 + 4 + 4 + 4 + 4
_CACHE = {}
XCAST_DMA = False
import os as _os
ABLATE = int(_os.environ.get("ABL", "0"))  # 1: no acc/postamble (timing probe)


def build_nc(reps=1, unroll=False, version=4):
    """Build the kernel program.

    reps > 1 wraps the whole computation in a hardware For_i loop (all-engine
    barrier between iterations) so test.py can measure per-execution HW time
    by slope: (t(R) - t(1)) / (R - 1).  The graded kernel() path uses reps=1.
    """
    nc = bacc.Bacc("TRN2", target_bir_lowering=False, debug=False,
                   num_devices=N_CORES)
    xshape = [IN, N] if version >= 4 else [N, IN]
    X_d = nc.dram_tensor("X", xshape, F32, kind="ExternalInput")
    vW_d = nc.dram_tensor("vW", [IN, F], F32, kind="ExternalInput")
    vb_d = nc.dram_tensor("vb", [F], F32, kind="ExternalInput")
    qw_d = nc.dram_tensor("qw", [F, NH], F32, kind="ExternalInput")
    kw_d = nc.dram_tensor("kw", [F, NH], F32, kind="ExternalInput")
    qb_d = nc.dram_tensor("qb", [NH], F32, kind="ExternalInput")
    kb_d = nc.dram_tensor("kb", [NH], F32, kind="ExternalInput")
    id_d = nc.dram_tensor("ident", [128, 128], F32, kind="ExternalInput")
    prm_d = nc.dram_tensor("prm", [128, PRM_COLS], F32, kind="ExternalInput")
    out_d = nc.dram_tensor("out", [N, NH * F], F32, kind="ExternalOutput")

    body = {1: _emit_body, 2: _emit_body_v2, 3: _emit_body_v3,
            4: _emit_body_v4}[version]
    extra = {"prm_d": prm_d} if version >= 4 else {}
    with tile.TileContext(nc) as tc:
        from contextlib import ExitStack
        with ExitStack() as rep_ctx:
            if reps > 1 and not unroll:
                rep_ctx.enter_context(tc.For_i(0, reps))
            for _ in range(reps if unroll else 1):
                body(nc, tc, X_d, vW_d, vb_d, qw_d, kw_d, qb_d, kb_d,
                     id_d, out_d, **extra)
    nc.compile()
    return nc


def _emit_body_v2(nc, tc, X_d, vW_d, vb_d, qw_d, kw_d, qb_d, kb_d, id_d,
                  out_d):
    """Software-pipelined main loop.

    Per (head, i-block) "block" (NB = NH*2 of them), per j-chunk step:
      PE:  logit matmul (K=2 rank-2 outer product) -> lt PSUM [128,1024]
      Act: exp straight out of PSUM -> et SBUF (the ONLY Act work)
      PE:  acc matmul [V|1]^T @ et -> acc PSUM [65,1024] accumulated over 16 j
    Steps are emitted with a 1-step skew (logit(s) before acc(s-1)) so PE's
    in-order queue never parks an exp-dependent acc in front of independent
    logit work.  Postamble (PE transpose + DVE normalize into an SBUF staging
    buffer) is interleaved into the following block's steps; output leaves the
    chip in one final DMA.
    """
    NB = NH * 2
    blocks = [(h, ib) for h in range(NH) for ib in range(2)]
    with tc.tile_pool(name="persist", bufs=1) as pp:
        ident = pp.tile([128, 128], F32)
        nc.sync.dma_start(ident[:], id_d[:])
        id_r = pp.tile([128, 128], F32R)
        nc.vector.tensor_copy(id_r[:], ident[:])
        vt_sb = pp.tile([F, N], F32R)         # V^T, bias folded
        qt = pp.tile([NH, N], F32)
        kt = pp.tile([NH, N], F32)
        ab_all = pp.tile([2, NH * N], F32R)   # row0 alpha, row1 beta; head h at cols h*N
        pm_all = pp.tile([2, NH * N], F32R)   # row0 P, row1 M
        vp1 = pp.tile([128, NT * (F + 1)], F32R)   # [V | 1] per j-tile
        obuf = pp.tile([128, NT * NH * F], F32)    # staged output

        # ---------- preamble: X^T, V^T, q/k ----------
        with tc.tile_pool(name="pre_sb", bufs=1) as sp:
            xsb = sp.tile([128, NT * IN], F32)
            nc.sync.dma_start(
                xsb[:].rearrange("p (t c) -> p t c", t=NT),
                X_d[:].rearrange("(t p) c -> p t c", p=128))
            vwsb = sp.tile([128, 128], F32)
            nc.sync.dma_start(
                vwsb[:].rearrange("p (t f) -> p t f", t=2),
                vW_d[:].rearrange("(t p) f -> p t f", p=128))
            vb_t = sp.tile([F, 1], F32)
            nc.sync.dma_start(vb_t[:], vb_d[:].unsqueeze(1))
            qw_t = sp.tile([F, NH], F32)
            nc.sync.dma_start(qw_t[:], qw_d[:])
            kw_t = sp.tile([F, NH], F32)
            nc.sync.dma_start(kw_t[:], kw_d[:])
            qb_t = sp.tile([NH, 1], F32)
            nc.sync.dma_start(qb_t[:], qb_d[:].unsqueeze(1))
            kb_t = sp.tile([NH, 1], F32)
            nc.sync.dma_start(kb_t[:], kb_d[:].unsqueeze(1))

            xt = sp.tile([128, 2 * N], F32R)  # X^T: chunk cc at cc*N
            vw_r = sp.tile([128, 128], F32R)
            nc.vector.tensor_copy(vw_r[:], vwsb[:])
            qw_r = sp.tile([F, NH], F32R)
            nc.vector.tensor_copy(qw_r[:], qw_t[:])
            kw_r = sp.tile([F, NH], F32R)
            nc.vector.tensor_copy(kw_r[:], kw_t[:])

            with tc.tile_pool(name="pre_ps", bufs=2, space="PSUM") as xp:
                for t in range(NT):
                    for cc in range(2):
                        tp = xp.tile([128, 128], F32)
                        nc.tensor.transpose(
                            tp[:], xsb[:, t * IN + cc * 128:
                                       t * IN + cc * 128 + 128], ident[:])
                        nc.vector.tensor_copy(
                            xt[:, cc * N + t * 128: cc * N + t * 128 + 128],
                            tp[:])

            with tc.tile_pool(name="vt_ps", bufs=1, space="PSUM") as vpp:
                vt_ps = vpp.tile([F, N], F32)
                for nb in range(4):
                    for cc in range(2):
                        nc.tensor.matmul(
                            vt_ps[:, nb * 512: nb * 512 + 512],
                            vw_r[:, cc * F: cc * F + F],
                            xt[:, cc * N + nb * 512: cc * N + nb * 512 + 512],
                            start=(cc == 0), stop=(cc == 1))
                nc.vector.tensor_scalar_add(vt_sb[:], vt_ps[:], vb_t[:])

            with tc.tile_pool(name="qk_ps", bufs=1, space="PSUM") as qpp:
                qt_ps = qpp.tile([NH, N], F32)
                kt_ps = qpp.tile([NH, N], F32)
                for nb in range(4):
                    nc.tensor.matmul(
                        qt_ps[:, nb * 512: nb * 512 + 512], qw_r[:],
                        vt_sb[:, nb * 512: nb * 512 + 512],
                        start=True, stop=True)
                    nc.tensor.matmul(
                        kt_ps[:, nb * 512: nb * 512 + 512], kw_r[:],
                        vt_sb[:, nb * 512: nb * 512 + 512],
                        start=True, stop=True)
                nc.vector.tensor_scalar_add(qt[:], qt_ps[:], qb_t[:])
                nc.vector.tensor_scalar_add(kt[:], kt_ps[:], kb_t[:])

            # per-head vectors, written [alpha0..3 | beta0..3] then paired
            abq = sp.tile([2 * NH, N], F32R)
            pmq = sp.tile([2 * NH, N], F32R)
            nc.vector.scalar_tensor_tensor(abq[0:NH, :], kt[:], 0.01, kt[:],
                                           ALU.mult, ALU.max)
            nc.vector.scalar_tensor_tensor(abq[NH:2 * NH, :], kt[:], 0.01,
                                           kt[:], ALU.mult, ALU.min)
            nc.vector.tensor_scalar_max(pmq[0:NH, :], qt[:], 0.0)
            nc.vector.tensor_scalar_min(pmq[NH:2 * NH, :], qt[:], 0.0)
            nc.sync.dma_start(
                ab_all[0:1, :].rearrange("o (h n) -> o h n", h=NH),
                abq[0:NH, :].unsqueeze(0))
            nc.sync.dma_start(
                ab_all[1:2, :].rearrange("o (h n) -> o h n", h=NH),
                abq[NH:2 * NH, :].unsqueeze(0))
            nc.sync.dma_start(
                pm_all[0:1, :].rearrange("o (h n) -> o h n", h=NH),
                pmq[0:NH, :].unsqueeze(0))
            nc.sync.dma_start(
                pm_all[1:2, :].rearrange("o (h n) -> o h n", h=NH),
                pmq[NH:2 * NH, :].unsqueeze(0))

        # ---------- Vp1 = [V | 1] per j-tile ----------
        nc.vector.memset(vp1[:].bitcast(F32), 1.0)
        with tc.tile_pool(name="v_ps", bufs=2, space="PSUM") as vp:
            for t in range(NT):
                v_ps = vp.tile([128, F], F32R)
                nc.tensor.transpose(
                    v_ps[:], vt_sb[:, t * 128: t * 128 + 128],
                    id_r[0:F, 0:F])
                nc.vector.tensor_copy(
                    vp1[:, t * (F + 1): t * (F + 1) + F], v_ps[:])

        # ---------- software-pipelined main loop ----------
        S = NB * NT  # 128 steps
        with tc.tile_pool(name="lt_ps", bufs=2, space="PSUM") as ltp, \
             tc.tile_pool(name="acc_ps", bufs=1, space="PSUM") as accp, \
             tc.tile_pool(name="ht_ps", bufs=2, space="PSUM") as htp, \
             tc.tile_pool(name="et_sb", bufs=4) as etp, \
             tc.tile_pool(name="post_sb", bufs=2) as postp:
            lts, ets, accs, hsbs = {}, {}, {}, {}

            def emit_logit(s):
                b, jc = divmod(s, NT)
                h, ib = blocks[b]
                lt = ltp.tile([128, 1024], F32, tag="lt", name=f"lt{s}")
                for hf in range(2):
                    nc.tensor.matmul(
                        lt[:, hf * 512: hf * 512 + 512],
                        ab_all[:, h * N + jc * 128: h * N + jc * 128 + 128],
                        pm_all[:, h * N + ib * 1024 + hf * 512:
                               h * N + ib * 1024 + hf * 512 + 512],
                        start=True, stop=True)
                lts[s] = lt

            def emit_exp(s):
                et = etp.tile([128, 1024], F32R, tag="et", name=f"et{s}")
                nc.scalar.activation(et[:], lts[s][:], AF.Exp)
                ets[s] = et

            def emit_acc(s):
                b, jc = divmod(s, NT)
                if jc == 0:
                    accs[b] = accp.tile([F + 1, 1024], F32, tag="acc", name=f"acc{b}")
                for hf in range(2):
                    nc.tensor.matmul(
                        accs[b][:, hf * 512: hf * 512 + 512],
                        vp1[:, jc * (F + 1): (jc + 1) * (F + 1)],
                        ets[s][:, hf * 512: hf * 512 + 512],
                        start=(jc == 0), stop=(jc == NT - 1))

            def emit_hsb(b):
                hsb = postp.tile([F + 1, 1024], F32, tag="hsb", name=f"hsb{b}")
                nc.vector.tensor_copy(hsb[:], accs[b][:])
                hsbs[b] = hsb

            def emit_trans(b, t8):
                h, ib = blocks[b]
                ht = htp.tile([128, F + 1], F32, tag="ht", name=f"ht{b}_{t8}")
                nc.tensor.transpose(
                    ht[:], hsbs[b][:, t8 * 128: t8 * 128 + 128],
                    id65[:])
                rcp = postp.tile([128, 1], F32, tag="rcp", name=f"rcp{b}_{t8}")
                nc.vector.reciprocal(rcp[:], ht[:, F:F + 1])
                t = ib * 8 + t8
                nc.vector.tensor_scalar_mul(
                    obuf[:, t * (NH * F) + h * F: t * (NH * F) + h * F + F],
                    ht[:, 0:F], rcp[:])

            for s in range(S + 1):
                if s < S:
                    emit_logit(s)
                if s >= 1:
                    emit_exp(s - 1)
                    emit_acc(s - 1)
                    if (s - 1) % NT == NT - 1:
                        emit_hsb((s - 1) // NT)
                    # spread previous block's 8 transposes over this block
                    b_prev = s // NT - 1
                    jc = s % NT
                    if b_prev >= 0 and s < S and jc % 2 == 1:
                        emit_trans(b_prev, jc // 2)
            for t8 in range(8):  # drain last block
                emit_trans(NB - 1, t8)

        nc.sync.dma_start(
            out_d[:].rearrange("(t p) c -> p t c", p=128),
            obuf[:].rearrange("p (t c) -> p t c", t=NT))


def _emit_body(nc, tc, X_d, vW_d, vb_d, qw_d, kw_d, qb_d, kb_d, id_d, out_d):
    if True:
        with tc.tile_pool(name="persist", bufs=1) as pp:
            ident = pp.tile([128, 128], F32)
            nc.sync.dma_start(ident[:], id_d[:])
            id_r = pp.tile([128, 128], F32R)
            nc.vector.tensor_copy(id_r[:], ident[:])
            vt_sb = pp.tile([F, N], F32R)         # V^T, bias folded
            qt = pp.tile([NH, N], F32)
            kt = pp.tile([NH, N], F32)
            ab_hs = [pp.tile([2, N], F32R, name=f"abh{h}", tag=f"ab{h}") for h in range(NH)]
            pm_hs = [pp.tile([2, N], F32R, name=f"pmh{h}", tag=f"pm{h}") for h in range(NH)]
            vp1 = pp.tile([128, NT * (F + 1)], F32R)   # [V | 1] per j-tile

            # ---------- preamble: X^T, V^T, q/k ----------
            with tc.tile_pool(name="pre_sb", bufs=1) as sp:
                xsb = sp.tile([128, NT * IN], F32)
                nc.sync.dma_start(
                    xsb[:].rearrange("p (t c) -> p t c", t=NT),
                    X_d[:].rearrange("(t p) c -> p t c", p=128))
                vwsb = sp.tile([128, 128], F32)
                nc.sync.dma_start(
                    vwsb[:].rearrange("p (t f) -> p t f", t=2),
                    vW_d[:].rearrange("(t p) f -> p t f", p=128))
                vb_t = sp.tile([F, 1], F32)
                nc.sync.dma_start(vb_t[:], vb_d[:].unsqueeze(1))
                qw_t = sp.tile([F, NH], F32)
                nc.sync.dma_start(qw_t[:], qw_d[:])
                kw_t = sp.tile([F, NH], F32)
                nc.sync.dma_start(kw_t[:], kw_d[:])
                qb_t = sp.tile([NH, 1], F32)
                nc.sync.dma_start(qb_t[:], qb_d[:].unsqueeze(1))
                kb_t = sp.tile([NH, 1], F32)
                nc.sync.dma_start(kb_t[:], kb_d[:].unsqueeze(1))

                xt = sp.tile([128, 2 * N], F32R)  # X^T: chunk cc at cc*N
                vw_r = sp.tile([128, 128], F32R)
                nc.vector.tensor_copy(vw_r[:], vwsb[:])
                qw_r = sp.tile([F, NH], F32R)
                nc.vector.tensor_copy(qw_r[:], qw_t[:])
                kw_r = sp.tile([F, NH], F32R)
                nc.vector.tensor_copy(kw_r[:], kw_t[:])

                with tc.tile_pool(name="pre_ps", bufs=2, space="PSUM") as xp:
                    for t in range(NT):
                        for cc in range(2):
                            tp = xp.tile([128, 128], F32)
                            nc.tensor.transpose(
                                tp[:], xsb[:, t * IN + cc * 128:
                                           t * IN + cc * 128 + 128], ident[:])
                            nc.vector.tensor_copy(
                                xt[:, cc * N + t * 128: cc * N + t * 128 + 128],
                                tp[:])

                with tc.tile_pool(name="vt_ps", bufs=1, space="PSUM") as vpp:
                    vt_ps = vpp.tile([F, N], F32)
                    for nb in range(4):
                        for cc in range(2):
                            nc.tensor.matmul(
                                vt_ps[:, nb * 512: nb * 512 + 512],
                                vw_r[:, cc * F: cc * F + F],
                                xt[:, cc * N + nb * 512: cc * N + nb * 512 + 512],
                                start=(cc == 0), stop=(cc == 1))
                    nc.vector.tensor_scalar_add(vt_sb[:], vt_ps[:], vb_t[:])

                with tc.tile_pool(name="qk_ps", bufs=1, space="PSUM") as qpp:
                    qt_ps = qpp.tile([NH, N], F32)
                    kt_ps = qpp.tile([NH, N], F32)
                    for nb in range(4):
                        nc.tensor.matmul(
                            qt_ps[:, nb * 512: nb * 512 + 512], qw_r[:],
                            vt_sb[:, nb * 512: nb * 512 + 512],
                            start=True, stop=True)
                        nc.tensor.matmul(
                            kt_ps[:, nb * 512: nb * 512 + 512], kw_r[:],
                            vt_sb[:, nb * 512: nb * 512 + 512],
                            start=True, stop=True)
                    nc.vector.tensor_scalar_add(qt[:], qt_ps[:], qb_t[:])
                    nc.vector.tensor_scalar_add(kt[:], kt_ps[:], kb_t[:])

            # ---------- per-head vectors (fp32r) ----------
            with tc.tile_pool(name="vec_sb", bufs=1) as vs:
                a4 = vs.tile([NH, N], F32R)
                b4 = vs.tile([NH, N], F32R)
                p4 = vs.tile([NH, N], F32R)
                m4 = vs.tile([NH, N], F32R)
                nc.vector.scalar_tensor_tensor(a4[:], kt[:], 0.01, kt[:],
                                               ALU.mult, ALU.max)
                nc.vector.scalar_tensor_tensor(b4[:], kt[:], 0.01, kt[:],
                                               ALU.mult, ALU.min)
                nc.vector.tensor_scalar_max(p4[:], qt[:], 0.0)
                nc.vector.tensor_scalar_min(m4[:], qt[:], 0.0)
                for h in range(NH):
                    nc.sync.dma_start(ab_hs[h][0:1, :], a4[h:h + 1, :])
                    nc.sync.dma_start(ab_hs[h][1:2, :], b4[h:h + 1, :])
                    nc.sync.dma_start(pm_hs[h][0:1, :], p4[h:h + 1, :])
                    nc.sync.dma_start(pm_hs[h][1:2, :], m4[h:h + 1, :])

            # ---------- Vp1 = [V | 1] per j-tile ----------
            nc.vector.memset(vp1[:].bitcast(F32), 1.0)
            with tc.tile_pool(name="v_ps", bufs=2, space="PSUM") as vp:
                for t in range(NT):
                    v_ps = vp.tile([128, F], F32R)
                    nc.tensor.transpose(
                        v_ps[:], vt_sb[:, t * 128: t * 128 + 128],
                        id_r[0:F, 0:F])
                    nc.vector.tensor_copy(
                        vp1[:, t * (F + 1): t * (F + 1) + F], v_ps[:])

            # ---------- main loop ----------
            hsbs = {}
            with tc.tile_pool(name="lt_ps", bufs=3, space="PSUM") as ltp, \
                 tc.tile_pool(name="acc_ps", bufs=1, space="PSUM") as accp, \
                 tc.tile_pool(name="et_sb", bufs=3) as etp:
                for h in range(NH):
                    ab_h = ab_hs[h][:]
                    pm_h = pm_hs[h][:]
                    for ib in range(2):
                        acc = accp.tile([F + 1, 1024], F32, tag="acc")
                        for jc in range(NT):
                            lt = ltp.tile([128, 1024], F32, tag="lt", name=f"lt{s}")
                            for hf in range(2):
                                nc.tensor.matmul(
                                    lt[:, hf * 512: hf * 512 + 512],
                                    ab_h[:, jc * 128: jc * 128 + 128],
                                    pm_h[:, ib * 1024 + hf * 512:
                                         ib * 1024 + hf * 512 + 512],
                                    start=True, stop=True)
                            et = etp.tile([128, 1024], F32R, tag="et", name=f"et{s}")
                            nc.scalar.activation(et[:], lt[:], AF.Exp)
                            for hf in range(2):
                                nc.tensor.matmul(
                                    acc[:, hf * 512: hf * 512 + 512],
                                    vp1[:, jc * (F + 1): (jc + 1) * (F + 1)],
                                    et[:, hf * 512: hf * 512 + 512],
                                    start=(jc == 0), stop=(jc == NT - 1))
                        hsb = pp.tile([F + 1, 1024], F32, name=f"hsb{h}_{ib}",
                                      tag=f"hsb{h}_{ib}")
                        nc.vector.tensor_copy(hsb[:], acc[:])
                        hsbs[(h, ib)] = hsb

            # ---------- postamble: transpose + normalize + store ----------
            with tc.tile_pool(name="ht_ps", bufs=4, space="PSUM") as htp, \
                 tc.tile_pool(name="post_sb", bufs=4) as postp:
                for h in range(NH):
                    for ib in range(2):
                        hsb = hsbs[(h, ib)]
                        for t8 in range(8):
                            ht = htp.tile([128, F + 1], F32, tag="ht")
                            nc.tensor.transpose(
                                ht[:], hsb[:, t8 * 128: t8 * 128 + 128],
                                ident[0:F + 1, 0:F + 1])
                            rcp = postp.tile([128, 1], F32, tag="rcp", name=f"rcp{b}_{t8}")
                            nc.vector.reciprocal(rcp[:], ht[:, F:F + 1])
                            ob = postp.tile([128, F], F32, tag="ob")
                            nc.vector.tensor_scalar_mul(ob[:], ht[:, 0:F], rcp[:])
                            r0 = ib * 1024 + t8 * 128
                            nc.sync.dma_start(
                                out_d[r0:r0 + 128, h * F: h * F + F], ob[:])


def _get_nc():
    if "nc" not in _CACHE:
        _CACHE["nc"] = build_nc()
    return _CACHE["nc"]


def make_in_maps(X, vW, vb, qW, qb, kW, kb):
    ident = np.eye(128, dtype=np.float32)
    in_maps = []
    for c in range(N_CORES):
        b, h0 = c // 2, NH * (c % 2)
        qwc = np.ascontiguousarray(qW[:, h0:h0 + NH])
        kwc = np.ascontiguousarray(kW[:, h0:h0 + NH])
        qbc = np.ascontiguousarray(qb[h0:h0 + NH])
        kbc = np.ascontiguousarray(kb[h0:h0 + NH])
        prm = np.zeros((128, PRM_COLS), dtype=np.float32)
        prm[:, 0:128] = ident
        prm[:, 128:256] = vW.reshape(2, 128, F).transpose(1, 0, 2).reshape(128, 128)
        prm[0:F, 256] = vb
        prm[0:F, 257:261] = qwc
        prm[0:F, 261:265] = kwc
        prm[0:1, 265:269] = qbc
        prm[0:1, 269:273] = kbc
        in_maps.append({
            "X": np.ascontiguousarray(X[b].T),
            "vW": np.ascontiguousarray(vW),
            "vb": np.ascontiguousarray(vb),
            "qw": qwc,
            "kw": kwc,
            "qb": qbc,
            "kb": kbc,
            "ident": ident,
            "prm": prm,
        })
    return in_maps


def assemble(results):
    full = np.empty((B, N, H * F), dtype=np.float32)
    for c in range(N_CORES):
        b, h0 = c // 2, NH * (c % 2)
        full[b][:, h0 * F:(h0 + NH) * F] = results[c]["out"]
    return full


def kernel(X, vW, vb, qW, qb, kW, kb):
    X, vW, vb = np.asarray(X), np.asarray(vW), np.asarray(vb)
    qW, qb, kW, kb = np.asarray(qW), np.asarray(qb), np.asarray(kW), np.asarray(kb)
    nc = _get_nc()
    res = run_bass_kernel_spmd(nc, make_in_maps(X, vW, vb, qW, qb, kW, kb),
                               list(range(N_CORES)))
    return assemble(res.results)


def _emit_body_v3(nc, tc, X_d, vW_d, vb_d, qw_d, kw_d, qb_d, kb_d, id_d,
                  out_d):
    """v2 main loop + pipelined preamble and per-block output DMAs.

    Preamble works in 4 node-groups of 512: X DMA group g -> 8 PE transposes
    into a [128,512] PSUM tile -> 2 wide copies (DVE/Pool) -> V^T matmul ->
    bias-add -> q/k matmul -> bias-add -> alpha/beta/P/M chunk -> pack DMA.
    First exp can start after group 0's chain (~7us) instead of after the
    whole preamble.  Act engine does exp ONLY (table preloaded at t=0).
    """
    NB = NH * 2
    blocks = [(h, ib) for h in range(NH) for ib in range(2)]
    with tc.tile_pool(name="persist", bufs=1) as pp:
        # Exp activation-table preload, before anything else on Act.
        zz = pp.tile([1, 2], F32R)
        nc.vector.memset(zz[:].bitcast(F32), 0.0)
        nc.scalar.activation(zz[:], zz[:], AF.Exp)

        id65 = pp.tile([F + 1, F + 1], F32)   # identity for postamble transposes
        id_r = pp.tile([128, 128], F32R)
        vt_sb = pp.tile([F, N], F32R)         # V^T, bias folded
        qt = pp.tile([NH, N], F32)
        kt = pp.tile([NH, N], F32)
        ab_all = pp.tile([2, NH * N], F32R)   # row0 alpha, row1 beta
        pm_all = pp.tile([2, NH * N], F32R)   # row0 P, row1 M
        vp1 = pp.tile([128, NT * (F + 1)], F32R)
        obuf = pp.tile([128, NT * NH * F], F32)
        nc.vector.memset(vp1[:].bitcast(F32), 1.0)

        with tc.tile_pool(name="pre_sb", bufs=1) as sp:
            xsb = sp.tile([128, NT * IN], F32)
            vwsb = sp.tile([128, 128], F32)
            vb_t = sp.tile([F, 1], F32)
            qw_t = sp.tile([F, NH], F32)
            kw_t = sp.tile([F, NH], F32)
            qb_t = sp.tile([NH, 1], F32)
            kb_t = sp.tile([NH, 1], F32)
            abq = sp.tile([2 * NH, N], F32R)  # rows 0-3 alpha, 4-7 beta
            pmq = sp.tile([2 * NH, N], F32R)  # rows 0-3 P, 4-7 M

            # input DMAs: ident+vW first (needed by transposes / V^T), then
            # X in 4 groups; small params via other queues.
            nc.sync.dma_start(ident[:], id_d[:])
            nc.sync.dma_start(
                vwsb[:].rearrange("p (t f) -> p t f", t=2),
                vW_d[:].rearrange("(t p) f -> p t f", p=128))
            for g in range(4):
                nc.sync.dma_start(
                    xsb[:, g * 4 * IN:(g + 1) * 4 * IN]
                        .rearrange("p (t c) -> p t c", t=4),
                    X_d[g * 512:(g + 1) * 512, :]
                        .rearrange("(t p) c -> p t c", p=128))
            nc.scalar.dma_start(vb_t[:], vb_d[:].unsqueeze(1))
            nc.scalar.dma_start(qw_t[:], qw_d[:])
            nc.scalar.dma_start(kw_t[:], kw_d[:])
            nc.gpsimd.dma_start(qb_t[:], qb_d[:].unsqueeze(1))
            nc.gpsimd.dma_start(kb_t[:], kb_d[:].unsqueeze(1))

            xt = sp.tile([128, 2 * N], F32R)  # X^T: chunk cc at cc*N
            vw_r = sp.tile([128, 128], F32R)
            nc.vector.tensor_copy(id_r[:], ident[:])
            nc.gpsimd.tensor_copy(vw_r[:], vwsb[:])
            qw_r = sp.tile([F, NH], F32R)
            nc.vector.tensor_copy(qw_r[:], qw_t[:])
            kw_r = sp.tile([F, NH], F32R)
            nc.vector.tensor_copy(kw_r[:], kw_t[:])

            with tc.tile_pool(name="tp_ps", bufs=2, space="PSUM") as xp, \
                 tc.tile_pool(name="vt_ps", bufs=1, space="PSUM") as vpp, \
                 tc.tile_pool(name="qk_ps", bufs=2, space="PSUM") as qpp, \
                 tc.tile_pool(name="v_ps", bufs=1, space="PSUM") as vsp:
                for g in range(4):
                    # X^T for this group's 4 node-tiles (both 128-col chunks)
                    for cc in range(2):
                        tp = xp.tile([128, 512], F32, tag="tp",
                                     name=f"tp{g}_{cc}")
                        for tt in range(4):
                            t = 4 * g + tt
                            nc.tensor.transpose(
                                tp[:, tt * 128: tt * 128 + 128],
                                xsb[:, t * IN + cc * 128:
                                    t * IN + cc * 128 + 128], ident[:])
                        nc.scalar.copy(
                            xt[:, cc * N + g * 512: cc * N + g * 512 + 512],
                            tp[:])
                    # V^T chunk
                    vt_ps = vpp.tile([F, 512], F32, tag="vtps",
                                     name=f"vtps{g}")
                    for cc in range(2):
                        nc.tensor.matmul(
                            vt_ps[:],
                            vw_r[:, cc * F: cc * F + F],
                            xt[:, cc * N + g * 512: cc * N + g * 512 + 512],
                            start=(cc == 0), stop=(cc == 1))
                    nc.vector.tensor_scalar_add(
                        vt_sb[:, g * 512:(g + 1) * 512], vt_ps[:], vb_t[:])
                    # q / k chunks
                    qt_ps = qpp.tile([NH, 512], F32, tag="qk",
                                     name=f"qtps{g}")
                    nc.tensor.matmul(qt_ps[:], qw_r[:],
                                     vt_sb[:, g * 512: g * 512 + 512],
                                     start=True, stop=True)
                    kt_ps = qpp.tile([NH, 512], F32, tag="qk",
                                     name=f"ktps{g}")
                    nc.tensor.matmul(kt_ps[:], kw_r[:],
                                     vt_sb[:, g * 512: g * 512 + 512],
                                     start=True, stop=True)
                    sl = slice(g * 512, (g + 1) * 512)
                    nc.vector.tensor_scalar_add(qt[:, sl], qt_ps[:], qb_t[:])
                    nc.gpsimd.tensor_scalar_add(kt[:, sl], kt_ps[:], kb_t[:])
                    # alpha/beta (from k), P/M (from q) for this chunk
                    nc.vector.scalar_tensor_tensor(
                        abq[0:NH, sl], kt[:, sl], 0.01, kt[:, sl],
                        ALU.mult, ALU.max)
                    nc.gpsimd.scalar_tensor_tensor(
                        abq[NH:2 * NH, sl], kt[:, sl], 0.01, kt[:, sl],
                        ALU.mult, ALU.min)
                    nc.vector.tensor_scalar_max(pmq[0:NH, sl], qt[:, sl], 0.0)
                    nc.gpsimd.tensor_scalar_min(pmq[NH:2 * NH, sl],
                                                qt[:, sl], 0.0)
                    # pack into [2, NH*N] layout (head-major columns)
                    for row in range(2):
                        nc.gpsimd.dma_start(
                            ab_all[row:row + 1, :]
                                .rearrange("o (h n) -> o h n", h=NH)
                                [:, :, g * 512:(g + 1) * 512],
                            abq[row * NH:(row + 1) * NH, sl].unsqueeze(0))
                        nc.gpsimd.dma_start(
                            pm_all[row:row + 1, :]
                                .rearrange("o (h n) -> o h n", h=NH)
                                [:, :, g * 512:(g + 1) * 512],
                            pmq[row * NH:(row + 1) * NH, sl].unsqueeze(0))
                    # Vp1 tiles for this group
                    v_ps = vsp.tile([128, 4 * F], F32R, tag="vps",
                                    name=f"vps{g}")
                    for tt in range(4):
                        t = 4 * g + tt
                        nc.tensor.transpose(
                            v_ps[:, tt * F: tt * F + F],
                            vt_sb[:, t * 128: t * 128 + 128],
                            id_r[0:F, 0:F])
                    eng = nc.vector if g % 2 == 0 else nc.gpsimd
                    eng.tensor_copy(
                        vp1[:].rearrange("p (t c) -> p t c", c=F + 1)
                            [:, 4 * g: 4 * g + 4, 0:F],
                        v_ps[:].rearrange("p (t c) -> p t c", c=F))

        # ---------- software-pipelined main loop ----------
        S = NB * NT  # 128 steps
        with tc.tile_pool(name="lt_ps", bufs=2, space="PSUM") as ltp, \
             tc.tile_pool(name="acc_ps", bufs=1, space="PSUM") as accp, \
             tc.tile_pool(name="ht_ps", bufs=2, space="PSUM") as htp, \
             tc.tile_pool(name="et_sb", bufs=4) as etp, \
             tc.tile_pool(name="post_sb", bufs=2) as postp:
            lts, ets, accs, hsbs = {}, {}, {}, {}

            def emit_logit(s):
                b, jc = divmod(s, NT)
                h, ib = blocks[b]
                lt = ltp.tile([128, 1024], F32, tag="lt", name=f"lt{s}")
                for hf in range(2):
                    nc.tensor.matmul(
                        lt[:, hf * 512: hf * 512 + 512],
                        ab_all[:, h * N + jc * 128: h * N + jc * 128 + 128],
                        pm_all[:, h * N + ib * 1024 + hf * 512:
                               h * N + ib * 1024 + hf * 512 + 512],
                        start=True, stop=True)
                lts[s] = lt

            def emit_exp(s):
                et = etp.tile([128, 1024], F32R, tag="et", name=f"et{s}")
                nc.scalar.activation(et[:], lts[s][:], AF.Exp)
                ets[s] = et

            def emit_acc(s):
                b, jc = divmod(s, NT)
                if jc == 0:
                    accs[b] = accp.tile([F + 1, 1024], F32, tag="acc",
                                        name=f"acc{b}")
                for hf in range(2):
                    nc.tensor.matmul(
                        accs[b][:, hf * 512: hf * 512 + 512],
                        vp1[:, jc * (F + 1): (jc + 1) * (F + 1)],
                        ets[s][:, hf * 512: hf * 512 + 512],
                        start=(jc == 0), stop=(jc == NT - 1))

            def emit_hsb(b):
                hsb = postp.tile([F + 1, 1024], F32, tag="hsb",
                                 name=f"hsb{b}")
                nc.vector.tensor_copy(hsb[:, 0:512], accs[b][:, 0:512])
                nc.gpsimd.tensor_copy(hsb[:, 512:1024], accs[b][:, 512:1024])
                hsbs[b] = hsb

            def emit_trans(b, t8):
                h, ib = blocks[b]
                ht = htp.tile([128, F + 1], F32, tag="ht", name=f"ht{b}_{t8}")
                nc.tensor.transpose(
                    ht[:], hsbs[b][:, t8 * 128: t8 * 128 + 128],
                    id65[:])
                rcp = postp.tile([128, 1], F32, tag="rcp",
                                 name=f"rcp{b}_{t8}")
                nc.vector.reciprocal(rcp[:], ht[:, F:F + 1])
                t = ib * 8 + t8
                nc.vector.tensor_scalar_mul(
                    obuf[:, t * (NH * F) + h * F: t * (NH * F) + h * F + F],
                    ht[:, 0:F], rcp[:])
                if t8 == 7:
                    emit_outdma(b)

            def emit_outdma(b):
                h, ib = blocks[b]
                nc.sync.dma_start(
                    out_d[ib * 1024:(ib + 1) * 1024, h * F:(h + 1) * F]
                        .rearrange("(t p) c -> p t c", p=128),
                    obuf[:].rearrange("p (t c) -> p t c", c=NH * F)
                        [:, ib * 8:(ib + 1) * 8, h * F:(h + 1) * F])

            for s in range(S + 1):
                if s < S:
                    emit_logit(s)
                if s >= 1:
                    emit_exp(s - 1)
                    emit_acc(s - 1)
                    if (s - 1) % NT == NT - 1:
                        emit_hsb((s - 1) // NT)
                    b_prev = s // NT - 1
                    jc = s % NT
                    if b_prev >= 0 and s < S and jc % 2 == 1:
                        emit_trans(b_prev, jc // 2)
            for t8 in range(8):  # drain last block
                emit_trans(NB - 1, t8)


def _emit_body_v4(nc, tc, X_d, vW_d, vb_d, qw_d, kw_d, qb_d, kb_d, id_d,
                  out_d, prm_d=None):
    """v3 + lane-aligned preamble, no per-chunk pack DMAs.

    q/k are produced by matmuls whose lhsT is zero-padded so head h's scalar
    lands (duplicated) on partitions {32h, 32h+1}; q/k biases ride a ones row
    appended to V^T (so the q/k matmul adds them via K=65).  alpha/beta/P/M
    are then single strided DVE/Pool ops straight out of PSUM into the
    matmul-legal [128, N] layouts (alpha_h/P_h at partition 32h, beta_h/M_h
    at 32h+1).  Head 3 (base 96 — illegal for PE) is staged to a [2, N] tile
    by one DMA per tensor at preamble end; its blocks run last.
    """
    NB = NH * 2
    blocks = [(h, ib) for h in range(NH) for ib in range(2)]
    with tc.tile_pool(name="persist", bufs=1) as pp:
        zz = pp.tile([1, 2], F32R)
        nc.vector.memset(zz[:].bitcast(F32), 0.0)
        nc.scalar.activation(zz[:], zz[:], AF.Exp)

        id65 = pp.tile([F + 1, F + 1], F32)   # identity for postamble transposes
        id_r = pp.tile([128, 128], F32R)
        vt1 = pp.tile([F + 1, N], F32R)       # V^T rows 0..63, row 64 = ones
        ab_sp = pp.tile([128, N], F32R)       # part 32h = alpha_h, 32h+1 = beta_h
        pm_sp = pp.tile([128, N], F32R)       # part 32h = P_h, 32h+1 = M_h
        ab3 = pp.tile([2, N], F32R)
        pm3 = pp.tile([2, N], F32R)
        vp1 = pp.tile([128, NT * (F + 1)], F32R)
        obuf = pp.tile([128, NT * NH * F], F32)
        nc.vector.memset(vp1[:].bitcast(F32), 1.0)
        nc.vector.memset(vt1[F:F + 1, :].bitcast(F32), 1.0)

        if True:
            sp = pp  # preamble tensors live in the persistent pool: their
            # SBUF never gets recycled under the main loop's et/hsb tiles,
            # so the first exp isn't serialized behind the preamble's tail.
            xt = sp.tile([128, 2 * N], F32R)  # X^T: chunk cc at cc*N

            # One packed-param DMA (ident | vW | vb | qw | kw | qb | kb)
            # then the four X^T groups (host supplies X transposed), all FIFO
            # on the sync HWDGE queue: params land by ~2us, X owns the bus
            # right after, and each 512-node group is immediately matmul-ready
            # (no on-chip transposes).
            prm = sp.tile([128, PRM_COLS], F32)
            nc.sync.dma_start(prm[:], prm_d[:])
            xtr = sp.tile([128, 2 * N], F32)
            for g in range(4):
                if XCAST_DMA:
                    # gpsimd DMA casts f32 -> f32r in flight (the only
                    # engine allowed to), so X^T lands matmul-ready.
                    nc.gpsimd.dma_start(
                        xt[:].rearrange("p (cc n) -> p cc n", cc=2)
                            [:, :, g * 512:(g + 1) * 512],
                        X_d[:].rearrange("(cc p) n -> p cc n", p=128)
                            [:, :, g * 512:(g + 1) * 512])
                else:
                    nc.sync.dma_start(
                        xtr[:].rearrange("p (cc n) -> p cc n", cc=2)
                            [:, :, g * 512:(g + 1) * 512],
                        X_d[:].rearrange("(cc p) n -> p cc n", p=128)
                            [:, :, g * 512:(g + 1) * 512])
                    for cc in range(2):
                        eng = nc.vector if cc == 0 else nc.scalar
                        s0 = cc * N + g * 512
                        if cc == 0:
                            eng.tensor_copy(xt[:, s0:s0 + 512],
                                            xtr[:, s0:s0 + 512])
                        else:
                            eng.copy(xt[:, s0:s0 + 512], xtr[:, s0:s0 + 512])
            ident = prm[:, 0:128]
            vwsb = prm[:, 128:256]
            vb_t = prm[0:F, 256:257]
            qw_t = prm[0:F, 257:261]
            kw_t = prm[0:F, 261:265]
            qb_row = prm[0:1, 265:269]
            kb_row = prm[0:1, 269:273]

            vw_r = sp.tile([128, 128], F32R)
            kscr0 = sp.tile([128, 512], F32)
            kscr1 = sp.tile([128, 512], F32)
            kscr = [kscr0, kscr1]
            nc.vector.tensor_copy(id_r[:], ident[:])
            nc.gpsimd.tensor_copy(id65[:], ident[0:F + 1, 0:F + 1])
            nc.gpsimd.tensor_copy(vw_r[:], vwsb[:])

            # padded q/k lhsT: [65, 128]; rows 0..63 = w dup at {32h,32h+1},
            # row 64 = bias dup there too; zero elsewhere.
            # Padded lhsT columns: even col 32h = +w_h (+bias), odd col
            # 32h+1 = -w_h (-bias).  Odd PSUM lanes then hold -k / -q, so a
            # single full-width max() yields [alpha; -beta] / [P; -M]; the
            # rank-2 logit contraction multiplies the two odd rows together
            # and the negations cancel.
            qkw = sp.tile([F + 1, 128], F32R)
            kkw = sp.tile([F + 1, 128], F32R)
            nc.vector.memset(qkw[:].bitcast(F32), 0.0)
            nc.vector.memset(kkw[:].bitcast(F32), 0.0)
            for rr in range(2):
                sgn = 1.0 if rr == 0 else -1.0
                nc.vector.tensor_scalar_mul(
                    qkw[0:F, :].rearrange("f (h r) -> f h r", r=32)
                        [:, :, rr:rr + 1],
                    qw_t[:].unsqueeze(2), sgn)
                nc.vector.tensor_scalar_mul(
                    qkw[F:F + 1, :].rearrange("o (h r) -> o h r", r=32)
                        [:, :, rr:rr + 1],
                    qb_row[:].unsqueeze(2), sgn)
                nc.vector.tensor_scalar_mul(
                    kkw[0:F, :].rearrange("f (h r) -> f h r", r=32)
                        [:, :, rr:rr + 1],
                    kw_t[:].unsqueeze(2), sgn)
                nc.vector.tensor_scalar_mul(
                    kkw[F:F + 1, :].rearrange("o (h r) -> o h r", r=32)
                        [:, :, rr:rr + 1],
                    kb_row[:].unsqueeze(2), sgn)

            with tc.tile_pool(name="vt_ps", bufs=1, space="PSUM") as vpp, \
                 tc.tile_pool(name="qk_ps", bufs=2, space="PSUM") as qpp, \
                 tc.tile_pool(name="v_ps", bufs=1, space="PSUM") as vsp:
                for g in range(4):
                    sl = slice(g * 512, (g + 1) * 512)
                    vt_ps = vpp.tile([F, 512], F32, tag="vtps",
                                     name=f"vtps{g}")
                    for cc in range(2):
                        nc.tensor.matmul(
                            vt_ps[:],
                            vw_r[:, cc * F: cc * F + F],
                            xt[:, cc * N + g * 512: cc * N + g * 512 + 512],
                            start=(cc == 0), stop=(cc == 1))
                    nc.vector.tensor_scalar_add(vt1[0:F, sl], vt_ps[:],
                                                vb_t[:])
                    qt_ps = qpp.tile([128, 512], F32, tag="qk",
                                     name=f"qtps{g}")
                    nc.tensor.matmul(qt_ps[:], qkw[:], vt1[:, sl],
                                     start=True, stop=True)
                    kt_ps = qpp.tile([128, 512], F32, tag="qk",
                                     name=f"ktps{g}")
                    nc.tensor.matmul(kt_ps[:], kkw[:], vt1[:, sl],
                                     start=True, stop=True)
                    # LeakyReLU with one PSUM read per instruction (the
                    # HW forbids two): Act scales 0.01*k into SBUF scratch,
                    # DVE maxes it against k.  Thanks to the negated odd
                    # lanes this yields [alpha; -beta]; Relu gives [P; -M].
                    nc.scalar.mul(kscr[g % 2][:], kt_ps[:], 0.01)
                    nc.vector.tensor_tensor(
                        ab_sp[:, sl], kscr[g % 2][:], kt_ps[:], ALU.max)
                    nc.scalar.activation(pm_sp[:, sl], qt_ps[:], AF.Relu)
                    v_ps = vsp.tile([128, 4 * F], F32R, tag="vps",
                                    name=f"vps{g}")
                    for tt in range(4):
                        t = 4 * g + tt
                        nc.tensor.transpose(
                            v_ps[:, tt * F: tt * F + F],
                            vt1[0:F, t * 128: t * 128 + 128],
                            id_r[0:F, 0:F])
                    nc.vector.tensor_copy(
                        vp1[:].rearrange("p (t c) -> p t c", c=F + 1)
                            [:, 4 * g: 4 * g + 4, 0:F],
                        v_ps[:].rearrange("p (t c) -> p t c", c=F))
            # head 3 lives at base 96 — stage its pairs to base-0 tiles
            nc.sync.dma_start(ab3[:], ab_sp[96:98, :])
            nc.sync.dma_start(pm3[:], pm_sp[96:98, :])

        # ---------- software-pipelined main loop ----------
        # A shield pool pins the 4 banks the preamble just released, so the
        # first two lt tiles claim the never-used banks 4-7 and the first
        # logits aren't serialized behind the tail of the preamble.
        # Pool creation order fixes PSUM bank assignment (first-fit from
        # bank 0): acc and ht soak up the banks the preamble just released
        # (they are needed later / tolerate the wait), so the lt tiles land
        # on the four never-touched banks and the first logits run as soon
        # as their operands are ready.
        S = NB * NT
        with tc.tile_pool(name="acc_ps", bufs=1, space="PSUM") as accp, \
             tc.tile_pool(name="lt_ps", bufs=3, space="PSUM") as ltp, \
             tc.tile_pool(name="et_sb", bufs=6) as etp, \
             tc.tile_pool(name="post_sb", bufs=2) as postp:
            lts, ets, accs, hsbs = {}, {}, {}, {}

            def abpm(h):
                if h < 3:
                    return (ab_sp[32 * h: 32 * h + 2, :],
                            pm_sp[32 * h: 32 * h + 2, :])
                return ab3[:], pm3[:]

            def emit_logit(s):
                b, jc = divmod(s, NT)
                h, ib = blocks[b]
                ab_h, pm_h = abpm(h)
                lt = ltp.tile([128, 1024], F32, tag="lt", name=f"lt{s}")
                for hf in range(2):
                    nc.tensor.matmul(
                        lt[:, hf * 512: hf * 512 + 512],
                        ab_h[:, jc * 128: jc * 128 + 128],
                        pm_h[:, ib * 1024 + hf * 512:
                             ib * 1024 + hf * 512 + 512],
                        start=True, stop=True)
                lts[s] = lt

            def emit_exp(s):
                et = etp.tile([128, 1024], F32R, tag="et", name=f"et{s}")
                nc.scalar.activation(et[:], lts[s][:], AF.Exp)
                ets[s] = et

            def emit_acc(s):
                b, jc = divmod(s, NT)
                if jc == 0:
                    accs[b] = accp.tile([F + 1, 1024], F32, tag="acc",
                                        name=f"acc{b}")
                for hf in range(2):
                    nc.tensor.matmul(
                        accs[b][:, hf * 512: hf * 512 + 512],
                        vp1[:, jc * (F + 1): (jc + 1) * (F + 1)],
                        ets[s][:, hf * 512: hf * 512 + 512],
                        start=(jc == 0), stop=(jc == NT - 1))

            def emit_hsb(b):
                hsb = postp.tile([F + 1, 1024], F32, tag="hsb",
                                 name=f"hsb{b}")
                nc.vector.tensor_copy(hsb[:, 0:512], accs[b][:, 0:512])
                nc.vector.tensor_copy(hsb[:, 512:1024], accs[b][:, 512:1024])
                hsbs[b] = hsb

            def emit_trans(b, t8):
                h, ib = blocks[b]
                # ht tiles share the lt tag: one 3-deep rotation covers
                # both, freeing two PSUM banks for the deeper lt buffering
                ht = ltp.tile([128, F + 1], F32, tag="lt", name=f"ht{b}_{t8}")
                nc.tensor.transpose(
                    ht[:], hsbs[b][:, t8 * 128: t8 * 128 + 128],
                    id65[:])
                rcp = postp.tile([128, 1], F32, tag="rcp",
                                 name=f"rcp{b}_{t8}")
                nc.vector.reciprocal(rcp[:], ht[:, F:F + 1])
                t = ib * 8 + t8
                nc.vector.tensor_scalar_mul(
                    obuf[:, t * (NH * F) + h * F: t * (NH * F) + h * F + F],
                    ht[:, 0:F], rcp[:])
                if b == NB - 1:
                    if t8 == 3:
                        emit_outdma(b, 0, 4)
                    elif t8 == 7:
                        emit_outdma(b, 4, 8)
                elif t8 == 7:
                    emit_outdma(b, 0, 8)

            def emit_outdma(b, t0, t1):
                h, ib = blocks[b]
                nc.sync.dma_start(
                    out_d[ib * 1024 + t0 * 128: ib * 1024 + t1 * 128,
                          h * F:(h + 1) * F]
                        .rearrange("(t p) c -> p t c", p=128),
                    obuf[:].rearrange("p (t c) -> p t c", c=NH * F)
                        [:, ib * 8 + t0: ib * 8 + t1, h * F:(h + 1) * F])

            if ABLATE >= 1:
                nc.vector.memset(obuf[:], 0.0)
            if ABLATE == 2:
                # pure Act throughput: one logit tile, 128 exps off it
                emit_logit(0)
                for s in range(S):
                    emit_exp(0)
                for b in range(NB):
                    emit_outdma(b, 0, 8)
                return
            for s in range(S + 3):
                if s < S:
                    emit_logit(s)
                if 1 <= s <= S:
                    emit_exp(s - 1)
                if s >= 3 and ABLATE != 1:
                    a = s - 3
                    emit_acc(a)
                    if a % NT == NT - 1:
                        emit_hsb(a // NT)
                    b_prev = a // NT - 1
                    jc = a % NT
                    if b_prev >= 0 and jc % 2 == 1:
                        emit_trans(b_prev, jc // 2)
            if ABLATE != 1:
                for t8 in range(8):
                    emit_trans(NB - 1, t8)
            else:
                for b in range(NB):
                    emit_outdma(b, 0, 8)



# revision 9
# speedup vs baseline: 1.0117x; 1.0117x over previous
"""Multi-head graph attention (rank-2 LeakyReLU-softmax) Trainium2 kernel.

Reference computation (per batch b, head h):
    V = X @ vW + vb                       (N, F)
    q = V @ qW[:,h] + qb[h]               (N,)   per-node scalar
    k = V @ kW[:,h] + kb[h]               (N,)
    A_ij = softmax_j( LeakyReLU(q_i * k_j) )
    out[b,i,h,:] = sum_j A_ij V_j

Key identity used here: with P = max(q,0), M = min(q,0),
alpha = LeakyReLU(k) = max(k, 0.01k), beta = min(k, 0.01k),
    LeakyReLU(q_i * k_j) == alpha_j * P_i + beta_j * M_i      (exactly)
since for each i exactly one of P_i / M_i is nonzero.  So the N x N logit
matrix is a rank-2 outer product, built on the TensorEngine as a K=2
matmul (fp32r), exponentiated on the ScalarEngine straight out of PSUM,
and contracted against [V | 1] without the N x N matrix ever leaving the
chip.  The trailing all-ones column of Vp1 yields the softmax denominator
as row 64 of the same accumulation.

Sharding: core c -> batch b = c//2, heads h0 = 4*(c%2) .. h0+3.
"""

import numpy as np

import concourse.bacc as bacc
import concourse.tile as tile
import concourse.mybir as mybir
from concourse.bass_utils import run_bass_kernel_spmd

B, N, IN, F, H = 4, 2048, 256, 64, 8
NH = H // 2          # heads per core
NT = N // 128        # 16 i-tiles / j-chunks
F32 = mybir.dt.float32
F32R = mybir.dt.float32r
AF = mybir.ActivationFunctionType
ALU = mybir.AluOpType

N_CORES = 8
# packed param tensor columns: ident(128) | vW 2 chunks(128) | vb(1) | qw(4)
# | kw(4) | qb(4) | kb(4)
PRM_COLS = 128 + 128 + 1 + 4 + 4 + 4 + 4
_CACHE = {}
XCAST_DMA = False
import os as _os
ABLATE = int(_os.environ.get("ABL", "0"))  # 1: no acc/postamble (timing probe)
ACC_BF16 = int(_os.environ.get("ACCBF", "1"))  # bf16 vp1/et for the acc matmul
HSB_GP = int(_os.environ.get("HSBGP", "0"))    # gpsimd can't read PSUM (walrus)


def build_nc(reps=1, unroll=False, version=4):
    """Build the kernel program.

    reps > 1 wraps the whole computation in a hardware For_i loop (all-engine
    barrier between iterations) so test.py can measure per-execution HW time
    by slope: (t(R) - t(1)) / (R - 1).  The graded kernel() path uses reps=1.
    """
    nc = bacc.Bacc("TRN2", target_bir_lowering=False, debug=False,
                   num_devices=N_CORES)
    xshape = [IN, N] if version >= 4 else [N, IN]
    X_d = nc.dram_tensor("X", xshape, F32, kind="ExternalInput")
    vW_d = nc.dram_tensor("vW", [IN, F], F32, kind="ExternalInput")
    vb_d = nc.dram_tensor("vb", [F], F32, kind="ExternalInput")
    qw_d = nc.dram_tensor("qw", [F, NH], F32, kind="ExternalInput")
    kw_d = nc.dram_tensor("kw", [F, NH], F32, kind="ExternalInput")
    qb_d = nc.dram_tensor("qb", [NH], F32, kind="ExternalInput")
    kb_d = nc.dram_tensor("kb", [NH], F32, kind="ExternalInput")
    id_d = nc.dram_tensor("ident", [128, 128], F32, kind="ExternalInput")
    prm_d = nc.dram_tensor("prm", [128, PRM_COLS], F32, kind="ExternalInput")
    out_d = nc.dram_tensor("out", [N, NH * F], F32, kind="ExternalOutput")

    body = {1: _emit_body, 2: _emit_body_v2, 3: _emit_body_v3,
            4: _emit_body_v4}[version]
    extra = {"prm_d": prm_d} if version >= 4 else {}
    with tile.TileContext(nc) as tc:
        from contextlib import ExitStack
        with ExitStack() as rep_ctx:
            if reps > 1 and not unroll:
                rep_ctx.enter_context(tc.For_i(0, reps))
            for _ in range(reps if unroll else 1):
                body(nc, tc, X_d, vW_d, vb_d, qw_d, kw_d, qb_d, kb_d,
                     id_d, out_d, **extra)
    nc.compile()
    return nc


def _emit_body_v2(nc, tc, X_d, vW_d, vb_d, qw_d, kw_d, qb_d, kb_d, id_d,
                  out_d):
    """Software-pipelined main loop.

    Per (head, i-block) "block" (NB = NH*2 of them), per j-chunk step:
      PE:  logit matmul (K=2 rank-2 outer product) -> lt PSUM [128,1024]
      Act: exp straight out of PSUM -> et SBUF (the ONLY Act work)
      PE:  acc matmul [V|1]^T @ et -> acc PSUM [65,1024] accumulated over 16 j
    Steps are emitted with a 1-step skew (logit(s) before acc(s-1)) so PE's
    in-order queue never parks an exp-dependent acc in front of independent
    logit work.  Postamble (PE transpose + DVE normalize into an SBUF staging
    buffer) is interleaved into the following block's steps; output leaves the
    chip in one final DMA.
    """
    NB = NH * 2
    blocks = [(h, ib) for h in range(NH) for ib in range(2)]
    with tc.tile_pool(name="persist", bufs=1) as pp:
        ident = pp.tile([128, 128], F32)
        nc.sync.dma_start(ident[:], id_d[:])
        id_r = pp.tile([128, 128], F32R)
        nc.vector.tensor_copy(id_r[:], ident[:])
        vt_sb = pp.tile([F, N], F32R)         # V^T, bias folded
        qt = pp.tile([NH, N], F32)
        kt = pp.tile([NH, N], F32)
        ab_all = pp.tile([2, NH * N], F32R)   # row0 alpha, row1 beta; head h at cols h*N
        pm_all = pp.tile([2, NH * N], F32R)   # row0 P, row1 M
        vp1 = pp.tile([128, NT * (F + 1)], F32R)   # [V | 1] per j-tile
        obuf = pp.tile([128, NT * NH * F], F32)    # staged output

        # ---------- preamble: X^T, V^T, q/k ----------
        with tc.tile_pool(name="pre_sb", bufs=1) as sp:
            xsb = sp.tile([128, NT * IN], F32)
            nc.sync.dma_start(
                xsb[:].rearrange("p (t c) -> p t c", t=NT),
                X_d[:].rearrange("(t p) c -> p t c", p=128))
            vwsb = sp.tile([128, 128], F32)
            nc.sync.dma_start(
                vwsb[:].rearrange("p (t f) -> p t f", t=2),
                vW_d[:].rearrange("(t p) f -> p t f", p=128))
            vb_t = sp.tile([F, 1], F32)
            nc.sync.dma_start(vb_t[:], vb_d[:].unsqueeze(1))
            qw_t = sp.tile([F, NH], F32)
            nc.sync.dma_start(qw_t[:], qw_d[:])
            kw_t = sp.tile([F, NH], F32)
            nc.sync.dma_start(kw_t[:], kw_d[:])
            qb_t = sp.tile([NH, 1], F32)
            nc.sync.dma_start(qb_t[:], qb_d[:].unsqueeze(1))
            kb_t = sp.tile([NH, 1], F32)
            nc.sync.dma_start(kb_t[:], kb_d[:].unsqueeze(1))

            xt = sp.tile([128, 2 * N], F32R)  # X^T: chunk cc at cc*N
            vw_r = sp.tile([128, 128], F32R)
            nc.vector.tensor_copy(vw_r[:], vwsb[:])
            qw_r = sp.tile([F, NH], F32R)
            nc.vector.tensor_copy(qw_r[:], qw_t[:])
            kw_r = sp.tile([F, NH], F32R)
            nc.vector.tensor_copy(kw_r[:], kw_t[:])

            with tc.tile_pool(name="pre_ps", bufs=2, space="PSUM") as xp:
                for t in range(NT):
                    for cc in range(2):
                        tp = xp.tile([128, 128], F32)
                        nc.tensor.transpose(
                            tp[:], xsb[:, t * IN + cc * 128:
                                       t * IN + cc * 128 + 128], ident[:])
                        nc.vector.tensor_copy(
                            xt[:, cc * N + t * 128: cc * N + t * 128 + 128],
                            tp[:])

            with tc.tile_pool(name="vt_ps", bufs=1, space="PSUM") as vpp:
                vt_ps = vpp.tile([F, N], F32)
                for nb in range(4):
                    for cc in range(2):
                        nc.tensor.matmul(
                            vt_ps[:, nb * 512: nb * 512 + 512],
                            vw_r[:, cc * F: cc * F + F],
                            xt[:, cc * N + nb * 512: cc * N + nb * 512 + 512],
                            start=(cc == 0), stop=(cc == 1))
                nc.vector.tensor_scalar_add(vt_sb[:], vt_ps[:], vb_t[:])

            with tc.tile_pool(name="qk_ps", bufs=1, space="PSUM") as qpp:
                qt_ps = qpp.tile([NH, N], F32)
                kt_ps = qpp.tile([NH, N], F32)
                for nb in range(4):
                    nc.tensor.matmul(
                        qt_ps[:, nb * 512: nb * 512 + 512], qw_r[:],
                        vt_sb[:, nb * 512: nb * 512 + 512],
                        start=True, stop=True)
                    nc.tensor.matmul(
                        kt_ps[:, nb * 512: nb * 512 + 512], kw_r[:],
                        vt_sb[:, nb * 512: nb * 512 + 512],
                        start=True, stop=True)
                nc.vector.tensor_scalar_add(qt[:], qt_ps[:], qb_t[:])
                nc.vector.tensor_scalar_add(kt[:], kt_ps[:], kb_t[:])

            # per-head vectors, written [alpha0..3 | beta0..3] then paired
            abq = sp.tile([2 * NH, N], F32R)
            pmq = sp.tile([2 * NH, N], F32R)
            nc.vector.scalar_tensor_tensor(abq[0:NH, :], kt[:], 0.01, kt[:],
                                           ALU.mult, ALU.max)
            nc.vector.scalar_tensor_tensor(abq[NH:2 * NH, :], kt[:], 0.01,
                                           kt[:], ALU.mult, ALU.min)
            nc.vector.tensor_scalar_max(pmq[0:NH, :], qt[:], 0.0)
            nc.vector.tensor_scalar_min(pmq[NH:2 * NH, :], qt[:], 0.0)
            nc.sync.dma_start(
                ab_all[0:1, :].rearrange("o (h n) -> o h n", h=NH),
                abq[0:NH, :].unsqueeze(0))
            nc.sync.dma_start(
                ab_all[1:2, :].rearrange("o (h n) -> o h n", h=NH),
                abq[NH:2 * NH, :].unsqueeze(0))
            nc.sync.dma_start(
                pm_all[0:1, :].rearrange("o (h n) -> o h n", h=NH),
                pmq[0:NH, :].unsqueeze(0))
            nc.sync.dma_start(
                pm_all[1:2, :].rearrange("o (h n) -> o h n", h=NH),
                pmq[NH:2 * NH, :].unsqueeze(0))

        # ---------- Vp1 = [V | 1] per j-tile ----------
        nc.vector.memset(vp1[:].bitcast(F32), 1.0)
        with tc.tile_pool(name="v_ps", bufs=2, space="PSUM") as vp:
            for t in range(NT):
                v_ps = vp.tile([128, F], F32R)
                nc.tensor.transpose(
                    v_ps[:], vt_sb[:, t * 128: t * 128 + 128],
                    id_r[0:F, 0:F])
                nc.vector.tensor_copy(
                    vp1[:, t * (F + 1): t * (F + 1) + F], v_ps[:])

        # ---------- software-pipelined main loop ----------
        S = NB * NT  # 128 steps
        with tc.tile_pool(name="lt_ps", bufs=2, space="PSUM") as ltp, \
             tc.tile_pool(name="acc_ps", bufs=1, space="PSUM") as accp, \
             tc.tile_pool(name="ht_ps", bufs=2, space="PSUM") as htp, \
             tc.tile_pool(name="et_sb", bufs=4) as etp, \
             tc.tile_pool(name="post_sb", bufs=2) as postp:
            lts, ets, accs, hsbs = {}, {}, {}, {}

            def emit_logit(s):
                b, jc = divmod(s, NT)
                h, ib = blocks[b]
                lt = ltp.tile([128, 1024], F32, tag="lt", name=f"lt{s}")
                for hf in range(2):
                    nc.tensor.matmul(
                        lt[:, hf * 512: hf * 512 + 512],
                        ab_all[:, h * N + jc * 128: h * N + jc * 128 + 128],
                        pm_all[:, h * N + ib * 1024 + hf * 512:
                               h * N + ib * 1024 + hf * 512 + 512],
                        start=True, stop=True)
                lts[s] = lt

            def emit_exp(s):
                et = etp.tile([128, 1024], F32R, tag="et", name=f"et{s}")
                nc.scalar.activation(et[:], lts[s][:], AF.Exp)
                ets[s] = et

            def emit_acc(s):
                b, jc = divmod(s, NT)
                if jc == 0:
                    accs[b] = accp.tile([F + 1, 1024], F32, tag="acc", name=f"acc{b}")
                for hf in range(2):
                    nc.tensor.matmul(
                        accs[b][:, hf * 512: hf * 512 + 512],
                        vp1[:, jc * (F + 1): (jc + 1) * (F + 1)],
                        ets[s][:, hf * 512: hf * 512 + 512],
                        start=(jc == 0), stop=(jc == NT - 1))

            def emit_hsb(b):
                hsb = postp.tile([F + 1, 1024], F32, tag="hsb", name=f"hsb{b}")
                nc.vector.tensor_copy(hsb[:], accs[b][:])
                hsbs[b] = hsb

            def emit_trans(b, t8):
                h, ib = blocks[b]
                ht = htp.tile([128, F + 1], F32, tag="ht", name=f"ht{b}_{t8}")
                nc.tensor.transpose(
                    ht[:], hsbs[b][:, t8 * 128: t8 * 128 + 128],
                    id65[:])
                rcp = postp.tile([128, 1], F32, tag="rcp", name=f"rcp{b}_{t8}")
                nc.vector.reciprocal(rcp[:], ht[:, F:F + 1])
                t = ib * 8 + t8
                nc.vector.tensor_scalar_mul(
                    obuf[:, t * (NH * F) + h * F: t * (NH * F) + h * F + F],
                    ht[:, 0:F], rcp[:])

            for s in range(S + 1):
                if s < S:
                    emit_logit(s)
                if s >= 1:
                    emit_exp(s - 1)
                    emit_acc(s - 1)
                    if (s - 1) % NT == NT - 1:
                        emit_hsb((s - 1) // NT)
                    # spread previous block's 8 transposes over this block
                    b_prev = s // NT - 1
                    jc = s % NT
                    if b_prev >= 0 and s < S and jc % 2 == 1:
                        emit_trans(b_prev, jc // 2)
            for t8 in range(8):  # drain last block
                emit_trans(NB - 1, t8)

        nc.sync.dma_start(
            out_d[:].rearrange("(t p) c -> p t c", p=128),
            obuf[:].rearrange("p (t c) -> p t c", t=NT))


def _emit_body(nc, tc, X_d, vW_d, vb_d, qw_d, kw_d, qb_d, kb_d, id_d, out_d):
    if True:
        with tc.tile_pool(name="persist", bufs=1) as pp:
            ident = pp.tile([128, 128], F32)
            nc.sync.dma_start(ident[:], id_d[:])
            id_r = pp.tile([128, 128], F32R)
            nc.vector.tensor_copy(id_r[:], ident[:])
            vt_sb = pp.tile([F, N], F32R)         # V^T, bias folded
            qt = pp.tile([NH, N], F32)
            kt = pp.tile([NH, N], F32)
            ab_hs = [pp.tile([2, N], F32R, name=f"abh{h}", tag=f"ab{h}") for h in range(NH)]
            pm_hs = [pp.tile([2, N], F32R, name=f"pmh{h}", tag=f"pm{h}") for h in range(NH)]
            vp1 = pp.tile([128, NT * (F + 1)], F32R)   # [V | 1] per j-tile

            # ---------- preamble: X^T, V^T, q/k ----------
            with tc.tile_pool(name="pre_sb", bufs=1) as sp:
                xsb = sp.tile([128, NT * IN], F32)
                nc.sync.dma_start(
                    xsb[:].rearrange("p (t c) -> p t c", t=NT),
                    X_d[:].rearrange("(t p) c -> p t c", p=128))
                vwsb = sp.tile([128, 128], F32)
                nc.sync.dma_start(
                    vwsb[:].rearrange("p (t f) -> p t f", t=2),
                    vW_d[:].rearrange("(t p) f -> p t f", p=128))
                vb_t = sp.tile([F, 1], F32)
                nc.sync.dma_start(vb_t[:], vb_d[:].unsqueeze(1))
                qw_t = sp.tile([F, NH], F32)
                nc.sync.dma_start(qw_t[:], qw_d[:])
                kw_t = sp.tile([F, NH], F32)
                nc.sync.dma_start(kw_t[:], kw_d[:])
                qb_t = sp.tile([NH, 1], F32)
                nc.sync.dma_start(qb_t[:], qb_d[:].unsqueeze(1))
                kb_t = sp.tile([NH, 1], F32)
                nc.sync.dma_start(kb_t[:], kb_d[:].unsqueeze(1))

                xt = sp.tile([128, 2 * N], F32R)  # X^T: chunk cc at cc*N
                vw_r = sp.tile([128, 128], F32R)
                nc.vector.tensor_copy(vw_r[:], vwsb[:])
                qw_r = sp.tile([F, NH], F32R)
                nc.vector.tensor_copy(qw_r[:], qw_t[:])
                kw_r = sp.tile([F, NH], F32R)
                nc.vector.tensor_copy(kw_r[:], kw_t[:])

                with tc.tile_pool(name="pre_ps", bufs=2, space="PSUM") as xp:
                    for t in range(NT):
                        for cc in range(2):
                            tp = xp.tile([128, 128], F32)
                            nc.tensor.transpose(
                                tp[:], xsb[:, t * IN + cc * 128:
                                           t * IN + cc * 128 + 128], ident[:])
                            nc.vector.tensor_copy(
                                xt[:, cc * N + t * 128: cc * N + t * 128 + 128],
                                tp[:])

                with tc.tile_pool(name="vt_ps", bufs=1, space="PSUM") as vpp:
                    vt_ps = vpp.tile([F, N], F32)
                    for nb in range(4):
                        for cc in range(2):
                            nc.tensor.matmul(
                                vt_ps[:, nb * 512: nb * 512 + 512],
                                vw_r[:, cc * F: cc * F + F],
                                xt[:, cc * N + nb * 512: cc * N + nb * 512 + 512],
                                start=(cc == 0), stop=(cc == 1))
                    nc.vector.tensor_scalar_add(vt_sb[:], vt_ps[:], vb_t[:])

                with tc.tile_pool(name="qk_ps", bufs=1, space="PSUM") as qpp:
                    qt_ps = qpp.tile([NH, N], F32)
                    kt_ps = qpp.tile([NH, N], F32)
                    for nb in range(4):
                        nc.tensor.matmul(
                            qt_ps[:, nb * 512: nb * 512 + 512], qw_r[:],
                            vt_sb[:, nb * 512: nb * 512 + 512],
                            start=True, stop=True)
                        nc.tensor.matmul(
                            kt_ps[:, nb * 512: nb * 512 + 512], kw_r[:],
                            vt_sb[:, nb * 512: nb * 512 + 512],
                            start=True, stop=True)
                    nc.vector.tensor_scalar_add(qt[:], qt_ps[:], qb_t[:])
                    nc.vector.tensor_scalar_add(kt[:], kt_ps[:], kb_t[:])

            # ---------- per-head vectors (fp32r) ----------
            with tc.tile_pool(name="vec_sb", bufs=1) as vs:
                a4 = vs.tile([NH, N], F32R)
                b4 = vs.tile([NH, N], F32R)
                p4 = vs.tile([NH, N], F32R)
                m4 = vs.tile([NH, N], F32R)
                nc.vector.scalar_tensor_tensor(a4[:], kt[:], 0.01, kt[:],
                                               ALU.mult, ALU.max)
                nc.vector.scalar_tensor_tensor(b4[:], kt[:], 0.01, kt[:],
                                               ALU.mult, ALU.min)
                nc.vector.tensor_scalar_max(p4[:], qt[:], 0.0)
                nc.vector.tensor_scalar_min(m4[:], qt[:], 0.0)
                for h in range(NH):
                    nc.sync.dma_start(ab_hs[h][0:1, :], a4[h:h + 1, :])
                    nc.sync.dma_start(ab_hs[h][1:2, :], b4[h:h + 1, :])
                    nc.sync.dma_start(pm_hs[h][0:1, :], p4[h:h + 1, :])
                    nc.sync.dma_start(pm_hs[h][1:2, :], m4[h:h + 1, :])

            # ---------- Vp1 = [V | 1] per j-tile ----------
            nc.vector.memset(vp1[:].bitcast(F32), 1.0)
            with tc.tile_pool(name="v_ps", bufs=2, space="PSUM") as vp:
                for t in range(NT):
                    v_ps = vp.tile([128, F], F32R)
                    nc.tensor.transpose(
                        v_ps[:], vt_sb[:, t * 128: t * 128 + 128],
                        id_r[0:F, 0:F])
                    nc.vector.tensor_copy(
                        vp1[:, t * (F + 1): t * (F + 1) + F], v_ps[:])

            # ---------- main loop ----------
            hsbs = {}
            with tc.tile_pool(name="lt_ps", bufs=3, space="PSUM") as ltp, \
                 tc.tile_pool(name="acc_ps", bufs=1, space="PSUM") as accp, \
                 tc.tile_pool(name="et_sb", bufs=3) as etp:
                for h in range(NH):
                    ab_h = ab_hs[h][:]
                    pm_h = pm_hs[h][:]
                    for ib in range(2):
                        acc = accp.tile([F + 1, 1024], F32, tag="acc")
                        for jc in range(NT):
                            lt = ltp.tile([128, 1024], F32, tag="lt", name=f"lt{s}")
                            for hf in range(2):
                                nc.tensor.matmul(
                                    lt[:, hf * 512: hf * 512 + 512],
                                    ab_h[:, jc * 128: jc * 128 + 128],
                                    pm_h[:, ib * 1024 + hf * 512:
                                         ib * 1024 + hf * 512 + 512],
                                    start=True, stop=True)
                            et = etp.tile([128, 1024], F32R, tag="et", name=f"et{s}")
                            nc.scalar.activation(et[:], lt[:], AF.Exp)
                            for hf in range(2):
                                nc.tensor.matmul(
                                    acc[:, hf * 512: hf * 512 + 512],
                                    vp1[:, jc * (F + 1): (jc + 1) * (F + 1)],
                                    et[:, hf * 512: hf * 512 + 512],
                                    start=(jc == 0), stop=(jc == NT - 1))
                        hsb = pp.tile([F + 1, 1024], F32, name=f"hsb{h}_{ib}",
                                      tag=f"hsb{h}_{ib}")
                        nc.vector.tensor_copy(hsb[:], acc[:])
                        hsbs[(h, ib)] = hsb

            # ---------- postamble: transpose + normalize + store ----------
            with tc.tile_pool(name="ht_ps", bufs=4, space="PSUM") as htp, \
                 tc.tile_pool(name="post_sb", bufs=4) as postp:
                for h in range(NH):
                    for ib in range(2):
                        hsb = hsbs[(h, ib)]
                        for t8 in range(8):
                            ht = htp.tile([128, F + 1], F32, tag="ht")
                            nc.tensor.transpose(
                                ht[:], hsb[:, t8 * 128: t8 * 128 + 128],
                                ident[0:F + 1, 0:F + 1])
                            rcp = postp.tile([128, 1], F32, tag="rcp", name=f"rcp{b}_{t8}")
                            nc.vector.reciprocal(rcp[:], ht[:, F:F + 1])
                            ob = postp.tile([128, F], F32, tag="ob")
                            nc.vector.tensor_scalar_mul(ob[:], ht[:, 0:F], rcp[:])
                            r0 = ib * 1024 + t8 * 128
                            nc.sync.dma_start(
                                out_d[r0:r0 + 128, h * F: h * F + F], ob[:])


def _get_nc():
    if "nc" not in _CACHE:
        _CACHE["nc"] = build_nc()
    return _CACHE["nc"]


def make_in_maps(X, vW, vb, qW, qb, kW, kb):
    ident = np.eye(128, dtype=np.float32)
    in_maps = []
    for c in range(N_CORES):
        b, h0 = c // 2, NH * (c % 2)
        qwc = np.ascontiguousarray(qW[:, h0:h0 + NH])
        kwc = np.ascontiguousarray(kW[:, h0:h0 + NH])
        qbc = np.ascontiguousarray(qb[h0:h0 + NH])
        kbc = np.ascontiguousarray(kb[h0:h0 + NH])
        prm = np.zeros((128, PRM_COLS), dtype=np.float32)
        prm[:, 0:128] = ident
        prm[:, 128:256] = vW.reshape(2, 128, F).transpose(1, 0, 2).reshape(128, 128)
        prm[0:F, 256] = vb
        prm[0:F, 257:261] = qwc
        prm[0:F, 261:265] = kwc
        prm[0:1, 265:269] = qbc
        prm[0:1, 269:273] = kbc
        in_maps.append({
            "X": np.ascontiguousarray(X[b].T),
            "vW": np.ascontiguousarray(vW),
            "vb": np.ascontiguousarray(vb),
            "qw": qwc,
            "kw": kwc,
            "qb": qbc,
            "kb": kbc,
            "ident": ident,
            "prm": prm,
        })
    return in_maps


def assemble(results):
    full = np.empty((B, N, H * F), dtype=np.float32)
    for c in range(N_CORES):
        b, h0 = c // 2, NH * (c % 2)
        full[b][:, h0 * F:(h0 + NH) * F] = results[c]["out"]
    return full


def kernel(X, vW, vb, qW, qb, kW, kb):
    X, vW, vb = np.asarray(X), np.asarray(vW), np.asarray(vb)
    qW, qb, kW, kb = np.asarray(qW), np.asarray(qb), np.asarray(kW), np.asarray(kb)
    nc = _get_nc()
    res = run_bass_kernel_spmd(nc, make_in_maps(X, vW, vb, qW, qb, kW, kb),
                               list(range(N_CORES)))
    return assemble(res.results)


def _emit_body_v3(nc, tc, X_d, vW_d, vb_d, qw_d, kw_d, qb_d, kb_d, id_d,
                  out_d):
    """v2 main loop + pipelined preamble and per-block output DMAs.

    Preamble works in 4 node-groups of 512: X DMA group g -> 8 PE transposes
    into a [128,512] PSUM tile -> 2 wide copies (DVE/Pool) -> V^T matmul ->
    bias-add -> q/k matmul -> bias-add -> alpha/beta/P/M chunk -> pack DMA.
    First exp can start after group 0's chain (~7us) instead of after the
    whole preamble.  Act engine does exp ONLY (table preloaded at t=0).
    """
    NB = NH * 2
    blocks = [(h, ib) for h in range(NH) for ib in range(2)]
    with tc.tile_pool(name="persist", bufs=1) as pp:
        # Exp activation-table preload, before anything else on Act.
        zz = pp.tile([1, 2], F32R)
        nc.vector.memset(zz[:].bitcast(F32), 0.0)
        nc.scalar.activation(zz[:], zz[:], AF.Exp)

        id65 = pp.tile([F + 1, F + 1], F32)   # identity for postamble transposes
        id_r = pp.tile([128, 128], F32R)
        vt_sb = pp.tile([F, N], F32R)         # V^T, bias folded
        qt = pp.tile([NH, N], F32)
        kt = pp.tile([NH, N], F32)
        ab_all = pp.tile([2, NH * N], F32R)   # row0 alpha, row1 beta
        pm_all = pp.tile([2, NH * N], F32R)   # row0 P, row1 M
        vp1 = pp.tile([128, NT * (F + 1)], F32R)
        obuf = pp.tile([128, NT * NH * F], F32)
        nc.vector.memset(vp1[:].bitcast(F32), 1.0)

        with tc.tile_pool(name="pre_sb", bufs=1) as sp:
            xsb = sp.tile([128, NT * IN], F32)
            vwsb = sp.tile([128, 128], F32)
            vb_t = sp.tile([F, 1], F32)
            qw_t = sp.tile([F, NH], F32)
            kw_t = sp.tile([F, NH], F32)
            qb_t = sp.tile([NH, 1], F32)
            kb_t = sp.tile([NH, 1], F32)
            abq = sp.tile([2 * NH, N], F32R)  # rows 0-3 alpha, 4-7 beta
            pmq = sp.tile([2 * NH, N], F32R)  # rows 0-3 P, 4-7 M

            # input DMAs: ident+vW first (needed by transposes / V^T), then
            # X in 4 groups; small params via other queues.
            nc.sync.dma_start(ident[:], id_d[:])
            nc.sync.dma_start(
                vwsb[:].rearrange("p (t f) -> p t f", t=2),
                vW_d[:].rearrange("(t p) f -> p t f", p=128))
            for g in range(4):
                nc.sync.dma_start(
                    xsb[:, g * 4 * IN:(g + 1) * 4 * IN]
                        .rearrange("p (t c) -> p t c", t=4),
                    X_d[g * 512:(g + 1) * 512, :]
                        .rearrange("(t p) c -> p t c", p=128))
            nc.scalar.dma_start(vb_t[:], vb_d[:].unsqueeze(1))
            nc.scalar.dma_start(qw_t[:], qw_d[:])
            nc.scalar.dma_start(kw_t[:], kw_d[:])
            nc.gpsimd.dma_start(qb_t[:], qb_d[:].unsqueeze(1))
            nc.gpsimd.dma_start(kb_t[:], kb_d[:].unsqueeze(1))

            xt = sp.tile([128, 2 * N], F32R)  # X^T: chunk cc at cc*N
            vw_r = sp.tile([128, 128], F32R)
            nc.vector.tensor_copy(id_r[:], ident[:])
            nc.gpsimd.tensor_copy(vw_r[:], vwsb[:])
            qw_r = sp.tile([F, NH], F32R)
            nc.vector.tensor_copy(qw_r[:], qw_t[:])
            kw_r = sp.tile([F, NH], F32R)
            nc.vector.tensor_copy(kw_r[:], kw_t[:])

            with tc.tile_pool(name="tp_ps", bufs=2, space="PSUM") as xp, \
                 tc.tile_pool(name="vt_ps", bufs=1, space="PSUM") as vpp, \
                 tc.tile_pool(name="qk_ps", bufs=2, space="PSUM") as qpp, \
                 tc.tile_pool(name="v_ps", bufs=1, space="PSUM") as vsp:
                for g in range(4):
                    # X^T for this group's 4 node-tiles (both 128-col chunks)
                    for cc in range(2):
                        tp = xp.tile([128, 512], F32, tag="tp",
                                     name=f"tp{g}_{cc}")
                        for tt in range(4):
                            t = 4 * g + tt
                            nc.tensor.transpose(
                                tp[:, tt * 128: tt * 128 + 128],
                                xsb[:, t * IN + cc * 128:
                                    t * IN + cc * 128 + 128], ident[:])
                        nc.scalar.copy(
                            xt[:, cc * N + g * 512: cc * N + g * 512 + 512],
                            tp[:])
                    # V^T chunk
                    vt_ps = vpp.tile([F, 512], F32, tag="vtps",
                                     name=f"vtps{g}")
                    for cc in range(2):
                        nc.tensor.matmul(
                            vt_ps[:],
                            vw_r[:, cc * F: cc * F + F],
                            xt[:, cc * N + g * 512: cc * N + g * 512 + 512],
                            start=(cc == 0), stop=(cc == 1))
                    nc.vector.tensor_scalar_add(
                        vt_sb[:, g * 512:(g + 1) * 512], vt_ps[:], vb_t[:])
                    # q / k chunks
                    qt_ps = qpp.tile([NH, 512], F32, tag="qk",
                                     name=f"qtps{g}")
                    nc.tensor.matmul(qt_ps[:], qw_r[:],
                                     vt_sb[:, g * 512: g * 512 + 512],
                                     start=True, stop=True)
                    kt_ps = qpp.tile([NH, 512], F32, tag="qk",
                                     name=f"ktps{g}")
                    nc.tensor.matmul(kt_ps[:], kw_r[:],
                                     vt_sb[:, g * 512: g * 512 + 512],
                                     start=True, stop=True)
                    sl = slice(g * 512, (g + 1) * 512)
                    nc.vector.tensor_scalar_add(qt[:, sl], qt_ps[:], qb_t[:])
                    nc.gpsimd.tensor_scalar_add(kt[:, sl], kt_ps[:], kb_t[:])
                    # alpha/beta (from k), P/M (from q) for this chunk
                    nc.vector.scalar_tensor_tensor(
                        abq[0:NH, sl], kt[:, sl], 0.01, kt[:, sl],
                        ALU.mult, ALU.max)
                    nc.gpsimd.scalar_tensor_tensor(
                        abq[NH:2 * NH, sl], kt[:, sl], 0.01, kt[:, sl],
                        ALU.mult, ALU.min)
                    nc.vector.tensor_scalar_max(pmq[0:NH, sl], qt[:, sl], 0.0)
                    nc.gpsimd.tensor_scalar_min(pmq[NH:2 * NH, sl],
                                                qt[:, sl], 0.0)
                    # pack into [2, NH*N] layout (head-major columns)
                    for row in range(2):
                        nc.gpsimd.dma_start(
                            ab_all[row:row + 1, :]
                                .rearrange("o (h n) -> o h n", h=NH)
                                [:, :, g * 512:(g + 1) * 512],
                            abq[row * NH:(row + 1) * NH, sl].unsqueeze(0))
                        nc.gpsimd.dma_start(
                            pm_all[row:row + 1, :]
                                .rearrange("o (h n) -> o h n", h=NH)
                                [:, :, g * 512:(g + 1) * 512],
                            pmq[row * NH:(row + 1) * NH, sl].unsqueeze(0))
                    # Vp1 tiles for this group
                    v_ps = vsp.tile([128, 4 * F], F32R, tag="vps",
                                    name=f"vps{g}")
                    for tt in range(4):
                        t = 4 * g + tt
                        nc.tensor.transpose(
                            v_ps[:, tt * F: tt * F + F],
                            vt_sb[:, t * 128: t * 128 + 128],
                            id_r[0:F, 0:F])
                    eng = nc.vector if g % 2 == 0 else nc.gpsimd
                    eng.tensor_copy(
                        vp1[:].rearrange("p (t c) -> p t c", c=F + 1)
                            [:, 4 * g: 4 * g + 4, 0:F],
                        v_ps[:].rearrange("p (t c) -> p t c", c=F))

        # ---------- software-pipelined main loop ----------
        S = NB * NT  # 128 steps
        with tc.tile_pool(name="lt_ps", bufs=2, space="PSUM") as ltp, \
             tc.tile_pool(name="acc_ps", bufs=1, space="PSUM") as accp, \
             tc.tile_pool(name="ht_ps", bufs=2, space="PSUM") as htp, \
             tc.tile_pool(name="et_sb", bufs=4) as etp, \
             tc.tile_pool(name="post_sb", bufs=2) as postp:
            lts, ets, accs, hsbs = {}, {}, {}, {}

            def emit_logit(s):
                b, jc = divmod(s, NT)
                h, ib = blocks[b]
                lt = ltp.tile([128, 1024], F32, tag="lt", name=f"lt{s}")
                for hf in range(2):
                    nc.tensor.matmul(
                        lt[:, hf * 512: hf * 512 + 512],
                        ab_all[:, h * N + jc * 128: h * N + jc * 128 + 128],
                        pm_all[:, h * N + ib * 1024 + hf * 512:
                               h * N + ib * 1024 + hf * 512 + 512],
                        start=True, stop=True)
                lts[s] = lt

            def emit_exp(s):
                et = etp.tile([128, 1024], F32R, tag="et", name=f"et{s}")
                nc.scalar.activation(et[:], lts[s][:], AF.Exp)
                ets[s] = et

            def emit_acc(s):
                b, jc = divmod(s, NT)
                if jc == 0:
                    accs[b] = accp.tile([F + 1, 1024], F32, tag="acc",
                                        name=f"acc{b}")
                for hf in range(2):
                    nc.tensor.matmul(
                        accs[b][:, hf * 512: hf * 512 + 512],
                        vp1[:, jc * (F + 1): (jc + 1) * (F + 1)],
                        ets[s][:, hf * 512: hf * 512 + 512],
                        start=(jc == 0), stop=(jc == NT - 1))

            def emit_hsb(b):
                hsb = postp.tile([F + 1, 1024], F32, tag="hsb",
                                 name=f"hsb{b}")
                nc.vector.tensor_copy(hsb[:, 0:512], accs[b][:, 0:512])
                nc.gpsimd.tensor_copy(hsb[:, 512:1024], accs[b][:, 512:1024])
                hsbs[b] = hsb

            def emit_trans(b, t8):
                h, ib = blocks[b]
                ht = htp.tile([128, F + 1], F32, tag="ht", name=f"ht{b}_{t8}")
                nc.tensor.transpose(
                    ht[:], hsbs[b][:, t8 * 128: t8 * 128 + 128],
                    id65[:])
                rcp = postp.tile([128, 1], F32, tag="rcp",
                                 name=f"rcp{b}_{t8}")
                nc.vector.reciprocal(rcp[:], ht[:, F:F + 1])
                t = ib * 8 + t8
                nc.vector.tensor_scalar_mul(
                    obuf[:, t * (NH * F) + h * F: t * (NH * F) + h * F + F],
                    ht[:, 0:F], rcp[:])
                if t8 == 7:
                    emit_outdma(b)

            def emit_outdma(b):
                h, ib = blocks[b]
                nc.sync.dma_start(
                    out_d[ib * 1024:(ib + 1) * 1024, h * F:(h + 1) * F]
                        .rearrange("(t p) c -> p t c", p=128),
                    obuf[:].rearrange("p (t c) -> p t c", c=NH * F)
                        [:, ib * 8:(ib + 1) * 8, h * F:(h + 1) * F])

            for s in range(S + 1):
                if s < S:
                    emit_logit(s)
                if s >= 1:
                    emit_exp(s - 1)
                    emit_acc(s - 1)
                    if (s - 1) % NT == NT - 1:
                        emit_hsb((s - 1) // NT)
                    b_prev = s // NT - 1
                    jc = s % NT
                    if b_prev >= 0 and s < S and jc % 2 == 1:
                        emit_trans(b_prev, jc // 2)
            for t8 in range(8):  # drain last block
                emit_trans(NB - 1, t8)


def _emit_body_v4(nc, tc, X_d, vW_d, vb_d, qw_d, kw_d, qb_d, kb_d, id_d,
                  out_d, prm_d=None):
    """v3 + lane-aligned preamble, no per-chunk pack DMAs.

    q/k are produced by matmuls whose lhsT is zero-padded so head h's scalar
    lands (duplicated) on partitions {32h, 32h+1}; q/k biases ride a ones row
    appended to V^T (so the q/k matmul adds them via K=65).  alpha/beta/P/M
    are then single strided DVE/Pool ops straight out of PSUM into the
    matmul-legal [128, N] layouts (alpha_h/P_h at partition 32h, beta_h/M_h
    at 32h+1).  Head 3 (base 96 — illegal for PE) is staged to a [2, N] tile
    by one DMA per tensor at preamble end; its blocks run last.
    """
    NB = NH * 2
    blocks = [(h, ib) for h in range(NH) for ib in range(2)]
    with tc.tile_pool(name="persist", bufs=1) as pp:
        zz = pp.tile([1, 2], F32R)
        nc.vector.memset(zz[:].bitcast(F32), 0.0)
        nc.scalar.activation(zz[:], zz[:], AF.Exp)

        id65 = pp.tile([F + 1, F + 1], F32)   # identity for postamble transposes
        id_r = pp.tile([128, 128], F32R)
        vt1 = pp.tile([F + 1, N], F32R)       # V^T rows 0..63, row 64 = ones
        ab_sp = pp.tile([128, N], F32R)       # part 32h = alpha_h, 32h+1 = beta_h
        pm_sp = pp.tile([128, N], F32R)       # part 32h = P_h, 32h+1 = M_h
        ab3 = pp.tile([2, N], F32R)
        pm3 = pp.tile([2, N], F32R)
        ACDT = mybir.dt.bfloat16 if ACC_BF16 else F32R
        vp1 = pp.tile([128, NT * (F + 1)], ACDT)
        obuf = pp.tile([128, NT * NH * F], F32)
        if ACC_BF16:
            nc.vector.memset(vp1[:], 1.0)
        else:
            nc.vector.memset(vp1[:].bitcast(F32), 1.0)
        nc.vector.memset(vt1[F:F + 1, :].bitcast(F32), 1.0)

        if True:
            sp = pp  # preamble tensors live in the persistent pool: their
            # SBUF never gets recycled under the main loop's et/hsb tiles,
            # so the first exp isn't serialized behind the preamble's tail.
            xt = sp.tile([128, 2 * N], F32R)  # X^T: chunk cc at cc*N

            # One packed-param DMA (ident | vW | vb | qw | kw | qb | kb)
            # then the four X^T groups (host supplies X transposed), all FIFO
            # on the sync HWDGE queue: params land by ~2us, X owns the bus
            # right after, and each 512-node group is immediately matmul-ready
            # (no on-chip transposes).
            prm = sp.tile([128, PRM_COLS], F32)
            nc.sync.dma_start(prm[:], prm_d[:])
            xtr = sp.tile([128, 2 * N], F32)
            for g in range(4):
                if XCAST_DMA:
                    # gpsimd DMA casts f32 -> f32r in flight (the only
                    # engine allowed to), so X^T lands matmul-ready.
                    nc.gpsimd.dma_start(
                        xt[:].rearrange("p (cc n) -> p cc n", cc=2)
                            [:, :, g * 512:(g + 1) * 512],
                        X_d[:].rearrange("(cc p) n -> p cc n", p=128)
                            [:, :, g * 512:(g + 1) * 512])
                else:
                    nc.sync.dma_start(
                        xtr[:].rearrange("p (cc n) -> p cc n", cc=2)
                            [:, :, g * 512:(g + 1) * 512],
                        X_d[:].rearrange("(cc p) n -> p cc n", p=128)
                            [:, :, g * 512:(g + 1) * 512])
                    for cc in range(2):
                        eng = nc.vector if cc == 0 else nc.scalar
                        s0 = cc * N + g * 512
                        if cc == 0:
                            eng.tensor_copy(xt[:, s0:s0 + 512],
                                            xtr[:, s0:s0 + 512])
                        else:
                            eng.copy(xt[:, s0:s0 + 512], xtr[:, s0:s0 + 512])
            ident = prm[:, 0:128]
            vwsb = prm[:, 128:256]
            vb_t = prm[0:F, 256:257]
            qw_t = prm[0:F, 257:261]
            kw_t = prm[0:F, 261:265]
            qb_row = prm[0:1, 265:269]
            kb_row = prm[0:1, 269:273]

            vw_r = sp.tile([128, 128], F32R)
            kscr0 = sp.tile([128, 512], F32)
            kscr1 = sp.tile([128, 512], F32)
            kscr = [kscr0, kscr1]
            nc.vector.tensor_copy(id_r[:], ident[:])
            nc.gpsimd.tensor_copy(id65[:], ident[0:F + 1, 0:F + 1])
            nc.gpsimd.tensor_copy(vw_r[:], vwsb[:])

            # padded q/k lhsT: [65, 128]; rows 0..63 = w dup at {32h,32h+1},
            # row 64 = bias dup there too; zero elsewhere.
            # Padded lhsT columns: even col 32h = +w_h (+bias), odd col
            # 32h+1 = -w_h (-bias).  Odd PSUM lanes then hold -k / -q, so a
            # single full-width max() yields [alpha; -beta] / [P; -M]; the
            # rank-2 logit contraction multiplies the two odd rows together
            # and the negations cancel.
            qkw = sp.tile([F + 1, 128], F32R)
            kkw = sp.tile([F + 1, 128], F32R)
            nc.vector.memset(qkw[:].bitcast(F32), 0.0)
            nc.vector.memset(kkw[:].bitcast(F32), 0.0)
            for rr in range(2):
                sgn = 1.0 if rr == 0 else -1.0
                nc.vector.tensor_scalar_mul(
                    qkw[0:F, :].rearrange("f (h r) -> f h r", r=32)
                        [:, :, rr:rr + 1],
                    qw_t[:].unsqueeze(2), sgn)
                nc.vector.tensor_scalar_mul(
                    qkw[F:F + 1, :].rearrange("o (h r) -> o h r", r=32)
                        [:, :, rr:rr + 1],
                    qb_row[:].unsqueeze(2), sgn)
                nc.vector.tensor_scalar_mul(
                    kkw[0:F, :].rearrange("f (h r) -> f h r", r=32)
                        [:, :, rr:rr + 1],
                    kw_t[:].unsqueeze(2), sgn)
                nc.vector.tensor_scalar_mul(
                    kkw[F:F + 1, :].rearrange("o (h r) -> o h r", r=32)
                        [:, :, rr:rr + 1],
                    kb_row[:].unsqueeze(2), sgn)

            with tc.tile_pool(name="vt_ps", bufs=1, space="PSUM") as vpp, \
                 tc.tile_pool(name="qk_ps", bufs=2, space="PSUM") as qpp, \
                 tc.tile_pool(name="v_ps", bufs=1, space="PSUM") as vsp:
                for g in range(4):
                    sl = slice(g * 512, (g + 1) * 512)
                    vt_ps = vpp.tile([F, 512], F32, tag="vtps",
                                     name=f"vtps{g}")
                    for cc in range(2):
                        nc.tensor.matmul(
                            vt_ps[:],
                            vw_r[:, cc * F: cc * F + F],
                            xt[:, cc * N + g * 512: cc * N + g * 512 + 512],
                            start=(cc == 0), stop=(cc == 1))
                    nc.vector.tensor_scalar_add(vt1[0:F, sl], vt_ps[:],
                                                vb_t[:])
                    qt_ps = qpp.tile([128, 512], F32, tag="qk",
                                     name=f"qtps{g}")
                    nc.tensor.matmul(qt_ps[:], qkw[:], vt1[:, sl],
                                     start=True, stop=True)
                    kt_ps = qpp.tile([128, 512], F32, tag="qk",
                                     name=f"ktps{g}")
                    nc.tensor.matmul(kt_ps[:], kkw[:], vt1[:, sl],
                                     start=True, stop=True)
                    # LeakyReLU with one PSUM read per instruction (the
                    # HW forbids two): Act scales 0.01*k into SBUF scratch,
                    # DVE maxes it against k.  Thanks to the negated odd
                    # lanes this yields [alpha; -beta]; Relu gives [P; -M].
                    nc.scalar.mul(kscr[g % 2][:], kt_ps[:], 0.01)
                    nc.vector.tensor_tensor(
                        ab_sp[:, sl], kscr[g % 2][:], kt_ps[:], ALU.max)
                    nc.scalar.activation(pm_sp[:, sl], qt_ps[:], AF.Relu)
                    v_ps = vsp.tile([128, 4 * F], F32R, tag="vps",
                                    name=f"vps{g}")
                    for tt in range(4):
                        t = 4 * g + tt
                        nc.tensor.transpose(
                            v_ps[:, tt * F: tt * F + F],
                            vt1[0:F, t * 128: t * 128 + 128],
                            id_r[0:F, 0:F])
                    nc.vector.tensor_copy(
                        vp1[:].rearrange("p (t c) -> p t c", c=F + 1)
                            [:, 4 * g: 4 * g + 4, 0:F],
                        v_ps[:].bitcast(F32).rearrange(
                            "p (t c) -> p t c", c=F))
            # head 3 lives at base 96 — stage its pairs to base-0 tiles
            nc.sync.dma_start(ab3[:], ab_sp[96:98, :])
            nc.sync.dma_start(pm3[:], pm_sp[96:98, :])

        # ---------- software-pipelined main loop ----------
        # A shield pool pins the 4 banks the preamble just released, so the
        # first two lt tiles claim the never-used banks 4-7 and the first
        # logits aren't serialized behind the tail of the preamble.
        # Pool creation order fixes PSUM bank assignment (first-fit from
        # bank 0): acc and ht soak up the banks the preamble just released
        # (they are needed later / tolerate the wait), so the lt tiles land
        # on the four never-touched banks and the first logits run as soon
        # as their operands are ready.
        S = NB * NT
        with tc.tile_pool(name="acc_ps", bufs=1, space="PSUM") as accp, \
             tc.tile_pool(name="lt_ps", bufs=3, space="PSUM") as ltp, \
             tc.tile_pool(name="et_sb", bufs=6) as etp, \
             tc.tile_pool(name="post_sb", bufs=2) as postp:
            lts, ets, accs, hsbs = {}, {}, {}, {}

            def abpm(h):
                if h < 3:
                    return (ab_sp[32 * h: 32 * h + 2, :],
                            pm_sp[32 * h: 32 * h + 2, :])
                return ab3[:], pm3[:]

            def emit_logit(s):
                b, jc = divmod(s, NT)
                h, ib = blocks[b]
                ab_h, pm_h = abpm(h)
                lt = ltp.tile([128, 1024], F32, tag="lt", name=f"lt{s}")
                for hf in range(2):
                    nc.tensor.matmul(
                        lt[:, hf * 512: hf * 512 + 512],
                        ab_h[:, jc * 128: jc * 128 + 128],
                        pm_h[:, ib * 1024 + hf * 512:
                             ib * 1024 + hf * 512 + 512],
                        start=True, stop=True)
                lts[s] = lt

            def emit_exp(s):
                et = etp.tile([128, 1024], ACDT, tag="et", name=f"et{s}")
                nc.scalar.activation(et[:], lts[s][:], AF.Exp)
                ets[s] = et

            def emit_acc(s):
                b, jc = divmod(s, NT)
                if jc == 0:
                    accs[b] = accp.tile([F + 1, 1024], F32, tag="acc",
                                        name=f"acc{b}")
                for hf in range(2):
                    nc.tensor.matmul(
                        accs[b][:, hf * 512: hf * 512 + 512],
                        vp1[:, jc * (F + 1): (jc + 1) * (F + 1)],
                        ets[s][:, hf * 512: hf * 512 + 512],
                        start=(jc == 0), stop=(jc == NT - 1))

            def emit_hsb(b):
                hsb = postp.tile([F + 1, 1024], F32, tag="hsb",
                                 name=f"hsb{b}")
                eng = nc.gpsimd if HSB_GP else nc.vector
                eng.tensor_copy(hsb[:, 0:512], accs[b][:, 0:512])
                eng.tensor_copy(hsb[:, 512:1024], accs[b][:, 512:1024])
                hsbs[b] = hsb

            def emit_trans(b, t8):
                h, ib = blocks[b]
                # ht tiles share the lt tag: one 3-deep rotation covers
                # both, freeing two PSUM banks for the deeper lt buffering
                ht = ltp.tile([128, F + 1], F32, tag="lt", name=f"ht{b}_{t8}")
                nc.tensor.transpose(
                    ht[:], hsbs[b][:, t8 * 128: t8 * 128 + 128],
                    id65[:])
                rcp = postp.tile([128, 1], F32, tag="rcp",
                                 name=f"rcp{b}_{t8}")
                nc.vector.reciprocal(rcp[:], ht[:, F:F + 1])
                t = ib * 8 + t8
                nc.vector.tensor_scalar_mul(
                    obuf[:, t * (NH * F) + h * F: t * (NH * F) + h * F + F],
                    ht[:, 0:F], rcp[:])
                if b == NB - 1:
                    if t8 == 3:
                        emit_outdma(b, 0, 4)
                    elif t8 == 7:
                        emit_outdma(b, 4, 8)
                elif t8 == 7:
                    emit_outdma(b, 0, 8)

            def emit_outdma(b, t0, t1):
                h, ib = blocks[b]
                nc.sync.dma_start(
                    out_d[ib * 1024 + t0 * 128: ib * 1024 + t1 * 128,
                          h * F:(h + 1) * F]
                        .rearrange("(t p) c -> p t c", p=128),
                    obuf[:].rearrange("p (t c) -> p t c", c=NH * F)
                        [:, ib * 8 + t0: ib * 8 + t1, h * F:(h + 1) * F])

            if ABLATE >= 1:
                nc.vector.memset(obuf[:], 0.0)
            if ABLATE in (2, 3, 4):
                # pure Act throughput: one logit tile, 128 exps off it.
                # 2: PSUM f32 -> SBUF f32r (the main-loop shape)
                # 3: PSUM f32 -> SBUF bf16
                # 4: SBUF f32 -> SBUF f32r
                emit_logit(0)
                sbsrc = None
                if ABLATE == 4:
                    sbsrc = etp.tile([128, 1024], F32, tag="sbsrc",
                                     name="sbsrc")
                    nc.vector.memset(sbsrc[:], 0.0)
                for s in range(S):
                    odt = mybir.dt.bfloat16 if ABLATE == 3 else F32R
                    et = etp.tile([128, 1024], odt, tag="et", name=f"et{s}")
                    src = sbsrc if ABLATE == 4 else lts[0]
                    nc.scalar.activation(et[:], src[:], AF.Exp)
                for b in range(NB):
                    emit_outdma(b, 0, 8)
                return
            if ABLATE in (7, 8, 9):
                # PE throughput probes, no postamble:
                # 7: acc matmuls only (K=128, M=65, N=512, f32r)
                # 8: logit matmuls only (K=2, M=128, N=512, f32r)
                # 9: acc matmuls only in bf16
                if ABLATE in (7, 9):
                    dt = mybir.dt.bfloat16 if ABLATE == 9 else F32R
                    et0 = etp.tile([128, 1024], dt, tag="et", name="et0")
                    if ABLATE == 9:
                        nc.vector.memset(et0[:], 1.0)
                        vp1b = etp.tile([128, NT * (F + 1)], dt, tag="vpb",
                                        name="vp1b")
                        nc.vector.tensor_copy(vp1b[:], vp1[:].bitcast(F32))
                        vsrc = vp1b
                    else:
                        nc.vector.memset(et0[:].bitcast(F32), 1.0)
                        vsrc = vp1
                    for s in range(S):
                        b, jc = divmod(s, NT)
                        if jc == 0:
                            accs[b] = accp.tile([F + 1, 1024], F32,
                                                tag="acc", name=f"acc{b}")
                        for hf in range(2):
                            nc.tensor.matmul(
                                accs[b][:, hf * 512: hf * 512 + 512],
                                vsrc[:, jc * (F + 1): (jc + 1) * (F + 1)],
                                et0[:, hf * 512: hf * 512 + 512],
                                start=(jc == 0), stop=(jc == NT - 1))
                else:
                    for s in range(S):
                        emit_logit(s)
                for b in range(NB):
                    emit_outdma(b, 0, 8)
                return
            if ABLATE == 10:
                # preamble + output DMA only
                for b in range(NB):
                    emit_outdma(b, 0, 8)
                return
            if ABLATE == 11:
                # ABL=6 with L/A emission batched in pairs (fewer PE
                # logit<->acc switches), accs ahead of logits in the queue
                et0 = etp.tile([128, 1024], F32R, tag="et", name="et0")
                nc.vector.memset(et0[:].bitcast(F32), 1.0)
                for s in range(S + 2):
                    if s % 2 == 0:
                        for a in (s - 2, s - 1):
                            if 0 <= a < S:
                                emit_acc(a)
                                if a % NT == NT - 1:
                                    emit_hsb(a // NT)
                                b_prev = a // NT - 1
                                jc = a % NT
                                if b_prev >= 0 and jc % 2 == 1:
                                    emit_trans(b_prev, jc // 2)
                        if s < S:
                            ets[s] = et0
                            ets[s + 1] = et0
                            emit_logit(s)
                            emit_logit(s + 1)
                for t8 in range(8):
                    emit_trans(NB - 1, t8)
                return
            if ABLATE == 13:
                # acc-only but every matmul uses a different vp1 chunk
                # (forces a weight change per matmul)
                et0 = etp.tile([128, 1024], F32R, tag="et", name="et0")
                nc.vector.memset(et0[:].bitcast(F32), 1.0)
                for s in range(S):
                    b, jc = divmod(s, NT)
                    if jc == 0:
                        accs[b] = accp.tile([F + 1, 1024], F32, tag="acc",
                                            name=f"acc{b}")
                    for hf in range(2):
                        w = ((jc + 8 * hf) % NT) * (F + 1)
                        nc.tensor.matmul(
                            accs[b][:, hf * 512: hf * 512 + 512],
                            vp1[:, w: w + F + 1],
                            et0[:, hf * 512: hf * 512 + 512],
                            start=(jc == 0), stop=(jc == NT - 1))
                for b in range(NB):
                    emit_outdma(b, 0, 8)
                return
            if ABLATE == 14:
                # logit-only, ONE [128,512] matmul per step (half the work
                # of ABL=8) — isolates per-instruction overhead
                for s in range(S):
                    b, jc = divmod(s, NT)
                    h, ib = blocks[b]
                    ab_h, pm_h = abpm(h)
                    lt = ltp.tile([128, 512], F32, tag="lt", name=f"lt{s}")
                    nc.tensor.matmul(
                        lt[:], ab_h[:, jc * 128: jc * 128 + 128],
                        pm_h[:, ib * 1024: ib * 1024 + 512],
                        start=True, stop=True)
                for b in range(NB):
                    emit_outdma(b, 0, 8)
                return
            if ABLATE == 15:
                # logit-only in bf16 (tests weight-load cost by dtype)
                BF16 = mybir.dt.bfloat16
                abb = etp.tile([128, N], BF16, tag="abb", name="abb")
                pmb = etp.tile([128, N], BF16, tag="pmb", name="pmb")
                nc.vector.tensor_copy(abb[:], ab_sp[:].bitcast(F32))
                nc.vector.tensor_copy(pmb[:], pm_sp[:].bitcast(F32))
                for s in range(S):
                    b, jc = divmod(s, NT)
                    h, ib = blocks[b]
                    h2 = min(h, 2)
                    lt = ltp.tile([128, 1024], F32, tag="lt", name=f"lt{s}")
                    for hf in range(2):
                        nc.tensor.matmul(
                            lt[:, hf * 512: hf * 512 + 512],
                            abb[32 * h2: 32 * h2 + 2,
                                jc * 128: jc * 128 + 128],
                            pmb[32 * h2: 32 * h2 + 2,
                                ib * 1024 + hf * 512:
                                ib * 1024 + hf * 512 + 512],
                            start=True, stop=True)
                for b in range(NB):
                    emit_outdma(b, 0, 8)
                return
            if ABLATE == 6:
                # full pipeline minus Act: logits + acc + postamble, with a
                # constant ones tile standing in for every exp result.
                et0 = etp.tile([128, 1024], F32R, tag="et", name="et0")
                nc.vector.memset(et0[:].bitcast(F32), 1.0)
                for s in range(S + 3):
                    if s < S:
                        emit_logit(s)
                    if 1 <= s <= S:
                        ets[s - 1] = et0
                    if s >= 3:
                        a = s - 3
                        emit_acc(a)
                        if a % NT == NT - 1:
                            emit_hsb(a // NT)
                        b_prev = a // NT - 1
                        jc = a % NT
                        if b_prev >= 0 and jc % 2 == 1:
                            emit_trans(b_prev, jc // 2)
                for t8 in range(8):
                    emit_trans(NB - 1, t8)
                return
            for s in range(S + 3):
                if s < S:
                    emit_logit(s)
                if 1 <= s <= S:
                    emit_exp(s - 1)
                if s >= 3 and ABLATE != 1:
                    a = s - 3
                    emit_acc(a)
                    if a % NT == NT - 1:
                        emit_hsb(a // NT)
                    b_prev = a // NT - 1
                    jc = a % NT
                    if b_prev >= 0 and jc % 2 == 1:
                        emit_trans(b_prev, jc // 2)
            if ABLATE != 1:
                for t8 in range(8):
                    emit_trans(NB - 1, t8)
            else:
                for b in range(NB):
                    emit_outdma(b, 0, 8)



# revision 40
# speedup vs baseline: 1.6986x; 1.6789x over previous
"""Multi-head graph attention (rank-2 LeakyReLU-softmax) Trainium2 kernel.

Reference computation (per batch b, head h):
    V = X @ vW + vb                       (N, F)
    q = V @ qW[:,h] + qb[h]               (N,)   per-node scalar
    k = V @ kW[:,h] + kb[h]               (N,)
    A_ij = softmax_j( LeakyReLU(q_i * k_j) )
    out[b,i,h,:] = sum_j A_ij V_j

Key identity used here: with P = max(q,0), M = min(q,0),
alpha = LeakyReLU(k) = max(k, 0.01k), beta = min(k, 0.01k),
    LeakyReLU(q_i * k_j) == alpha_j * P_i + beta_j * M_i      (exactly)
since for each i exactly one of P_i / M_i is nonzero.  So the N x N logit
matrix is a rank-2 outer product, built on the TensorEngine as a K=2
matmul (fp32r), exponentiated on the ScalarEngine straight out of PSUM,
and contracted against [V | 1] without the N x N matrix ever leaving the
chip.  The trailing all-ones column of Vp1 yields the softmax denominator
as row 64 of the same accumulation.

Sharding: core c -> batch b = c//2, heads h0 = 4*(c%2) .. h0+3.
"""

import numpy as np

import concourse.bacc as bacc
import concourse.tile as tile
import concourse.mybir as mybir
from concourse.bass_utils import run_bass_kernel_spmd

B, N, IN, F, H = 4, 2048, 256, 64, 8
NH = H // 2          # heads per core
NT = N // 128        # 16 i-tiles / j-chunks
F32 = mybir.dt.float32
F32R = mybir.dt.float32r
AF = mybir.ActivationFunctionType
ALU = mybir.AluOpType

N_CORES = 8
# packed param tensor columns: ident(128) | vW 2 chunks(128) | vb(1) | qw(4)
# | kw(4) | qb(4) | kb(4)
PRM_COLS = 128 + 128 + 1 + 4 + 4 + 4 + 4
_CACHE = {}
XCAST_DMA = False
import os as _os
ABLATE = int(_os.environ.get("ABL", "0"))  # 1: no acc/postamble (timing probe)
ACC_BF16 = int(_os.environ.get("ACCBF", "1"))  # bf16 vp1/et for the acc matmul
HSB_GP = int(_os.environ.get("HSBGP", "0"))    # gpsimd can't read PSUM (walrus)
LT_BUFS = int(_os.environ.get("LTBUFS", "2"))  # lt PSUM rotation depth
LOGIT8 = int(_os.environ.get("LOGIT8", "1"))   # bf16 hi/lo K=8-per-head logits
ACC_BUFS = int(_os.environ.get("ACCBUFS", "1"))  # acc PSUM tiles
HT_POOL = int(_os.environ.get("HTPOOL", "1"))  # 1: ht transposes own PSUM pool
HSB_SPLIT = int(_os.environ.get("HSBSPLIT", "0"))  # 1: hsb copy DVE+Act split


def build_nc(reps=1, unroll=False, version=4):
    """Build the kernel program.

    reps > 1 wraps the whole computation in a hardware For_i loop (all-engine
    barrier between iterations) so test.py can measure per-execution HW time
    by slope: (t(R) - t(1)) / (R - 1).  The graded kernel() path uses reps=1.
    """
    nc = bacc.Bacc("TRN2", target_bir_lowering=False, debug=False,
                   num_devices=N_CORES)
    xshape = [IN, N] if version >= 4 else [N, IN]
    X_d = nc.dram_tensor("X", xshape,
                         F32R if version >= 4 else F32,
                         kind="ExternalInput")
    vW_d = nc.dram_tensor("vW", [IN, F], F32, kind="ExternalInput")
    vb_d = nc.dram_tensor("vb", [F], F32, kind="ExternalInput")
    qw_d = nc.dram_tensor("qw", [F, NH], F32, kind="ExternalInput")
    kw_d = nc.dram_tensor("kw", [F, NH], F32, kind="ExternalInput")
    qb_d = nc.dram_tensor("qb", [NH], F32, kind="ExternalInput")
    kb_d = nc.dram_tensor("kb", [NH], F32, kind="ExternalInput")
    id_d = nc.dram_tensor("ident", [128, 128], F32, kind="ExternalInput")
    prm_d = nc.dram_tensor("prm", [128, PRM_COLS], F32, kind="ExternalInput")
    out_d = nc.dram_tensor("out", [N, NH * F], F32, kind="ExternalOutput")

    body = {1: _emit_body, 2: _emit_body_v2, 3: _emit_body_v3,
            4: _emit_body_v4}[version]
    extra = {"prm_d": prm_d} if version >= 4 else {}
    with tile.TileContext(nc) as tc:
        from contextlib import ExitStack
        with ExitStack() as rep_ctx:
            if reps > 1 and not unroll:
                rep_ctx.enter_context(tc.For_i(0, reps))
            for _ in range(reps if unroll else 1):
                body(nc, tc, X_d, vW_d, vb_d, qw_d, kw_d, qb_d, kb_d,
                     id_d, out_d, **extra)
    nc.compile()
    return nc


def _emit_body_v2(nc, tc, X_d, vW_d, vb_d, qw_d, kw_d, qb_d, kb_d, id_d,
                  out_d):
    """Software-pipelined main loop.

    Per (head, i-block) "block" (NB = NH*2 of them), per j-chunk step:
      PE:  logit matmul (K=2 rank-2 outer product) -> lt PSUM [128,1024]
      Act: exp straight out of PSUM -> et SBUF (the ONLY Act work)
      PE:  acc matmul [V|1]^T @ et -> acc PSUM [65,1024] accumulated over 16 j
    Steps are emitted with a 1-step skew (logit(s) before acc(s-1)) so PE's
    in-order queue never parks an exp-dependent acc in front of independent
    logit work.  Postamble (PE transpose + DVE normalize into an SBUF staging
    buffer) is interleaved into the following block's steps; output leaves the
    chip in one final DMA.
    """
    NB = NH * 2
    blocks = [(h, ib) for h in range(NH) for ib in range(2)]
    with tc.tile_pool(name="persist", bufs=1) as pp:
        ident = pp.tile([128, 128], F32)
        nc.sync.dma_start(ident[:], id_d[:])
        id_r = pp.tile([128, 128], F32R)
        nc.vector.tensor_copy(id_r[:], ident[:])
        vt_sb = pp.tile([F, N], F32R)         # V^T, bias folded
        qt = pp.tile([NH, N], F32)
        kt = pp.tile([NH, N], F32)
        ab_all = pp.tile([2, NH * N], F32R)   # row0 alpha, row1 beta; head h at cols h*N
        pm_all = pp.tile([2, NH * N], F32R)   # row0 P, row1 M
        vp1 = pp.tile([128, NT * (F + 1)], F32R)   # [V | 1] per j-tile
        obuf = pp.tile([128, NT * NH * F], F32)    # staged output

        # ---------- preamble: X^T, V^T, q/k ----------
        with tc.tile_pool(name="pre_sb", bufs=1) as sp:
            xsb = sp.tile([128, NT * IN], F32)
            nc.sync.dma_start(
                xsb[:].rearrange("p (t c) -> p t c", t=NT),
                X_d[:].rearrange("(t p) c -> p t c", p=128))
            vwsb = sp.tile([128, 128], F32)
            nc.sync.dma_start(
                vwsb[:].rearrange("p (t f) -> p t f", t=2),
                vW_d[:].rearrange("(t p) f -> p t f", p=128))
            vb_t = sp.tile([F, 1], F32)
            nc.sync.dma_start(vb_t[:], vb_d[:].unsqueeze(1))
            qw_t = sp.tile([F, NH], F32)
            nc.sync.dma_start(qw_t[:], qw_d[:])
            kw_t = sp.tile([F, NH], F32)
            nc.sync.dma_start(kw_t[:], kw_d[:])
            qb_t = sp.tile([NH, 1], F32)
            nc.sync.dma_start(qb_t[:], qb_d[:].unsqueeze(1))
            kb_t = sp.tile([NH, 1], F32)
            nc.sync.dma_start(kb_t[:], kb_d[:].unsqueeze(1))

            xt = sp.tile([128, 2 * N], F32R)  # X^T: chunk cc at cc*N
            vw_r = sp.tile([128, 128], F32R)
            nc.vector.tensor_copy(vw_r[:], vwsb[:])
            qw_r = sp.tile([F, NH], F32R)
            nc.vector.tensor_copy(qw_r[:], qw_t[:])
            kw_r = sp.tile([F, NH], F32R)
            nc.vector.tensor_copy(kw_r[:], kw_t[:])

            with tc.tile_pool(name="pre_ps", bufs=2, space="PSUM") as xp:
                for t in range(NT):
                    for cc in range(2):
                        tp = xp.tile([128, 128], F32)
                        nc.tensor.transpose(
                            tp[:], xsb[:, t * IN + cc * 128:
                                       t * IN + cc * 128 + 128], ident[:])
                        nc.vector.tensor_copy(
                            xt[:, cc * N + t * 128: cc * N + t * 128 + 128],
                            tp[:])

            with tc.tile_pool(name="vt_ps", bufs=1, space="PSUM") as vpp:
                vt_ps = vpp.tile([F, N], F32)
                for nb in range(4):
                    for cc in range(2):
                        nc.tensor.matmul(
                            vt_ps[:, nb * 512: nb * 512 + 512],
                            vw_r[:, cc * F: cc * F + F],
                            xt[:, cc * N + nb * 512: cc * N + nb * 512 + 512],
                            start=(cc == 0), stop=(cc == 1))
                nc.vector.tensor_scalar_add(vt_sb[:], vt_ps[:], vb_t[:])

            with tc.tile_pool(name="qk_ps", bufs=1, space="PSUM") as qpp:
                qt_ps = qpp.tile([NH, N], F32)
                kt_ps = qpp.tile([NH, N], F32)
                for nb in range(4):
                    nc.tensor.matmul(
                        qt_ps[:, nb * 512: nb * 512 + 512], qw_r[:],
                        vt_sb[:, nb * 512: nb * 512 + 512],
                        start=True, stop=True)
                    nc.tensor.matmul(
                        kt_ps[:, nb * 512: nb * 512 + 512], kw_r[:],
                        vt_sb[:, nb * 512: nb * 512 + 512],
                        start=True, stop=True)
                nc.vector.tensor_scalar_add(qt[:], qt_ps[:], qb_t[:])
                nc.vector.tensor_scalar_add(kt[:], kt_ps[:], kb_t[:])

            # per-head vectors, written [alpha0..3 | beta0..3] then paired
            abq = sp.tile([2 * NH, N], F32R)
            pmq = sp.tile([2 * NH, N], F32R)
            nc.vector.scalar_tensor_tensor(abq[0:NH, :], kt[:], 0.01, kt[:],
                                           ALU.mult, ALU.max)
            nc.vector.scalar_tensor_tensor(abq[NH:2 * NH, :], kt[:], 0.01,
                                           kt[:], ALU.mult, ALU.min)
            nc.vector.tensor_scalar_max(pmq[0:NH, :], qt[:], 0.0)
            nc.vector.tensor_scalar_min(pmq[NH:2 * NH, :], qt[:], 0.0)
            nc.sync.dma_start(
                ab_all[0:1, :].rearrange("o (h n) -> o h n", h=NH),
                abq[0:NH, :].unsqueeze(0))
            nc.sync.dma_start(
                ab_all[1:2, :].rearrange("o (h n) -> o h n", h=NH),
                abq[NH:2 * NH, :].unsqueeze(0))
            nc.sync.dma_start(
                pm_all[0:1, :].rearrange("o (h n) -> o h n", h=NH),
                pmq[0:NH, :].unsqueeze(0))
            nc.sync.dma_start(
                pm_all[1:2, :].rearrange("o (h n) -> o h n", h=NH),
                pmq[NH:2 * NH, :].unsqueeze(0))

        # ---------- Vp1 = [V | 1] per j-tile ----------
        nc.vector.memset(vp1[:].bitcast(F32), 1.0)
        with tc.tile_pool(name="v_ps", bufs=2, space="PSUM") as vp:
            for t in range(NT):
                v_ps = vp.tile([128, F], F32R)
                nc.tensor.transpose(
                    v_ps[:], vt_sb[:, t * 128: t * 128 + 128],
                    id_r[0:F, 0:F])
                nc.vector.tensor_copy(
                    vp1[:, t * (F + 1): t * (F + 1) + F], v_ps[:])

        # ---------- software-pipelined main loop ----------
        S = NB * NT  # 128 steps
        with tc.tile_pool(name="lt_ps", bufs=2, space="PSUM") as ltp, \
             tc.tile_pool(name="acc_ps", bufs=1, space="PSUM") as accp, \
             tc.tile_pool(name="ht_ps", bufs=2, space="PSUM") as htp, \
             tc.tile_pool(name="et_sb", bufs=4) as etp, \
             tc.tile_pool(name="post_sb", bufs=2) as postp:
            lts, ets, accs, hsbs = {}, {}, {}, {}

            def emit_logit(s):
                b, jc = divmod(s, NT)
                h, ib = blocks[b]
                lt = ltp.tile([128, 1024], F32, tag="lt", name=f"lt{s}")
                for hf in range(2):
                    nc.tensor.matmul(
                        lt[:, hf * 512: hf * 512 + 512],
                        ab_all[:, h * N + jc * 128: h * N + jc * 128 + 128],
                        pm_all[:, h * N + ib * 1024 + hf * 512:
                               h * N + ib * 1024 + hf * 512 + 512],
                        start=True, stop=True)
                lts[s] = lt

            def emit_exp(s):
                et = etp.tile([128, 1024], F32R, tag="et", name=f"et{s}")
                nc.scalar.activation(et[:], lts[s][:], AF.Exp)
                ets[s] = et

            def emit_acc(s):
                b, jc = divmod(s, NT)
                if jc == 0:
                    accs[b] = accp.tile([F + 1, 1024], F32, tag="acc", name=f"acc{b}")
                for hf in range(2):
                    nc.tensor.matmul(
                        accs[b][:, hf * 512: hf * 512 + 512],
                        vp1[:, jc * (F + 1): (jc + 1) * (F + 1)],
                        ets[s][:, hf * 512: hf * 512 + 512],
                        start=(jc == 0), stop=(jc == NT - 1))

            def emit_hsb(b):
                hsb = postp.tile([F + 1, 1024], F32, tag="hsb", name=f"hsb{b}")
                nc.vector.tensor_copy(hsb[:], accs[b][:])
                hsbs[b] = hsb

            def emit_trans(b, t8):
                h, ib = blocks[b]
                ht = htp.tile([128, F + 1], F32, tag="ht", name=f"ht{b}_{t8}")
                nc.tensor.transpose(
                    ht[:], hsbs[b][:, t8 * 128: t8 * 128 + 128],
                    id65[:])
                rcp = postp.tile([128, 1], F32, tag="rcp", name=f"rcp{b}_{t8}")
                nc.vector.reciprocal(rcp[:], ht[:, F:F + 1])
                t = ib * 8 + t8
                nc.vector.tensor_scalar_mul(
                    obuf[:, t * (NH * F) + h * F: t * (NH * F) + h * F + F],
                    ht[:, 0:F], rcp[:])

            for s in range(S + 1):
                if s < S:
                    emit_logit(s)
                if s >= 1:
                    emit_exp(s - 1)
                    emit_acc(s - 1)
                    if (s - 1) % NT == NT - 1:
                        emit_hsb((s - 1) // NT)
                    # spread previous block's 8 transposes over this block
                    b_prev = s // NT - 1
                    jc = s % NT
                    if b_prev >= 0 and s < S and jc % 2 == 1:
                        emit_trans(b_prev, jc // 2)
            for t8 in range(8):  # drain last block
                emit_trans(NB - 1, t8)

        nc.sync.dma_start(
            out_d[:].rearrange("(t p) c -> p t c", p=128),
            obuf[:].rearrange("p (t c) -> p t c", t=NT))


def _emit_body(nc, tc, X_d, vW_d, vb_d, qw_d, kw_d, qb_d, kb_d, id_d, out_d):
    if True:
        with tc.tile_pool(name="persist", bufs=1) as pp:
            ident = pp.tile([128, 128], F32)
            nc.sync.dma_start(ident[:], id_d[:])
            id_r = pp.tile([128, 128], F32R)
            nc.vector.tensor_copy(id_r[:], ident[:])
            vt_sb = pp.tile([F, N], F32R)         # V^T, bias folded
            qt = pp.tile([NH, N], F32)
            kt = pp.tile([NH, N], F32)
            ab_hs = [pp.tile([2, N], F32R, name=f"abh{h}", tag=f"ab{h}") for h in range(NH)]
            pm_hs = [pp.tile([2, N], F32R, name=f"pmh{h}", tag=f"pm{h}") for h in range(NH)]
            vp1 = pp.tile([128, NT * (F + 1)], F32R)   # [V | 1] per j-tile

            # ---------- preamble: X^T, V^T, q/k ----------
            with tc.tile_pool(name="pre_sb", bufs=1) as sp:
                xsb = sp.tile([128, NT * IN], F32)
                nc.sync.dma_start(
                    xsb[:].rearrange("p (t c) -> p t c", t=NT),
                    X_d[:].rearrange("(t p) c -> p t c", p=128))
                vwsb = sp.tile([128, 128], F32)
                nc.sync.dma_start(
                    vwsb[:].rearrange("p (t f) -> p t f", t=2),
                    vW_d[:].rearrange("(t p) f -> p t f", p=128))
                vb_t = sp.tile([F, 1], F32)
                nc.sync.dma_start(vb_t[:], vb_d[:].unsqueeze(1))
                qw_t = sp.tile([F, NH], F32)
                nc.sync.dma_start(qw_t[:], qw_d[:])
                kw_t = sp.tile([F, NH], F32)
                nc.sync.dma_start(kw_t[:], kw_d[:])
                qb_t = sp.tile([NH, 1], F32)
                nc.sync.dma_start(qb_t[:], qb_d[:].unsqueeze(1))
                kb_t = sp.tile([NH, 1], F32)
                nc.sync.dma_start(kb_t[:], kb_d[:].unsqueeze(1))

                xt = sp.tile([128, 2 * N], F32R)  # X^T: chunk cc at cc*N
                vw_r = sp.tile([128, 128], F32R)
                nc.vector.tensor_copy(vw_r[:], vwsb[:])
                qw_r = sp.tile([F, NH], F32R)
                nc.vector.tensor_copy(qw_r[:], qw_t[:])
                kw_r = sp.tile([F, NH], F32R)
                nc.vector.tensor_copy(kw_r[:], kw_t[:])

                with tc.tile_pool(name="pre_ps", bufs=2, space="PSUM") as xp:
                    for t in range(NT):
                        for cc in range(2):
                            tp = xp.tile([128, 128], F32)
                            nc.tensor.transpose(
                                tp[:], xsb[:, t * IN + cc * 128:
                                           t * IN + cc * 128 + 128], ident[:])
                            nc.vector.tensor_copy(
                                xt[:, cc * N + t * 128: cc * N + t * 128 + 128],
                                tp[:])

                with tc.tile_pool(name="vt_ps", bufs=1, space="PSUM") as vpp:
                    vt_ps = vpp.tile([F, N], F32)
                    for nb in range(4):
                        for cc in range(2):
                            nc.tensor.matmul(
                                vt_ps[:, nb * 512: nb * 512 + 512],
                                vw_r[:, cc * F: cc * F + F],
                                xt[:, cc * N + nb * 512: cc * N + nb * 512 + 512],
                                start=(cc == 0), stop=(cc == 1))
                    nc.vector.tensor_scalar_add(vt_sb[:], vt_ps[:], vb_t[:])

                with tc.tile_pool(name="qk_ps", bufs=1, space="PSUM") as qpp:
                    qt_ps = qpp.tile([NH, N], F32)
                    kt_ps = qpp.tile([NH, N], F32)
                    for nb in range(4):
                        nc.tensor.matmul(
                            qt_ps[:, nb * 512: nb * 512 + 512], qw_r[:],
                            vt_sb[:, nb * 512: nb * 512 + 512],
                            start=True, stop=True)
                        nc.tensor.matmul(
                            kt_ps[:, nb * 512: nb * 512 + 512], kw_r[:],
                            vt_sb[:, nb * 512: nb * 512 + 512],
                            start=True, stop=True)
                    nc.vector.tensor_scalar_add(qt[:], qt_ps[:], qb_t[:])
                    nc.vector.tensor_scalar_add(kt[:], kt_ps[:], kb_t[:])

            # ---------- per-head vectors (fp32r) ----------
            with tc.tile_pool(name="vec_sb", bufs=1) as vs:
                a4 = vs.tile([NH, N], F32R)
                b4 = vs.tile([NH, N], F32R)
                p4 = vs.tile([NH, N], F32R)
                m4 = vs.tile([NH, N], F32R)
                nc.vector.scalar_tensor_tensor(a4[:], kt[:], 0.01, kt[:],
                                               ALU.mult, ALU.max)
                nc.vector.scalar_tensor_tensor(b4[:], kt[:], 0.01, kt[:],
                                               ALU.mult, ALU.min)
                nc.vector.tensor_scalar_max(p4[:], qt[:], 0.0)
                nc.vector.tensor_scalar_min(m4[:], qt[:], 0.0)
                for h in range(NH):
                    nc.sync.dma_start(ab_hs[h][0:1, :], a4[h:h + 1, :])
                    nc.sync.dma_start(ab_hs[h][1:2, :], b4[h:h + 1, :])
                    nc.sync.dma_start(pm_hs[h][0:1, :], p4[h:h + 1, :])
                    nc.sync.dma_start(pm_hs[h][1:2, :], m4[h:h + 1, :])

            # ---------- Vp1 = [V | 1] per j-tile ----------
            nc.vector.memset(vp1[:].bitcast(F32), 1.0)
            with tc.tile_pool(name="v_ps", bufs=2, space="PSUM") as vp:
                for t in range(NT):
                    v_ps = vp.tile([128, F], F32R)
                    nc.tensor.transpose(
                        v_ps[:], vt_sb[:, t * 128: t * 128 + 128],
                        id_r[0:F, 0:F])
                    nc.vector.tensor_copy(
                        vp1[:, t * (F + 1): t * (F + 1) + F], v_ps[:])

            # ---------- main loop ----------
            hsbs = {}
            with tc.tile_pool(name="lt_ps", bufs=3, space="PSUM") as ltp, \
                 tc.tile_pool(name="acc_ps", bufs=1, space="PSUM") as accp, \
                 tc.tile_pool(name="et_sb", bufs=3) as etp:
                for h in range(NH):
                    ab_h = ab_hs[h][:]
                    pm_h = pm_hs[h][:]
                    for ib in range(2):
                        acc = accp.tile([F + 1, 1024], F32, tag="acc")
                        for jc in range(NT):
                            lt = ltp.tile([128, 1024], F32, tag="lt", name=f"lt{s}")
                            for hf in range(2):
                                nc.tensor.matmul(
                                    lt[:, hf * 512: hf * 512 + 512],
                                    ab_h[:, jc * 128: jc * 128 + 128],
                                    pm_h[:, ib * 1024 + hf * 512:
                                         ib * 1024 + hf * 512 + 512],
                                    start=True, stop=True)
                            et = etp.tile([128, 1024], F32R, tag="et", name=f"et{s}")
                            nc.scalar.activation(et[:], lt[:], AF.Exp)
                            for hf in range(2):
                                nc.tensor.matmul(
                                    acc[:, hf * 512: hf * 512 + 512],
                                    vp1[:, jc * (F + 1): (jc + 1) * (F + 1)],
                                    et[:, hf * 512: hf * 512 + 512],
                                    start=(jc == 0), stop=(jc == NT - 1))
                        hsb = pp.tile([F + 1, 1024], F32, name=f"hsb{h}_{ib}",
                                      tag=f"hsb{h}_{ib}")
                        nc.vector.tensor_copy(hsb[:], acc[:])
                        hsbs[(h, ib)] = hsb

            # ---------- postamble: transpose + normalize + store ----------
            with tc.tile_pool(name="ht_ps", bufs=4, space="PSUM") as htp, \
                 tc.tile_pool(name="post_sb", bufs=4) as postp:
                for h in range(NH):
                    for ib in range(2):
                        hsb = hsbs[(h, ib)]
                        for t8 in range(8):
                            ht = htp.tile([128, F + 1], F32, tag="ht")
                            nc.tensor.transpose(
                                ht[:], hsb[:, t8 * 128: t8 * 128 + 128],
                                ident[0:F + 1, 0:F + 1])
                            rcp = postp.tile([128, 1], F32, tag="rcp", name=f"rcp{b}_{t8}")
                            nc.vector.reciprocal(rcp[:], ht[:, F:F + 1])
                            ob = postp.tile([128, F], F32, tag="ob")
                            nc.vector.tensor_scalar_mul(ob[:], ht[:, 0:F], rcp[:])
                            r0 = ib * 1024 + t8 * 128
                            nc.sync.dma_start(
                                out_d[r0:r0 + 128, h * F: h * F + F], ob[:])


def _get_nc():
    if "nc" not in _CACHE:
        _CACHE["nc"] = build_nc()
    return _CACHE["nc"]


def make_in_maps(X, vW, vb, qW, qb, kW, kb):
    ident = np.eye(128, dtype=np.float32)
    in_maps = []
    for c in range(N_CORES):
        b, h0 = c // 2, NH * (c % 2)
        qwc = np.ascontiguousarray(qW[:, h0:h0 + NH])
        kwc = np.ascontiguousarray(kW[:, h0:h0 + NH])
        qbc = np.ascontiguousarray(qb[h0:h0 + NH])
        kbc = np.ascontiguousarray(kb[h0:h0 + NH])
        prm = np.zeros((128, PRM_COLS), dtype=np.float32)
        prm[:, 0:128] = ident
        prm[:, 128:256] = vW.reshape(2, 128, F).transpose(1, 0, 2).reshape(128, 128)
        prm[0:F, 256] = vb
        prm[0:F, 257:261] = qwc
        prm[0:F, 261:265] = kwc
        prm[0:1, 265:269] = qbc
        prm[0:1, 269:273] = kbc
        in_maps.append({
            "X": np.ascontiguousarray(X[b].T),
            "vW": np.ascontiguousarray(vW),
            "vb": np.ascontiguousarray(vb),
            "qw": qwc,
            "kw": kwc,
            "qb": qbc,
            "kb": kbc,
            "ident": ident,
            "prm": prm,
        })
    return in_maps


def assemble(results):
    full = np.empty((B, N, H * F), dtype=np.float32)
    for c in range(N_CORES):
        b, h0 = c // 2, NH * (c % 2)
        full[b][:, h0 * F:(h0 + NH) * F] = results[c]["out"]
    return full


def kernel(X, vW, vb, qW, qb, kW, kb):
    X, vW, vb = np.asarray(X), np.asarray(vW), np.asarray(vb)
    qW, qb, kW, kb = np.asarray(qW), np.asarray(qb), np.asarray(kW), np.asarray(kb)
    nc = _get_nc()
    res = run_bass_kernel_spmd(nc, make_in_maps(X, vW, vb, qW, qb, kW, kb),
                               list(range(N_CORES)))
    return assemble(res.results)


def _emit_body_v3(nc, tc, X_d, vW_d, vb_d, qw_d, kw_d, qb_d, kb_d, id_d,
                  out_d):
    """v2 main loop + pipelined preamble and per-block output DMAs.

    Preamble works in 4 node-groups of 512: X DMA group g -> 8 PE transposes
    into a [128,512] PSUM tile -> 2 wide copies (DVE/Pool) -> V^T matmul ->
    bias-add -> q/k matmul -> bias-add -> alpha/beta/P/M chunk -> pack DMA.
    First exp can start after group 0's chain (~7us) instead of after the
    whole preamble.  Act engine does exp ONLY (table preloaded at t=0).
    """
    NB = NH * 2
    blocks = [(h, ib) for h in range(NH) for ib in range(2)]
    with tc.tile_pool(name="persist", bufs=1) as pp:
        # Exp activation-table preload, before anything else on Act.
        zz = pp.tile([1, 2], F32R)
        nc.vector.memset(zz[:].bitcast(F32), 0.0)
        nc.scalar.activation(zz[:], zz[:], AF.Exp)

        id65 = pp.tile([F + 1, F + 1], F32)   # identity for postamble transposes
        id_r = pp.tile([128, 128], F32R)
        vt_sb = pp.tile([F, N], F32R)         # V^T, bias folded
        qt = pp.tile([NH, N], F32)
        kt = pp.tile([NH, N], F32)
        ab_all = pp.tile([2, NH * N], F32R)   # row0 alpha, row1 beta
        pm_all = pp.tile([2, NH * N], F32R)   # row0 P, row1 M
        vp1 = pp.tile([128, NT * (F + 1)], F32R)
        obuf = pp.tile([128, NT * NH * F], F32)
        nc.vector.memset(vp1[:].bitcast(F32), 1.0)

        with tc.tile_pool(name="pre_sb", bufs=1) as sp:
            xsb = sp.tile([128, NT * IN], F32)
            vwsb = sp.tile([128, 128], F32)
            vb_t = sp.tile([F, 1], F32)
            qw_t = sp.tile([F, NH], F32)
            kw_t = sp.tile([F, NH], F32)
            qb_t = sp.tile([NH, 1], F32)
            kb_t = sp.tile([NH, 1], F32)
            abq = sp.tile([2 * NH, N], F32R)  # rows 0-3 alpha, 4-7 beta
            pmq = sp.tile([2 * NH, N], F32R)  # rows 0-3 P, 4-7 M

            # input DMAs: ident+vW first (needed by transposes / V^T), then
            # X in 4 groups; small params via other queues.
            nc.sync.dma_start(ident[:], id_d[:])
            nc.sync.dma_start(
                vwsb[:].rearrange("p (t f) -> p t f", t=2),
                vW_d[:].rearrange("(t p) f -> p t f", p=128))
            for g in range(4):
                nc.sync.dma_start(
                    xsb[:, g * 4 * IN:(g + 1) * 4 * IN]
                        .rearrange("p (t c) -> p t c", t=4),
                    X_d[g * 512:(g + 1) * 512, :]
                        .rearrange("(t p) c -> p t c", p=128))
            nc.scalar.dma_start(vb_t[:], vb_d[:].unsqueeze(1))
            nc.scalar.dma_start(qw_t[:], qw_d[:])
            nc.scalar.dma_start(kw_t[:], kw_d[:])
            nc.gpsimd.dma_start(qb_t[:], qb_d[:].unsqueeze(1))
            nc.gpsimd.dma_start(kb_t[:], kb_d[:].unsqueeze(1))

            xt = sp.tile([128, 2 * N], F32R)  # X^T: chunk cc at cc*N
            vw_r = sp.tile([128, 128], F32R)
            nc.vector.tensor_copy(id_r[:], ident[:])
            nc.gpsimd.tensor_copy(vw_r[:], vwsb[:])
            qw_r = sp.tile([F, NH], F32R)
            nc.vector.tensor_copy(qw_r[:], qw_t[:])
            kw_r = sp.tile([F, NH], F32R)
            nc.vector.tensor_copy(kw_r[:], kw_t[:])

            with tc.tile_pool(name="tp_ps", bufs=2, space="PSUM") as xp, \
                 tc.tile_pool(name="vt_ps", bufs=1, space="PSUM") as vpp, \
                 tc.tile_pool(name="qk_ps", bufs=2, space="PSUM") as qpp, \
                 tc.tile_pool(name="v_ps", bufs=1, space="PSUM") as vsp:
                for g in range(4):
                    # X^T for this group's 4 node-tiles (both 128-col chunks)
                    for cc in range(2):
                        tp = xp.tile([128, 512], F32, tag="tp",
                                     name=f"tp{g}_{cc}")
                        for tt in range(4):
                            t = 4 * g + tt
                            nc.tensor.transpose(
                                tp[:, tt * 128: tt * 128 + 128],
                                xsb[:, t * IN + cc * 128:
                                    t * IN + cc * 128 + 128], ident[:])
                        nc.scalar.copy(
                            xt[:, cc * N + g * 512: cc * N + g * 512 + 512],
                            tp[:])
                    # V^T chunk
                    vt_ps = vpp.tile([F, 512], F32, tag="vtps",
                                     name=f"vtps{g}")
                    for cc in range(2):
                        nc.tensor.matmul(
                            vt_ps[:],
                            vw_r[:, cc * F: cc * F + F],
                            xt[:, cc * N + g * 512: cc * N + g * 512 + 512],
                            start=(cc == 0), stop=(cc == 1))
                    nc.vector.tensor_scalar_add(
                        vt_sb[:, g * 512:(g + 1) * 512], vt_ps[:], vb_t[:])
                    # q / k chunks
                    qt_ps = qpp.tile([NH, 512], F32, tag="qk",
                                     name=f"qtps{g}")
                    nc.tensor.matmul(qt_ps[:], qw_r[:],
                                     vt_sb[:, g * 512: g * 512 + 512],
                                     start=True, stop=True)
                    kt_ps = qpp.tile([NH, 512], F32, tag="qk",
                                     name=f"ktps{g}")
                    nc.tensor.matmul(kt_ps[:], kw_r[:],
                                     vt_sb[:, g * 512: g * 512 + 512],
                                     start=True, stop=True)
                    sl = slice(g * 512, (g + 1) * 512)
                    nc.vector.tensor_scalar_add(qt[:, sl], qt_ps[:], qb_t[:])
                    nc.gpsimd.tensor_scalar_add(kt[:, sl], kt_ps[:], kb_t[:])
                    # alpha/beta (from k), P/M (from q) for this chunk
                    nc.vector.scalar_tensor_tensor(
                        abq[0:NH, sl], kt[:, sl], 0.01, kt[:, sl],
                        ALU.mult, ALU.max)
                    nc.gpsimd.scalar_tensor_tensor(
                        abq[NH:2 * NH, sl], kt[:, sl], 0.01, kt[:, sl],
                        ALU.mult, ALU.min)
                    nc.vector.tensor_scalar_max(pmq[0:NH, sl], qt[:, sl], 0.0)
                    nc.gpsimd.tensor_scalar_min(pmq[NH:2 * NH, sl],
                                                qt[:, sl], 0.0)
                    # pack into [2, NH*N] layout (head-major columns)
                    for row in range(2):
                        nc.gpsimd.dma_start(
                            ab_all[row:row + 1, :]
                                .rearrange("o (h n) -> o h n", h=NH)
                                [:, :, g * 512:(g + 1) * 512],
                            abq[row * NH:(row + 1) * NH, sl].unsqueeze(0))
                        nc.gpsimd.dma_start(
                            pm_all[row:row + 1, :]
                                .rearrange("o (h n) -> o h n", h=NH)
                                [:, :, g * 512:(g + 1) * 512],
                            pmq[row * NH:(row + 1) * NH, sl].unsqueeze(0))
                    # Vp1 tiles for this group
                    v_ps = vsp.tile([128, 4 * F], F32R, tag="vps",
                                    name=f"vps{g}")
                    for tt in range(4):
                        t = 4 * g + tt
                        nc.tensor.transpose(
                            v_ps[:, tt * F: tt * F + F],
                            vt_sb[:, t * 128: t * 128 + 128],
                            id_r[0:F, 0:F])
                    eng = nc.vector if g % 2 == 0 else nc.gpsimd
                    eng.tensor_copy(
                        vp1[:].rearrange("p (t c) -> p t c", c=F + 1)
                            [:, 4 * g: 4 * g + 4, 0:F],
                        v_ps[:].rearrange("p (t c) -> p t c", c=F))

        # ---------- software-pipelined main loop ----------
        S = NB * NT  # 128 steps
        with tc.tile_pool(name="lt_ps", bufs=2, space="PSUM") as ltp, \
             tc.tile_pool(name="acc_ps", bufs=1, space="PSUM") as accp, \
             tc.tile_pool(name="ht_ps", bufs=2, space="PSUM") as htp, \
             tc.tile_pool(name="et_sb", bufs=4) as etp, \
             tc.tile_pool(name="post_sb", bufs=2) as postp:
            lts, ets, accs, hsbs = {}, {}, {}, {}

            def emit_logit(s):
                b, jc = divmod(s, NT)
                h, ib = blocks[b]
                lt = ltp.tile([128, 1024], F32, tag="lt", name=f"lt{s}")
                for hf in range(2):
                    nc.tensor.matmul(
                        lt[:, hf * 512: hf * 512 + 512],
                        ab_all[:, h * N + jc * 128: h * N + jc * 128 + 128],
                        pm_all[:, h * N + ib * 1024 + hf * 512:
                               h * N + ib * 1024 + hf * 512 + 512],
                        start=True, stop=True)
                lts[s] = lt

            def emit_exp(s):
                et = etp.tile([128, 1024], F32R, tag="et", name=f"et{s}")
                nc.scalar.activation(et[:], lts[s][:], AF.Exp)
                ets[s] = et

            def emit_acc(s):
                b, jc = divmod(s, NT)
                if jc == 0:
                    accs[b] = accp.tile([F + 1, 1024], F32, tag="acc",
                                        name=f"acc{b}")
                for hf in range(2):
                    nc.tensor.matmul(
                        accs[b][:, hf * 512: hf * 512 + 512],
                        vp1[:, jc * (F + 1): (jc + 1) * (F + 1)],
                        ets[s][:, hf * 512: hf * 512 + 512],
                        start=(jc == 0), stop=(jc == NT - 1))

            def emit_hsb(b):
                hsb = postp.tile([F + 1, 1024], F32, tag="hsb",
                                 name=f"hsb{b}")
                nc.vector.tensor_copy(hsb[:, 0:512], accs[b][:, 0:512])
                nc.gpsimd.tensor_copy(hsb[:, 512:1024], accs[b][:, 512:1024])
                hsbs[b] = hsb

            def emit_trans(b, t8):
                h, ib = blocks[b]
                ht = htp.tile([128, F + 1], F32, tag="ht", name=f"ht{b}_{t8}")
                nc.tensor.transpose(
                    ht[:], hsbs[b][:, t8 * 128: t8 * 128 + 128],
                    id65[:])
                rcp = postp.tile([128, 1], F32, tag="rcp",
                                 name=f"rcp{b}_{t8}")
                nc.vector.reciprocal(rcp[:], ht[:, F:F + 1])
                t = ib * 8 + t8
                nc.vector.tensor_scalar_mul(
                    obuf[:, t * (NH * F) + h * F: t * (NH * F) + h * F + F],
                    ht[:, 0:F], rcp[:])
                if t8 == 7:
                    emit_outdma(b)

            def emit_outdma(b):
                h, ib = blocks[b]
                nc.sync.dma_start(
                    out_d[ib * 1024:(ib + 1) * 1024, h * F:(h + 1) * F]
                        .rearrange("(t p) c -> p t c", p=128),
                    obuf[:].rearrange("p (t c) -> p t c", c=NH * F)
                        [:, ib * 8:(ib + 1) * 8, h * F:(h + 1) * F])

            for s in range(S + 1):
                if s < S:
                    emit_logit(s)
                if s >= 1:
                    emit_exp(s - 1)
                    emit_acc(s - 1)
                    if (s - 1) % NT == NT - 1:
                        emit_hsb((s - 1) // NT)
                    b_prev = s // NT - 1
                    jc = s % NT
                    if b_prev >= 0 and s < S and jc % 2 == 1:
                        emit_trans(b_prev, jc // 2)
            for t8 in range(8):  # drain last block
                emit_trans(NB - 1, t8)


def _emit_body_v4(nc, tc, X_d, vW_d, vb_d, qw_d, kw_d, qb_d, kb_d, id_d,
                  out_d, prm_d=None):
    """v3 + lane-aligned preamble, no per-chunk pack DMAs.

    q/k are produced by matmuls whose lhsT is zero-padded so head h's scalar
    lands (duplicated) on partitions {32h, 32h+1}; q/k biases ride a ones row
    appended to V^T (so the q/k matmul adds them via K=65).  alpha/beta/P/M
    are then single strided DVE/Pool ops straight out of PSUM into the
    matmul-legal [128, N] layouts (alpha_h/P_h at partition 32h, beta_h/M_h
    at 32h+1).  Head 3 (base 96 — illegal for PE) is staged to a [2, N] tile
    by one DMA per tensor at preamble end; its blocks run last.
    """
    NB = NH * 2
    blocks = [(h, ib) for h in range(NH) for ib in range(2)]
    with tc.tile_pool(name="persist", bufs=1) as pp:
        zz = pp.tile([1, 2], F32R)
        nc.vector.memset(zz[:].bitcast(F32), 0.0)
        nc.scalar.activation(zz[:], zz[:], AF.Exp)

        id65 = pp.tile([F + 1, F + 1], F32)   # identity for postamble transposes
        id_r = pp.tile([128, 128], F32R)
        vt1 = pp.tile([F + 1, N], F32R)       # V^T rows 0..63, row 64 = ones
        if not LOGIT8:
            ab_sp = pp.tile([128, N], F32R, tag="ab_sp")  # 32h = alpha_h
            pm_sp = pp.tile([128, N], F32R, tag="pm_sp")  # 32h = P_h
        # Per-head zero-padded alpha/beta weights: K=128 logit lhsT so every
        # main-loop matmul contracts over all 128 partitions (avoids PE
        # row-group reconfig between K=2 logits and K=128 accs).  Rows
        # 32h/32h+1 hold alpha_h/-beta_h, everything else stays zero; the
        # full pm_sp rides along as rhs since zero lhsT rows null out the
        # other heads.
        BF16 = mybir.dt.bfloat16
        F16 = mybir.dt.float16
        if LOGIT8:
            # fp16 alpha/beta/P/M: 11-bit mantissa keeps exp(logit) rounding
            # at ~3e-3 overall (bf16's 8-bit would be ~2e-2) and 2-byte
            # 128-col weights make the logit matmul FWL-eligible.  Same row
            # layout as the f32r path: head h at rows 32h/32h+1, zeros
            # elsewhere in the per-head lhsT tiles.
            abp = [pp.tile([128, N], F16, name=f"abp{h}", tag=f"abp{h}")
                   for h in range(NH)]
            pm8 = pp.tile([128, N], F16, tag="pm8")
            abh16 = pp.tile([128, N], F16, tag="abh16")
            for h in range(NH):
                eng = nc.vector if h % 2 == 0 else nc.gpsimd
                eng.memset(abp[h][:], 0.0)
        else:
            abp = [pp.tile([128, N], F32R, name=f"abp{h}") for h in range(NH)]
            for h in range(NH):
                eng = nc.vector if h % 2 == 0 else nc.gpsimd
                eng.memset(abp[h][:].bitcast(F32), 0.0)
        ACDT = mybir.dt.bfloat16 if ACC_BF16 else F32R
        vp1 = pp.tile([128, NT * (F + 1)], ACDT)
        obuf = pp.tile([128, NT * NH * F], F32)
        if ACC_BF16:
            nc.vector.memset(vp1[:], 1.0)
        else:
            nc.vector.memset(vp1[:].bitcast(F32), 1.0)
        nc.vector.memset(vt1[F:F + 1, :].bitcast(F32), 1.0)

        if True:
            sp = pp  # preamble tensors live in the persistent pool: their
            # SBUF never gets recycled under the main loop's et/hsb tiles,
            # so the first exp isn't serialized behind the preamble's tail.
            xt = sp.tile([128, 2 * N], F32R)  # X^T: chunk cc at cc*N

            # One packed-param DMA (ident | vW | vb | qw | kw | qb | kb)
            # then the four X^T groups (host supplies X transposed), all FIFO
            # on the sync HWDGE queue: params land by ~2us, X owns the bus
            # right after, and each 512-node group is immediately matmul-ready
            # (no on-chip transposes).
            prm = sp.tile([128, PRM_COLS], F32)
            nc.scalar.dma_start(prm[:], prm_d[:])
            # X is declared f32r in DRAM (same bits as f32), so each group
            # DMAs straight into the matmul-ready X^T tile; two HWDGE queues
            # split the 2 MB transfer.
            for g in range(4):
                dq = nc.sync if g < 2 else nc.scalar
                dq.dma_start(
                    xt[:].rearrange("p (cc n) -> p cc n", cc=2)
                        [:, :, g * 512:(g + 1) * 512],
                    X_d[:].rearrange("(cc p) n -> p cc n", p=128)
                        [:, :, g * 512:(g + 1) * 512])
            ident = prm[:, 0:128]
            vwsb = prm[:, 128:256]
            vb_t = prm[0:F, 256:257]
            qw_t = prm[0:F, 257:261]
            kw_t = prm[0:F, 261:265]
            qb_row = prm[0:1, 265:269]
            kb_row = prm[0:1, 269:273]

            vw_r = sp.tile([128, 128], F32R)
            kscr0 = sp.tile([128, 512], F32)
            kscr1 = sp.tile([128, 512], F32)
            kscr = [kscr0, kscr1]
            nc.vector.tensor_copy(id_r[:], ident[:])
            nc.gpsimd.tensor_copy(id65[:], ident[0:F + 1, 0:F + 1])
            nc.gpsimd.tensor_copy(vw_r[:], vwsb[:])

            # padded q/k lhsT: [65, 128]; rows 0..63 = w dup at {32h,32h+1},
            # row 64 = bias dup there too; zero elsewhere.
            # Padded lhsT columns: even col 32h = +w_h (+bias), odd col
            # 32h+1 = -w_h (-bias).  Odd PSUM lanes then hold -k / -q, so a
            # single full-width max() yields [alpha; -beta] / [P; -M]; the
            # rank-2 logit contraction multiplies the two odd rows together
            # and the negations cancel.
            qkw = sp.tile([F + 1, 128], F32R)
            kkw = sp.tile([F + 1, 128], F32R)
            nc.vector.memset(qkw[:].bitcast(F32), 0.0)
            nc.vector.memset(kkw[:].bitcast(F32), 0.0)
            for rr in range(2):
                sgn = 1.0 if rr == 0 else -1.0
                nc.vector.tensor_scalar_mul(
                    qkw[0:F, :].rearrange("f (h r) -> f h r", r=32)
                        [:, :, rr:rr + 1],
                    qw_t[:].unsqueeze(2), sgn)
                nc.vector.tensor_scalar_mul(
                    qkw[F:F + 1, :].rearrange("o (h r) -> o h r", r=32)
                        [:, :, rr:rr + 1],
                    qb_row[:].unsqueeze(2), sgn)
                nc.vector.tensor_scalar_mul(
                    kkw[0:F, :].rearrange("f (h r) -> f h r", r=32)
                        [:, :, rr:rr + 1],
                    kw_t[:].unsqueeze(2), sgn)
                nc.vector.tensor_scalar_mul(
                    kkw[F:F + 1, :].rearrange("o (h r) -> o h r", r=32)
                        [:, :, rr:rr + 1],
                    kb_row[:].unsqueeze(2), sgn)

            with tc.tile_pool(name="vt_ps", bufs=1, space="PSUM") as vpp, \
                 tc.tile_pool(name="qk_ps", bufs=2, space="PSUM") as qpp, \
                 tc.tile_pool(name="v_ps", bufs=1, space="PSUM") as vsp:
                for g in range(4):
                    sl = slice(g * 512, (g + 1) * 512)
                    vt_ps = vpp.tile([F, 512], F32, tag="vtps",
                                     name=f"vtps{g}")
                    for cc in range(2):
                        nc.tensor.matmul(
                            vt_ps[:],
                            vw_r[:, cc * F: cc * F + F],
                            xt[:, cc * N + g * 512: cc * N + g * 512 + 512],
                            start=(cc == 0), stop=(cc == 1))
                    nc.vector.tensor_scalar_add(vt1[0:F, sl], vt_ps[:],
                                                vb_t[:])
                    qt_ps = qpp.tile([128, 512], F32, tag="qk",
                                     name=f"qtps{g}")
                    nc.tensor.matmul(qt_ps[:], qkw[:], vt1[:, sl],
                                     start=True, stop=True)
                    kt_ps = qpp.tile([128, 512], F32, tag="qk",
                                     name=f"ktps{g}")
                    nc.tensor.matmul(kt_ps[:], kkw[:], vt1[:, sl],
                                     start=True, stop=True)
                    # LeakyReLU with one PSUM read per instruction (the
                    # HW forbids two): Act scales 0.01*k into SBUF scratch,
                    # DVE maxes it against k.  Thanks to the negated odd
                    # lanes this yields [alpha; -beta]; Relu gives [P; -M].
                    if LOGIT8:
                        # fp16 conversion folded into the producing ops
                        nc.scalar.mul(kscr[g % 2][:], kt_ps[:], 0.01)
                        nc.vector.tensor_tensor(
                            abh16[:, sl], kscr[g % 2][:], kt_ps[:], ALU.max)
                        nc.scalar.activation(pm8[:, sl], qt_ps[:], AF.Relu)
                    else:
                        nc.scalar.mul(kscr[g % 2][:], kt_ps[:], 0.01)
                        nc.vector.tensor_tensor(
                            ab_sp[:, sl], kscr[g % 2][:], kt_ps[:], ALU.max)
                        nc.scalar.activation(pm_sp[:, sl], qt_ps[:], AF.Relu)

                    v_ps = vsp.tile([128, 4 * F], F32R, tag="vps",
                                    name=f"vps{g}")
                    for tt in range(4):
                        t = 4 * g + tt
                        nc.tensor.transpose(
                            v_ps[:, tt * F: tt * F + F],
                            vt1[0:F, t * 128: t * 128 + 128],
                            id_r[0:F, 0:F])
                    nc.vector.tensor_copy(
                        vp1[:].rearrange("p (t c) -> p t c", c=F + 1)
                            [:, 4 * g: 4 * g + 4, 0:F],
                        v_ps[:].bitcast(F32).rearrange(
                            "p (t c) -> p t c", c=F))
            # scatter each head's alpha/-beta pair into its padded K=128 lhsT
            if LOGIT8:
                dq = [nc.sync, nc.scalar]
                for h in range(NH):
                    dq[h % 2].dma_start(abp[h][32 * h: 32 * h + 2, :],
                                        abh16[32 * h: 32 * h + 2, :])
            else:
                for h in range(NH):
                    nc.sync.dma_start(abp[h][32 * h: 32 * h + 2, :],
                                      ab_sp[32 * h: 32 * h + 2, :])

        # ---------- software-pipelined main loop ----------
        # A shield pool pins the 4 banks the preamble just released, so the
        # first two lt tiles claim the never-used banks 4-7 and the first
        # logits aren't serialized behind the tail of the preamble.
        # Pool creation order fixes PSUM bank assignment (first-fit from
        # bank 0): acc and ht soak up the banks the preamble just released
        # (they are needed later / tolerate the wait), so the lt tiles land
        # on the four never-touched banks and the first logits run as soon
        # as their operands are ready.
        S = NB * NT
        from contextlib import ExitStack
        with ExitStack() as mstk:
            accp = mstk.enter_context(
                tc.tile_pool(name="acc_ps", bufs=ACC_BUFS, space="PSUM"))
            ltp = mstk.enter_context(
                tc.tile_pool(name="lt_ps", bufs=LT_BUFS, space="PSUM"))
            htp = mstk.enter_context(
                tc.tile_pool(name="ht_ps", bufs=2, space="PSUM")) \
                if HT_POOL else ltp
            etp = mstk.enter_context(tc.tile_pool(name="et_sb", bufs=6))
            postp = mstk.enter_context(tc.tile_pool(name="post_sb", bufs=2))
            lts, ets, accs, hsbs = {}, {}, {}, {}

            def abpm(h):
                return abp[h][:], (pm8[:] if LOGIT8 else pm_sp[:])

            def emit_logit(s):
                b, jc = divmod(s, NT)
                h, ib = blocks[b]
                ab_h, pm_h = abpm(h)
                lt = ltp.tile([128, 1024], F32, tag="lt", name=f"lt{s}")
                for hf in range(2):
                    nc.tensor.matmul(
                        lt[:, hf * 512: hf * 512 + 512],
                        ab_h[:, jc * 128: jc * 128 + 128],
                        pm_h[:, ib * 1024 + hf * 512:
                             ib * 1024 + hf * 512 + 512],
                        start=True, stop=True)
                lts[s] = lt

            def emit_exp(s):
                et = etp.tile([128, 1024], ACDT, tag="et", name=f"et{s}")
                nc.scalar.activation(et[:], lts[s][:], AF.Exp)
                ets[s] = et

            def emit_acc(s):
                b, jc = divmod(s, NT)
                if jc == 0:
                    accs[b] = accp.tile([F + 1, 1024], F32, tag="acc",
                                        name=f"acc{b}")
                for hf in range(2):
                    nc.tensor.matmul(
                        accs[b][:, hf * 512: hf * 512 + 512],
                        vp1[:, jc * (F + 1): (jc + 1) * (F + 1)],
                        ets[s][:, hf * 512: hf * 512 + 512],
                        start=(jc == 0), stop=(jc == NT - 1))

            def emit_hsb(b):
                hsb = postp.tile([F + 1, 1024], F32, tag="hsb",
                                 name=f"hsb{b}")
                if HSB_SPLIT:
                    nc.vector.tensor_copy(hsb[:, 0:512], accs[b][:, 0:512])
                    nc.scalar.copy(hsb[:, 512:1024], accs[b][:, 512:1024])
                else:
                    nc.vector.tensor_copy(hsb[:, 0:512], accs[b][:, 0:512])
                    nc.vector.tensor_copy(hsb[:, 512:1024],
                                          accs[b][:, 512:1024])
                hsbs[b] = hsb

            def emit_trans(b, t8):
                h, ib = blocks[b]
                # ht tiles default to sharing the lt tag (one rotation covers
                # both); HT_POOL gives them their own 2-bank pool instead
                ht = htp.tile([128, F + 1], F32,
                              tag=("ht" if HT_POOL else "lt"),
                              name=f"ht{b}_{t8}")
                nc.tensor.transpose(
                    ht[:], hsbs[b][:, t8 * 128: t8 * 128 + 128],
                    id65[:])
                rcp = postp.tile([128, 1], F32, tag="rcp",
                                 name=f"rcp{b}_{t8}")
                nc.vector.reciprocal(rcp[:], ht[:, F:F + 1])
                t = ib * 8 + t8
                nc.vector.tensor_scalar_mul(
                    obuf[:, t * (NH * F) + h * F: t * (NH * F) + h * F + F],
                    ht[:, 0:F], rcp[:])
                if b == NB - 1:
                    if t8 == 3:
                        emit_outdma(b, 0, 4)
                    elif t8 == 7:
                        emit_outdma(b, 4, 8)
                elif t8 == 7:
                    emit_outdma(b, 0, 8)

            def emit_outdma(b, t0, t1):
                h, ib = blocks[b]
                nc.sync.dma_start(
                    out_d[ib * 1024 + t0 * 128: ib * 1024 + t1 * 128,
                          h * F:(h + 1) * F]
                        .rearrange("(t p) c -> p t c", p=128),
                    obuf[:].rearrange("p (t c) -> p t c", c=NH * F)
                        [:, ib * 8 + t0: ib * 8 + t1, h * F:(h + 1) * F])

            if ABLATE >= 1:
                nc.vector.memset(obuf[:], 0.0)
            if ABLATE == 17:
                # full logit+exp+acc pipeline, postamble skipped
                for s in range(S + 3):
                    if s < S:
                        emit_logit(s)
                    if 1 <= s <= S:
                        emit_exp(s - 1)
                    if s >= 3:
                        emit_acc(s - 3)
                for b in range(NB):
                    emit_outdma(b, 0, 8)
                return
            if ABLATE in (2, 3, 4):
                # pure Act throughput: one logit tile, 128 exps off it.
                # 2: PSUM f32 -> SBUF f32r (the main-loop shape)
                # 3: PSUM f32 -> SBUF bf16
                # 4: SBUF f32 -> SBUF f32r
                emit_logit(0)
                sbsrc = None
                if ABLATE == 4:
                    sbsrc = etp.tile([128, 1024], F32, tag="sbsrc",
                                     name="sbsrc")
                    nc.vector.memset(sbsrc[:], 0.0)
                for s in range(S):
                    odt = mybir.dt.bfloat16 if ABLATE == 3 else F32R
                    et = etp.tile([128, 1024], odt, tag="et", name=f"et{s}")
                    src = sbsrc if ABLATE == 4 else lts[0]
                    nc.scalar.activation(et[:], src[:], AF.Exp)
                for b in range(NB):
                    emit_outdma(b, 0, 8)
                return
            if ABLATE in (7, 8, 9):
                # PE throughput probes, no postamble:
                # 7: acc matmuls only (K=128, M=65, N=512, f32r)
                # 8: logit matmuls only (K=2, M=128, N=512, f32r)
                # 9: acc matmuls only in bf16
                if ABLATE in (7, 9):
                    dt = mybir.dt.bfloat16 if ABLATE == 9 else F32R
                    et0 = etp.tile([128, 1024], dt, tag="et", name="et0")
                    if ABLATE == 9:
                        nc.vector.memset(et0[:], 1.0)
                        vp1b = etp.tile([128, NT * (F + 1)], dt, tag="vpb",
                                        name="vp1b")
                        nc.vector.tensor_copy(vp1b[:], vp1[:].bitcast(F32))
                        vsrc = vp1b
                    else:
                        nc.vector.memset(et0[:].bitcast(F32), 1.0)
                        vsrc = vp1
                    for s in range(S):
                        b, jc = divmod(s, NT)
                        if jc == 0:
                            accs[b] = accp.tile([F + 1, 1024], F32,
                                                tag="acc", name=f"acc{b}")
                        for hf in range(2):
                            nc.tensor.matmul(
                                accs[b][:, hf * 512: hf * 512 + 512],
                                vsrc[:, jc * (F + 1): (jc + 1) * (F + 1)],
                                et0[:, hf * 512: hf * 512 + 512],
                                start=(jc == 0), stop=(jc == NT - 1))
                else:
                    for s in range(S):
                        emit_logit(s)
                for b in range(NB):
                    emit_outdma(b, 0, 8)
                return
            if ABLATE == 10:
                # preamble + output DMA only
                for b in range(NB):
                    emit_outdma(b, 0, 8)
                return
            if ABLATE == 11:
                # ABL=6 with L/A emission batched in pairs (fewer PE
                # logit<->acc switches), accs ahead of logits in the queue
                et0 = etp.tile([128, 1024], F32R, tag="et", name="et0")
                nc.vector.memset(et0[:].bitcast(F32), 1.0)
                for s in range(S + 2):
                    if s % 2 == 0:
                        for a in (s - 2, s - 1):
                            if 0 <= a < S:
                                emit_acc(a)
                                if a % NT == NT - 1:
                                    emit_hsb(a // NT)
                                b_prev = a // NT - 1
                                jc = a % NT
                                if b_prev >= 0 and jc % 2 == 1:
                                    emit_trans(b_prev, jc // 2)
                        if s < S:
                            ets[s] = et0
                            ets[s + 1] = et0
                            emit_logit(s)
                            emit_logit(s + 1)
                for t8 in range(8):
                    emit_trans(NB - 1, t8)
                return
            if ABLATE == 13:
                # acc-only but every matmul uses a different vp1 chunk
                # (forces a weight change per matmul)
                et0 = etp.tile([128, 1024], F32R, tag="et", name="et0")
                nc.vector.memset(et0[:].bitcast(F32), 1.0)
                for s in range(S):
                    b, jc = divmod(s, NT)
                    if jc == 0:
                        accs[b] = accp.tile([F + 1, 1024], F32, tag="acc",
                                            name=f"acc{b}")
                    for hf in range(2):
                        w = ((jc + 8 * hf) % NT) * (F + 1)
                        nc.tensor.matmul(
                            accs[b][:, hf * 512: hf * 512 + 512],
                            vp1[:, w: w + F + 1],
                            et0[:, hf * 512: hf * 512 + 512],
                            start=(jc == 0), stop=(jc == NT - 1))
                for b in range(NB):
                    emit_outdma(b, 0, 8)
                return
            if ABLATE == 14:
                # logit-only, ONE [128,512] matmul per step (half the work
                # of ABL=8) — isolates per-instruction overhead
                for s in range(S):
                    b, jc = divmod(s, NT)
                    h, ib = blocks[b]
                    ab_h, pm_h = abpm(h)
                    lt = ltp.tile([128, 512], F32, tag="lt", name=f"lt{s}")
                    nc.tensor.matmul(
                        lt[:], ab_h[:, jc * 128: jc * 128 + 128],
                        pm_h[:, ib * 1024: ib * 1024 + 512],
                        start=True, stop=True)
                for b in range(NB):
                    emit_outdma(b, 0, 8)
                return
            if ABLATE == 15:
                # logit-only in bf16 (tests weight-load cost by dtype)
                BF16 = mybir.dt.bfloat16
                abb = etp.tile([128, N], BF16, tag="abb", name="abb")
                pmb = etp.tile([128, N], BF16, tag="pmb", name="pmb")
                nc.vector.tensor_copy(abb[:], ab_sp[:].bitcast(F32))
                nc.vector.tensor_copy(pmb[:], pm_sp[:].bitcast(F32))
                for s in range(S):
                    b, jc = divmod(s, NT)
                    h, ib = blocks[b]
                    h2 = min(h, 2)
                    lt = ltp.tile([128, 1024], F32, tag="lt", name=f"lt{s}")
                    for hf in range(2):
                        nc.tensor.matmul(
                            lt[:, hf * 512: hf * 512 + 512],
                            abb[32 * h2: 32 * h2 + 2,
                                jc * 128: jc * 128 + 128],
                            pmb[32 * h2: 32 * h2 + 2,
                                ib * 1024 + hf * 512:
                                ib * 1024 + hf * 512 + 512],
                            start=True, stop=True)
                for b in range(NB):
                    emit_outdma(b, 0, 8)
                return
            if ABLATE == 6:
                # full pipeline minus Act: logits + acc + postamble, with a
                # constant ones tile standing in for every exp result.
                et0 = etp.tile([128, 1024], F32R, tag="et", name="et0")
                nc.vector.memset(et0[:].bitcast(F32), 1.0)
                for s in range(S + 3):
                    if s < S:
                        emit_logit(s)
                    if 1 <= s <= S:
                        ets[s - 1] = et0
                    if s >= 3:
                        a = s - 3
                        emit_acc(a)
                        if a % NT == NT - 1:
                            emit_hsb(a // NT)
                        b_prev = a // NT - 1
                        jc = a % NT
                        if b_prev >= 0 and jc % 2 == 1:
                            emit_trans(b_prev, jc // 2)
                for t8 in range(8):
                    emit_trans(NB - 1, t8)
                return
            for s in range(S + 3):
                if s < S:
                    emit_logit(s)
                if 1 <= s <= S:
                    emit_exp(s - 1)
                if s >= 3 and ABLATE != 1:
                    a = s - 3
                    emit_acc(a)
                    if a % NT == NT - 1:
                        emit_hsb(a // NT)
                    b_prev = a // NT - 1
                    jc = a % NT
                    if b_prev >= 0 and jc % 2 == 1:
                        emit_trans(b_prev, jc // 2)
            if ABLATE != 1:
                for t8 in range(8):
                    emit_trans(NB - 1, t8)
            else:
                for b in range(NB):
                    emit_outdma(b, 0, 8)



# revision 43
# speedup vs baseline: 1.7282x; 1.0175x over previous
"""Multi-head graph attention (rank-2 LeakyReLU-softmax) Trainium2 kernel.

Reference computation (per batch b, head h):
    V = X @ vW + vb                       (N, F)
    q = V @ qW[:,h] + qb[h]               (N,)   per-node scalar
    k = V @ kW[:,h] + kb[h]               (N,)
    A_ij = softmax_j( LeakyReLU(q_i * k_j) )
    out[b,i,h,:] = sum_j A_ij V_j

Key identity used here: with P = max(q,0), M = min(q,0),
alpha = LeakyReLU(k) = max(k, 0.01k), beta = min(k, 0.01k),
    LeakyReLU(q_i * k_j) == alpha_j * P_i + beta_j * M_i      (exactly)
since for each i exactly one of P_i / M_i is nonzero.  So the N x N logit
matrix is a rank-2 outer product, built on the TensorEngine as a K=2
matmul (fp32r), exponentiated on the ScalarEngine straight out of PSUM,
and contracted against [V | 1] without the N x N matrix ever leaving the
chip.  The trailing all-ones column of Vp1 yields the softmax denominator
as row 64 of the same accumulation.

Sharding: core c -> batch b = c//2, heads h0 = 4*(c%2) .. h0+3.
"""

import numpy as np

import concourse.bacc as bacc
import concourse.tile as tile
import concourse.mybir as mybir
from concourse.bass_utils import run_bass_kernel_spmd

B, N, IN, F, H = 4, 2048, 256, 64, 8
NH = H // 2          # heads per core
NT = N // 128        # 16 i-tiles / j-chunks
F32 = mybir.dt.float32
F32R = mybir.dt.float32r
AF = mybir.ActivationFunctionType
ALU = mybir.AluOpType

N_CORES = 8
# packed param tensor columns: ident(128) | vW 2 chunks(128) | vb(1) | qw(4)
# | kw(4) | qb(4) | kb(4)
PRM_COLS = 128 + 128 + 1 + 4 + 4 + 4 + 4
_CACHE = {}
XCAST_DMA = False
import os as _os
ABLATE = int(_os.environ.get("ABL", "0"))  # 1: no acc/postamble (timing probe)
ACC_BF16 = int(_os.environ.get("ACCBF", "1"))  # bf16 vp1/et for the acc matmul
HSB_GP = int(_os.environ.get("HSBGP", "0"))    # gpsimd can't read PSUM (walrus)
LT_BUFS = int(_os.environ.get("LTBUFS", "2"))  # lt PSUM rotation depth
LOGIT8 = int(_os.environ.get("LOGIT8", "1"))   # bf16 hi/lo K=8-per-head logits
ACC_BUFS = int(_os.environ.get("ACCBUFS", "1"))  # acc PSUM tiles
HT_POOL = int(_os.environ.get("HTPOOL", "1"))  # 1: ht transposes own PSUM pool
HSB_SPLIT = int(_os.environ.get("HSBSPLIT", "0"))  # 1: hsb copy DVE+Act split


def build_nc(reps=1, unroll=False, version=4):
    """Build the kernel program.

    reps > 1 wraps the whole computation in a hardware For_i loop (all-engine
    barrier between iterations) so test.py can measure per-execution HW time
    by slope: (t(R) - t(1)) / (R - 1).  The graded kernel() path uses reps=1.
    """
    nc = bacc.Bacc("TRN2", target_bir_lowering=False, debug=False,
                   num_devices=N_CORES)
    xshape = [IN, N] if version >= 4 else [N, IN]
    X_d = nc.dram_tensor("X", xshape,
                         F32R if version >= 4 else F32,
                         kind="ExternalInput")
    vW_d = nc.dram_tensor("vW", [IN, F], F32, kind="ExternalInput")
    vb_d = nc.dram_tensor("vb", [F], F32, kind="ExternalInput")
    qw_d = nc.dram_tensor("qw", [F, NH], F32, kind="ExternalInput")
    kw_d = nc.dram_tensor("kw", [F, NH], F32, kind="ExternalInput")
    qb_d = nc.dram_tensor("qb", [NH], F32, kind="ExternalInput")
    kb_d = nc.dram_tensor("kb", [NH], F32, kind="ExternalInput")
    id_d = nc.dram_tensor("ident", [128, 128], F32, kind="ExternalInput")
    prm_d = nc.dram_tensor("prm", [128, PRM_COLS], F32, kind="ExternalInput")
    out_d = nc.dram_tensor("out", [N, NH * F], F32, kind="ExternalOutput")

    body = {1: _emit_body, 2: _emit_body_v2, 3: _emit_body_v3,
            4: _emit_body_v4}[version]
    extra = {"prm_d": prm_d} if version >= 4 else {}
    with tile.TileContext(nc) as tc:
        from contextlib import ExitStack
        with ExitStack() as rep_ctx:
            if reps > 1 and not unroll:
                rep_ctx.enter_context(tc.For_i(0, reps))
            for _ in range(reps if unroll else 1):
                body(nc, tc, X_d, vW_d, vb_d, qw_d, kw_d, qb_d, kb_d,
                     id_d, out_d, **extra)
    nc.compile()
    return nc


def _emit_body_v2(nc, tc, X_d, vW_d, vb_d, qw_d, kw_d, qb_d, kb_d, id_d,
                  out_d):
    """Software-pipelined main loop.

    Per (head, i-block) "block" (NB = NH*2 of them), per j-chunk step:
      PE:  logit matmul (K=2 rank-2 outer product) -> lt PSUM [128,1024]
      Act: exp straight out of PSUM -> et SBUF (the ONLY Act work)
      PE:  acc matmul [V|1]^T @ et -> acc PSUM [65,1024] accumulated over 16 j
    Steps are emitted with a 1-step skew (logit(s) before acc(s-1)) so PE's
    in-order queue never parks an exp-dependent acc in front of independent
    logit work.  Postamble (PE transpose + DVE normalize into an SBUF staging
    buffer) is interleaved into the following block's steps; output leaves the
    chip in one final DMA.
    """
    NB = NH * 2
    blocks = [(h, ib) for h in range(NH) for ib in range(2)]
    with tc.tile_pool(name="persist", bufs=1) as pp:
        ident = pp.tile([128, 128], F32)
        nc.sync.dma_start(ident[:], id_d[:])
        id_r = pp.tile([128, 128], F32R)
        nc.vector.tensor_copy(id_r[:], ident[:])
        vt_sb = pp.tile([F, N], F32R)         # V^T, bias folded
        qt = pp.tile([NH, N], F32)
        kt = pp.tile([NH, N], F32)
        ab_all = pp.tile([2, NH * N], F32R)   # row0 alpha, row1 beta; head h at cols h*N
        pm_all = pp.tile([2, NH * N], F32R)   # row0 P, row1 M
        vp1 = pp.tile([128, NT * (F + 1)], F32R)   # [V | 1] per j-tile
        obuf = pp.tile([128, NT * NH * F], F32)    # staged output

        # ---------- preamble: X^T, V^T, q/k ----------
        with tc.tile_pool(name="pre_sb", bufs=1) as sp:
            xsb = sp.tile([128, NT * IN], F32)
            nc.sync.dma_start(
                xsb[:].rearrange("p (t c) -> p t c", t=NT),
                X_d[:].rearrange("(t p) c -> p t c", p=128))
            vwsb = sp.tile([128, 128], F32)
            nc.sync.dma_start(
                vwsb[:].rearrange("p (t f) -> p t f", t=2),
                vW_d[:].rearrange("(t p) f -> p t f", p=128))
            vb_t = sp.tile([F, 1], F32)
            nc.sync.dma_start(vb_t[:], vb_d[:].unsqueeze(1))
            qw_t = sp.tile([F, NH], F32)
            nc.sync.dma_start(qw_t[:], qw_d[:])
            kw_t = sp.tile([F, NH], F32)
            nc.sync.dma_start(kw_t[:], kw_d[:])
            qb_t = sp.tile([NH, 1], F32)
            nc.sync.dma_start(qb_t[:], qb_d[:].unsqueeze(1))
            kb_t = sp.tile([NH, 1], F32)
            nc.sync.dma_start(kb_t[:], kb_d[:].unsqueeze(1))

            xt = sp.tile([128, 2 * N], F32R)  # X^T: chunk cc at cc*N
            vw_r = sp.tile([128, 128], F32R)
            nc.vector.tensor_copy(vw_r[:], vwsb[:])
            qw_r = sp.tile([F, NH], F32R)
            nc.vector.tensor_copy(qw_r[:], qw_t[:])
            kw_r = sp.tile([F, NH], F32R)
            nc.vector.tensor_copy(kw_r[:], kw_t[:])

            with tc.tile_pool(name="pre_ps", bufs=2, space="PSUM") as xp:
                for t in range(NT):
                    for cc in range(2):
                        tp = xp.tile([128, 128], F32)
                        nc.tensor.transpose(
                            tp[:], xsb[:, t * IN + cc * 128:
                                       t * IN + cc * 128 + 128], ident[:])
                        nc.vector.tensor_copy(
                            xt[:, cc * N + t * 128: cc * N + t * 128 + 128],
                            tp[:])

            with tc.tile_pool(name="vt_ps", bufs=1, space="PSUM") as vpp:
                vt_ps = vpp.tile([F, N], F32)
                for nb in range(4):
                    for cc in range(2):
                        nc.tensor.matmul(
                            vt_ps[:, nb * 512: nb * 512 + 512],
                            vw_r[:, cc * F: cc * F + F],
                            xt[:, cc * N + nb * 512: cc * N + nb * 512 + 512],
                            start=(cc == 0), stop=(cc == 1))
                nc.vector.tensor_scalar_add(vt_sb[:], vt_ps[:], vb_t[:])

            with tc.tile_pool(name="qk_ps", bufs=1, space="PSUM") as qpp:
                qt_ps = qpp.tile([NH, N], F32)
                kt_ps = qpp.tile([NH, N], F32)
                for nb in range(4):
                    nc.tensor.matmul(
                        qt_ps[:, nb * 512: nb * 512 + 512], qw_r[:],
                        vt_sb[:, nb * 512: nb * 512 + 512],
                        start=True, stop=True)
                    nc.tensor.matmul(
                        kt_ps[:, nb * 512: nb * 512 + 512], kw_r[:],
                        vt_sb[:, nb * 512: nb * 512 + 512],
                        start=True, stop=True)
                nc.vector.tensor_scalar_add(qt[:], qt_ps[:], qb_t[:])
                nc.vector.tensor_scalar_add(kt[:], kt_ps[:], kb_t[:])

            # per-head vectors, written [alpha0..3 | beta0..3] then paired
            abq = sp.tile([2 * NH, N], F32R)
            pmq = sp.tile([2 * NH, N], F32R)
            nc.vector.scalar_tensor_tensor(abq[0:NH, :], kt[:], 0.01, kt[:],
                                           ALU.mult, ALU.max)
            nc.vector.scalar_tensor_tensor(abq[NH:2 * NH, :], kt[:], 0.01,
                                           kt[:], ALU.mult, ALU.min)
            nc.vector.tensor_scalar_max(pmq[0:NH, :], qt[:], 0.0)
            nc.vector.tensor_scalar_min(pmq[NH:2 * NH, :], qt[:], 0.0)
            nc.sync.dma_start(
                ab_all[0:1, :].rearrange("o (h n) -> o h n", h=NH),
                abq[0:NH, :].unsqueeze(0))
            nc.sync.dma_start(
                ab_all[1:2, :].rearrange("o (h n) -> o h n", h=NH),
                abq[NH:2 * NH, :].unsqueeze(0))
            nc.sync.dma_start(
                pm_all[0:1, :].rearrange("o (h n) -> o h n", h=NH),
                pmq[0:NH, :].unsqueeze(0))
            nc.sync.dma_start(
                pm_all[1:2, :].rearrange("o (h n) -> o h n", h=NH),
                pmq[NH:2 * NH, :].unsqueeze(0))

        # ---------- Vp1 = [V | 1] per j-tile ----------
        nc.vector.memset(vp1[:].bitcast(F32), 1.0)
        with tc.tile_pool(name="v_ps", bufs=2, space="PSUM") as vp:
            for t in range(NT):
                v_ps = vp.tile([128, F], F32R)
                nc.tensor.transpose(
                    v_ps[:], vt_sb[:, t * 128: t * 128 + 128],
                    id_r[0:F, 0:F])
                nc.vector.tensor_copy(
                    vp1[:, t * (F + 1): t * (F + 1) + F], v_ps[:])

        # ---------- software-pipelined main loop ----------
        S = NB * NT  # 128 steps
        with tc.tile_pool(name="lt_ps", bufs=2, space="PSUM") as ltp, \
             tc.tile_pool(name="acc_ps", bufs=1, space="PSUM") as accp, \
             tc.tile_pool(name="ht_ps", bufs=2, space="PSUM") as htp, \
             tc.tile_pool(name="et_sb", bufs=4) as etp, \
             tc.tile_pool(name="post_sb", bufs=2) as postp:
            lts, ets, accs, hsbs = {}, {}, {}, {}

            def emit_logit(s):
                b, jc = divmod(s, NT)
                h, ib = blocks[b]
                lt = ltp.tile([128, 1024], F32, tag="lt", name=f"lt{s}")
                for hf in range(2):
                    nc.tensor.matmul(
                        lt[:, hf * 512: hf * 512 + 512],
                        ab_all[:, h * N + jc * 128: h * N + jc * 128 + 128],
                        pm_all[:, h * N + ib * 1024 + hf * 512:
                               h * N + ib * 1024 + hf * 512 + 512],
                        start=True, stop=True)
                lts[s] = lt

            def emit_exp(s):
                et = etp.tile([128, 1024], F32R, tag="et", name=f"et{s}")
                nc.scalar.activation(et[:], lts[s][:], AF.Exp)
                ets[s] = et

            def emit_acc(s):
                b, jc = divmod(s, NT)
                if jc == 0:
                    accs[b] = accp.tile([F + 1, 1024], F32, tag="acc", name=f"acc{b}")
                for hf in range(2):
                    nc.tensor.matmul(
                        accs[b][:, hf * 512: hf * 512 + 512],
                        vp1[:, jc * (F + 1): (jc + 1) * (F + 1)],
                        ets[s][:, hf * 512: hf * 512 + 512],
                        start=(jc == 0), stop=(jc == NT - 1))

            def emit_hsb(b):
                hsb = postp.tile([F + 1, 1024], F32, tag="hsb", name=f"hsb{b}")
                nc.vector.tensor_copy(hsb[:], accs[b][:])
                hsbs[b] = hsb

            def emit_trans(b, t8):
                h, ib = blocks[b]
                ht = htp.tile([128, F + 1], F32, tag="ht", name=f"ht{b}_{t8}")
                nc.tensor.transpose(
                    ht[:], hsbs[b][:, t8 * 128: t8 * 128 + 128],
                    id65[:])
                rcp = postp.tile([128, 1], F32, tag="rcp", name=f"rcp{b}_{t8}")
                nc.vector.reciprocal(rcp[:], ht[:, F:F + 1])
                t = ib * 8 + t8
                nc.vector.tensor_scalar_mul(
                    obuf[:, t * (NH * F) + h * F: t * (NH * F) + h * F + F],
                    ht[:, 0:F], rcp[:])

            for s in range(S + 1):
                if s < S:
                    emit_logit(s)
                if s >= 1:
                    emit_exp(s - 1)
                    emit_acc(s - 1)
                    if (s - 1) % NT == NT - 1:
                        emit_hsb((s - 1) // NT)
                    # spread previous block's 8 transposes over this block
                    b_prev = s // NT - 1
                    jc = s % NT
                    if b_prev >= 0 and s < S and jc % 2 == 1:
                        emit_trans(b_prev, jc // 2)
            for t8 in range(8):  # drain last block
                emit_trans(NB - 1, t8)

        nc.sync.dma_start(
            out_d[:].rearrange("(t p) c -> p t c", p=128),
            obuf[:].rearrange("p (t c) -> p t c", t=NT))


def _emit_body(nc, tc, X_d, vW_d, vb_d, qw_d, kw_d, qb_d, kb_d, id_d, out_d):
    if True:
        with tc.tile_pool(name="persist", bufs=1) as pp:
            ident = pp.tile([128, 128], F32)
            nc.sync.dma_start(ident[:], id_d[:])
            id_r = pp.tile([128, 128], F32R)
            nc.vector.tensor_copy(id_r[:], ident[:])
            vt_sb = pp.tile([F, N], F32R)         # V^T, bias folded
            qt = pp.tile([NH, N], F32)
            kt = pp.tile([NH, N], F32)
            ab_hs = [pp.tile([2, N], F32R, name=f"abh{h}", tag=f"ab{h}") for h in range(NH)]
            pm_hs = [pp.tile([2, N], F32R, name=f"pmh{h}", tag=f"pm{h}") for h in range(NH)]
            vp1 = pp.tile([128, NT * (F + 1)], F32R)   # [V | 1] per j-tile

            # ---------- preamble: X^T, V^T, q/k ----------
            with tc.tile_pool(name="pre_sb", bufs=1) as sp:
                xsb = sp.tile([128, NT * IN], F32)
                nc.sync.dma_start(
                    xsb[:].rearrange("p (t c) -> p t c", t=NT),
                    X_d[:].rearrange("(t p) c -> p t c", p=128))
                vwsb = sp.tile([128, 128], F32)
                nc.sync.dma_start(
                    vwsb[:].rearrange("p (t f) -> p t f", t=2),
                    vW_d[:].rearrange("(t p) f -> p t f", p=128))
                vb_t = sp.tile([F, 1], F32)
                nc.sync.dma_start(vb_t[:], vb_d[:].unsqueeze(1))
                qw_t = sp.tile([F, NH], F32)
                nc.sync.dma_start(qw_t[:], qw_d[:])
                kw_t = sp.tile([F, NH], F32)
                nc.sync.dma_start(kw_t[:], kw_d[:])
                qb_t = sp.tile([NH, 1], F32)
                nc.sync.dma_start(qb_t[:], qb_d[:].unsqueeze(1))
                kb_t = sp.tile([NH, 1], F32)
                nc.sync.dma_start(kb_t[:], kb_d[:].unsqueeze(1))

                xt = sp.tile([128, 2 * N], F32R)  # X^T: chunk cc at cc*N
                vw_r = sp.tile([128, 128], F32R)
                nc.vector.tensor_copy(vw_r[:], vwsb[:])
                qw_r = sp.tile([F, NH], F32R)
                nc.vector.tensor_copy(qw_r[:], qw_t[:])
                kw_r = sp.tile([F, NH], F32R)
                nc.vector.tensor_copy(kw_r[:], kw_t[:])

                with tc.tile_pool(name="pre_ps", bufs=2, space="PSUM") as xp:
                    for t in range(NT):
                        for cc in range(2):
                            tp = xp.tile([128, 128], F32)
                            nc.tensor.transpose(
                                tp[:], xsb[:, t * IN + cc * 128:
                                           t * IN + cc * 128 + 128], ident[:])
                            nc.vector.tensor_copy(
                                xt[:, cc * N + t * 128: cc * N + t * 128 + 128],
                                tp[:])

                with tc.tile_pool(name="vt_ps", bufs=1, space="PSUM") as vpp:
                    vt_ps = vpp.tile([F, N], F32)
                    for nb in range(4):
                        for cc in range(2):
                            nc.tensor.matmul(
                                vt_ps[:, nb * 512: nb * 512 + 512],
                                vw_r[:, cc * F: cc * F + F],
                                xt[:, cc * N + nb * 512: cc * N + nb * 512 + 512],
                                start=(cc == 0), stop=(cc == 1))
                    nc.vector.tensor_scalar_add(vt_sb[:], vt_ps[:], vb_t[:])

                with tc.tile_pool(name="qk_ps", bufs=1, space="PSUM") as qpp:
                    qt_ps = qpp.tile([NH, N], F32)
                    kt_ps = qpp.tile([NH, N], F32)
                    for nb in range(4):
                        nc.tensor.matmul(
                            qt_ps[:, nb * 512: nb * 512 + 512], qw_r[:],
                            vt_sb[:, nb * 512: nb * 512 + 512],
                            start=True, stop=True)
                        nc.tensor.matmul(
                            kt_ps[:, nb * 512: nb * 512 + 512], kw_r[:],
                            vt_sb[:, nb * 512: nb * 512 + 512],
                            start=True, stop=True)
                    nc.vector.tensor_scalar_add(qt[:], qt_ps[:], qb_t[:])
                    nc.vector.tensor_scalar_add(kt[:], kt_ps[:], kb_t[:])

            # ---------- per-head vectors (fp32r) ----------
            with tc.tile_pool(name="vec_sb", bufs=1) as vs:
                a4 = vs.tile([NH, N], F32R)
                b4 = vs.tile([NH, N], F32R)
                p4 = vs.tile([NH, N], F32R)
                m4 = vs.tile([NH, N], F32R)
                nc.vector.scalar_tensor_tensor(a4[:], kt[:], 0.01, kt[:],
                                               ALU.mult, ALU.max)
                nc.vector.scalar_tensor_tensor(b4[:], kt[:], 0.01, kt[:],
                                               ALU.mult, ALU.min)
                nc.vector.tensor_scalar_max(p4[:], qt[:], 0.0)
                nc.vector.tensor_scalar_min(m4[:], qt[:], 0.0)
                for h in range(NH):
                    nc.sync.dma_start(ab_hs[h][0:1, :], a4[h:h + 1, :])
                    nc.sync.dma_start(ab_hs[h][1:2, :], b4[h:h + 1, :])
                    nc.sync.dma_start(pm_hs[h][0:1, :], p4[h:h + 1, :])
                    nc.sync.dma_start(pm_hs[h][1:2, :], m4[h:h + 1, :])

            # ---------- Vp1 = [V | 1] per j-tile ----------
            nc.vector.memset(vp1[:].bitcast(F32), 1.0)
            with tc.tile_pool(name="v_ps", bufs=2, space="PSUM") as vp:
                for t in range(NT):
                    v_ps = vp.tile([128, F], F32R)
                    nc.tensor.transpose(
                        v_ps[:], vt_sb[:, t * 128: t * 128 + 128],
                        id_r[0:F, 0:F])
                    nc.vector.tensor_copy(
                        vp1[:, t * (F + 1): t * (F + 1) + F], v_ps[:])

            # ---------- main loop ----------
            hsbs = {}
            with tc.tile_pool(name="lt_ps", bufs=3, space="PSUM") as ltp, \
                 tc.tile_pool(name="acc_ps", bufs=1, space="PSUM") as accp, \
                 tc.tile_pool(name="et_sb", bufs=3) as etp:
                for h in range(NH):
                    ab_h = ab_hs[h][:]
                    pm_h = pm_hs[h][:]
                    for ib in range(2):
                        acc = accp.tile([F + 1, 1024], F32, tag="acc")
                        for jc in range(NT):
                            lt = ltp.tile([128, 1024], F32, tag="lt", name=f"lt{s}")
                            for hf in range(2):
                                nc.tensor.matmul(
                                    lt[:, hf * 512: hf * 512 + 512],
                                    ab_h[:, jc * 128: jc * 128 + 128],
                                    pm_h[:, ib * 1024 + hf * 512:
                                         ib * 1024 + hf * 512 + 512],
                                    start=True, stop=True)
                            et = etp.tile([128, 1024], F32R, tag="et", name=f"et{s}")
                            nc.scalar.activation(et[:], lt[:], AF.Exp)
                            for hf in range(2):
                                nc.tensor.matmul(
                                    acc[:, hf * 512: hf * 512 + 512],
                                    vp1[:, jc * (F + 1): (jc + 1) * (F + 1)],
                                    et[:, hf * 512: hf * 512 + 512],
                                    start=(jc == 0), stop=(jc == NT - 1))
                        hsb = pp.tile([F + 1, 1024], F32, name=f"hsb{h}_{ib}",
                                      tag=f"hsb{h}_{ib}")
                        nc.vector.tensor_copy(hsb[:], acc[:])
                        hsbs[(h, ib)] = hsb

            # ---------- postamble: transpose + normalize + store ----------
            with tc.tile_pool(name="ht_ps", bufs=4, space="PSUM") as htp, \
                 tc.tile_pool(name="post_sb", bufs=4) as postp:
                for h in range(NH):
                    for ib in range(2):
                        hsb = hsbs[(h, ib)]
                        for t8 in range(8):
                            ht = htp.tile([128, F + 1], F32, tag="ht")
                            nc.tensor.transpose(
                                ht[:], hsb[:, t8 * 128: t8 * 128 + 128],
                                ident[0:F + 1, 0:F + 1])
                            rcp = postp.tile([128, 1], F32, tag="rcp", name=f"rcp{b}_{t8}")
                            nc.vector.reciprocal(rcp[:], ht[:, F:F + 1])
                            ob = postp.tile([128, F], F32, tag="ob")
                            nc.vector.tensor_scalar_mul(ob[:], ht[:, 0:F], rcp[:])
                            r0 = ib * 1024 + t8 * 128
                            nc.sync.dma_start(
                                out_d[r0:r0 + 128, h * F: h * F + F], ob[:])


def _get_nc():
    if "nc" not in _CACHE:
        _CACHE["nc"] = build_nc()
    return _CACHE["nc"]


def make_in_maps(X, vW, vb, qW, qb, kW, kb):
    ident = np.eye(128, dtype=np.float32)
    in_maps = []
    for c in range(N_CORES):
        b, h0 = c // 2, NH * (c % 2)
        qwc = np.ascontiguousarray(qW[:, h0:h0 + NH])
        kwc = np.ascontiguousarray(kW[:, h0:h0 + NH])
        qbc = np.ascontiguousarray(qb[h0:h0 + NH])
        kbc = np.ascontiguousarray(kb[h0:h0 + NH])
        prm = np.zeros((128, PRM_COLS), dtype=np.float32)
        prm[:, 0:128] = ident
        prm[:, 128:256] = vW.reshape(2, 128, F).transpose(1, 0, 2).reshape(128, 128)
        prm[0:F, 256] = vb
        prm[0:F, 257:261] = qwc
        prm[0:F, 261:265] = kwc
        prm[0:1, 265:269] = qbc
        prm[0:1, 269:273] = kbc
        in_maps.append({
            "X": np.ascontiguousarray(X[b].T),
            "vW": np.ascontiguousarray(vW),
            "vb": np.ascontiguousarray(vb),
            "qw": qwc,
            "kw": kwc,
            "qb": qbc,
            "kb": kbc,
            "ident": ident,
            "prm": prm,
        })
    return in_maps


def assemble(results):
    full = np.empty((B, N, H * F), dtype=np.float32)
    for c in range(N_CORES):
        b, h0 = c // 2, NH * (c % 2)
        full[b][:, h0 * F:(h0 + NH) * F] = results[c]["out"]
    return full


def kernel(X, vW, vb, qW, qb, kW, kb):
    X, vW, vb = np.asarray(X), np.asarray(vW), np.asarray(vb)
    qW, qb, kW, kb = np.asarray(qW), np.asarray(qb), np.asarray(kW), np.asarray(kb)
    nc = _get_nc()
    res = run_bass_kernel_spmd(nc, make_in_maps(X, vW, vb, qW, qb, kW, kb),
                               list(range(N_CORES)))
    return assemble(res.results)


def _emit_body_v3(nc, tc, X_d, vW_d, vb_d, qw_d, kw_d, qb_d, kb_d, id_d,
                  out_d):
    """v2 main loop + pipelined preamble and per-block output DMAs.

    Preamble works in 4 node-groups of 512: X DMA group g -> 8 PE transposes
    into a [128,512] PSUM tile -> 2 wide copies (DVE/Pool) -> V^T matmul ->
    bias-add -> q/k matmul -> bias-add -> alpha/beta/P/M chunk -> pack DMA.
    First exp can start after group 0's chain (~7us) instead of after the
    whole preamble.  Act engine does exp ONLY (table preloaded at t=0).
    """
    NB = NH * 2
    blocks = [(h, ib) for h in range(NH) for ib in range(2)]
    with tc.tile_pool(name="persist", bufs=1) as pp:
        # Exp activation-table preload, before anything else on Act.
        zz = pp.tile([1, 2], F32R)
        nc.vector.memset(zz[:].bitcast(F32), 0.0)
        nc.scalar.activation(zz[:], zz[:], AF.Exp)

        id65 = pp.tile([F + 1, F + 1], F32)   # identity for postamble transposes
        id_r = pp.tile([128, 128], F32R)
        vt_sb = pp.tile([F, N], F32R)         # V^T, bias folded
        qt = pp.tile([NH, N], F32)
        kt = pp.tile([NH, N], F32)
        ab_all = pp.tile([2, NH * N], F32R)   # row0 alpha, row1 beta
        pm_all = pp.tile([2, NH * N], F32R)   # row0 P, row1 M
        vp1 = pp.tile([128, NT * (F + 1)], F32R)
        obuf = pp.tile([128, NT * NH * F], F32)
        nc.vector.memset(vp1[:].bitcast(F32), 1.0)

        with tc.tile_pool(name="pre_sb", bufs=1) as sp:
            xsb = sp.tile([128, NT * IN], F32)
            vwsb = sp.tile([128, 128], F32)
            vb_t = sp.tile([F, 1], F32)
            qw_t = sp.tile([F, NH], F32)
            kw_t = sp.tile([F, NH], F32)
            qb_t = sp.tile([NH, 1], F32)
            kb_t = sp.tile([NH, 1], F32)
            abq = sp.tile([2 * NH, N], F32R)  # rows 0-3 alpha, 4-7 beta
            pmq = sp.tile([2 * NH, N], F32R)  # rows 0-3 P, 4-7 M

            # input DMAs: ident+vW first (needed by transposes / V^T), then
            # X in 4 groups; small params via other queues.
            nc.sync.dma_start(ident[:], id_d[:])
            nc.sync.dma_start(
                vwsb[:].rearrange("p (t f) -> p t f", t=2),
                vW_d[:].rearrange("(t p) f -> p t f", p=128))
            for g in range(4):
                nc.sync.dma_start(
                    xsb[:, g * 4 * IN:(g + 1) * 4 * IN]
                        .rearrange("p (t c) -> p t c", t=4),
                    X_d[g * 512:(g + 1) * 512, :]
                        .rearrange("(t p) c -> p t c", p=128))
            nc.scalar.dma_start(vb_t[:], vb_d[:].unsqueeze(1))
            nc.scalar.dma_start(qw_t[:], qw_d[:])
            nc.scalar.dma_start(kw_t[:], kw_d[:])
            nc.gpsimd.dma_start(qb_t[:], qb_d[:].unsqueeze(1))
            nc.gpsimd.dma_start(kb_t[:], kb_d[:].unsqueeze(1))

            xt = sp.tile([128, 2 * N], F32R)  # X^T: chunk cc at cc*N
            vw_r = sp.tile([128, 128], F32R)
            nc.vector.tensor_copy(id_r[:], ident[:])
            nc.gpsimd.tensor_copy(vw_r[:], vwsb[:])
            qw_r = sp.tile([F, NH], F32R)
            nc.vector.tensor_copy(qw_r[:], qw_t[:])
            kw_r = sp.tile([F, NH], F32R)
            nc.vector.tensor_copy(kw_r[:], kw_t[:])

            with tc.tile_pool(name="tp_ps", bufs=2, space="PSUM") as xp, \
                 tc.tile_pool(name="vt_ps", bufs=1, space="PSUM") as vpp, \
                 tc.tile_pool(name="qk_ps", bufs=2, space="PSUM") as qpp, \
                 tc.tile_pool(name="v_ps", bufs=1, space="PSUM") as vsp:
                for g in range(4):
                    # X^T for this group's 4 node-tiles (both 128-col chunks)
                    for cc in range(2):
                        tp = xp.tile([128, 512], F32, tag="tp",
                                     name=f"tp{g}_{cc}")
                        for tt in range(4):
                            t = 4 * g + tt
                            nc.tensor.transpose(
                                tp[:, tt * 128: tt * 128 + 128],
                                xsb[:, t * IN + cc * 128:
                                    t * IN + cc * 128 + 128], ident[:])
                        nc.scalar.copy(
                            xt[:, cc * N + g * 512: cc * N + g * 512 + 512],
                            tp[:])
                    # V^T chunk
                    vt_ps = vpp.tile([F, 512], F32, tag="vtps",
                                     name=f"vtps{g}")
                    for cc in range(2):
                        nc.tensor.matmul(
                            vt_ps[:],
                            vw_r[:, cc * F: cc * F + F],
                            xt[:, cc * N + g * 512: cc * N + g * 512 + 512],
                            start=(cc == 0), stop=(cc == 1))
                    nc.vector.tensor_scalar_add(
                        vt_sb[:, g * 512:(g + 1) * 512], vt_ps[:], vb_t[:])
                    # q / k chunks
                    qt_ps = qpp.tile([NH, 512], F32, tag="qk",
                                     name=f"qtps{g}")
                    nc.tensor.matmul(qt_ps[:], qw_r[:],
                                     vt_sb[:, g * 512: g * 512 + 512],
                                     start=True, stop=True)
                    kt_ps = qpp.tile([NH, 512], F32, tag="qk",
                                     name=f"ktps{g}")
                    nc.tensor.matmul(kt_ps[:], kw_r[:],
                                     vt_sb[:, g * 512: g * 512 + 512],
                                     start=True, stop=True)
                    sl = slice(g * 512, (g + 1) * 512)
                    nc.vector.tensor_scalar_add(qt[:, sl], qt_ps[:], qb_t[:])
                    nc.gpsimd.tensor_scalar_add(kt[:, sl], kt_ps[:], kb_t[:])
                    # alpha/beta (from k), P/M (from q) for this chunk
                    nc.vector.scalar_tensor_tensor(
                        abq[0:NH, sl], kt[:, sl], 0.01, kt[:, sl],
                        ALU.mult, ALU.max)
                    nc.gpsimd.scalar_tensor_tensor(
                        abq[NH:2 * NH, sl], kt[:, sl], 0.01, kt[:, sl],
                        ALU.mult, ALU.min)
                    nc.vector.tensor_scalar_max(pmq[0:NH, sl], qt[:, sl], 0.0)
                    nc.gpsimd.tensor_scalar_min(pmq[NH:2 * NH, sl],
                                                qt[:, sl], 0.0)
                    # pack into [2, NH*N] layout (head-major columns)
                    for row in range(2):
                        nc.gpsimd.dma_start(
                            ab_all[row:row + 1, :]
                                .rearrange("o (h n) -> o h n", h=NH)
                                [:, :, g * 512:(g + 1) * 512],
                            abq[row * NH:(row + 1) * NH, sl].unsqueeze(0))
                        nc.gpsimd.dma_start(
                            pm_all[row:row + 1, :]
                                .rearrange("o (h n) -> o h n", h=NH)
                                [:, :, g * 512:(g + 1) * 512],
                            pmq[row * NH:(row + 1) * NH, sl].unsqueeze(0))
                    # Vp1 tiles for this group
                    v_ps = vsp.tile([128, 4 * F], F32R, tag="vps",
                                    name=f"vps{g}")
                    for tt in range(4):
                        t = 4 * g + tt
                        nc.tensor.transpose(
                            v_ps[:, tt * F: tt * F + F],
                            vt_sb[:, t * 128: t * 128 + 128],
                            id_r[0:F, 0:F])
                    eng = nc.vector if g % 2 == 0 else nc.gpsimd
                    eng.tensor_copy(
                        vp1[:].rearrange("p (t c) -> p t c", c=F + 1)
                            [:, 4 * g: 4 * g + 4, 0:F],
                        v_ps[:].rearrange("p (t c) -> p t c", c=F))

        # ---------- software-pipelined main loop ----------
        S = NB * NT  # 128 steps
        with tc.tile_pool(name="lt_ps", bufs=2, space="PSUM") as ltp, \
             tc.tile_pool(name="acc_ps", bufs=1, space="PSUM") as accp, \
             tc.tile_pool(name="ht_ps", bufs=2, space="PSUM") as htp, \
             tc.tile_pool(name="et_sb", bufs=4) as etp, \
             tc.tile_pool(name="post_sb", bufs=2) as postp:
            lts, ets, accs, hsbs = {}, {}, {}, {}

            def emit_logit(s):
                b, jc = divmod(s, NT)
                h, ib = blocks[b]
                lt = ltp.tile([128, 1024], F32, tag="lt", name=f"lt{s}")
                for hf in range(2):
                    nc.tensor.matmul(
                        lt[:, hf * 512: hf * 512 + 512],
                        ab_all[:, h * N + jc * 128: h * N + jc * 128 + 128],
                        pm_all[:, h * N + ib * 1024 + hf * 512:
                               h * N + ib * 1024 + hf * 512 + 512],
                        start=True, stop=True)
                lts[s] = lt

            def emit_exp(s):
                et = etp.tile([128, 1024], F32R, tag="et", name=f"et{s}")
                nc.scalar.activation(et[:], lts[s][:], AF.Exp)
                ets[s] = et

            def emit_acc(s):
                b, jc = divmod(s, NT)
                if jc == 0:
                    accs[b] = accp.tile([F + 1, 1024], F32, tag="acc",
                                        name=f"acc{b}")
                for hf in range(2):
                    nc.tensor.matmul(
                        accs[b][:, hf * 512: hf * 512 + 512],
                        vp1[:, jc * (F + 1): (jc + 1) * (F + 1)],
                        ets[s][:, hf * 512: hf * 512 + 512],
                        start=(jc == 0), stop=(jc == NT - 1))

            def emit_hsb(b):
                hsb = postp.tile([F + 1, 1024], F32, tag="hsb",
                                 name=f"hsb{b}")
                nc.vector.tensor_copy(hsb[:, 0:512], accs[b][:, 0:512])
                nc.gpsimd.tensor_copy(hsb[:, 512:1024], accs[b][:, 512:1024])
                hsbs[b] = hsb

            def emit_trans(b, t8):
                h, ib = blocks[b]
                ht = htp.tile([128, F + 1], F32, tag="ht", name=f"ht{b}_{t8}")
                nc.tensor.transpose(
                    ht[:], hsbs[b][:, t8 * 128: t8 * 128 + 128],
                    id65[:])
                rcp = postp.tile([128, 1], F32, tag="rcp",
                                 name=f"rcp{b}_{t8}")
                nc.vector.reciprocal(rcp[:], ht[:, F:F + 1])
                t = ib * 8 + t8
                nc.vector.tensor_scalar_mul(
                    obuf[:, t * (NH * F) + h * F: t * (NH * F) + h * F + F],
                    ht[:, 0:F], rcp[:])
                if t8 == 7:
                    emit_outdma(b)

            def emit_outdma(b):
                h, ib = blocks[b]
                nc.sync.dma_start(
                    out_d[ib * 1024:(ib + 1) * 1024, h * F:(h + 1) * F]
                        .rearrange("(t p) c -> p t c", p=128),
                    obuf[:].rearrange("p (t c) -> p t c", c=NH * F)
                        [:, ib * 8:(ib + 1) * 8, h * F:(h + 1) * F])

            for s in range(S + 1):
                if s < S:
                    emit_logit(s)
                if s >= 1:
                    emit_exp(s - 1)
                    emit_acc(s - 1)
                    if (s - 1) % NT == NT - 1:
                        emit_hsb((s - 1) // NT)
                    b_prev = s // NT - 1
                    jc = s % NT
                    if b_prev >= 0 and s < S and jc % 2 == 1:
                        emit_trans(b_prev, jc // 2)
            for t8 in range(8):  # drain last block
                emit_trans(NB - 1, t8)


def _emit_body_v4(nc, tc, X_d, vW_d, vb_d, qw_d, kw_d, qb_d, kb_d, id_d,
                  out_d, prm_d=None):
    """v3 + lane-aligned preamble, no per-chunk pack DMAs.

    q/k are produced by matmuls whose lhsT is zero-padded so head h's scalar
    lands (duplicated) on partitions {32h, 32h+1}; q/k biases ride a ones row
    appended to V^T (so the q/k matmul adds them via K=65).  alpha/beta/P/M
    are then single strided DVE/Pool ops straight out of PSUM into the
    matmul-legal [128, N] layouts (alpha_h/P_h at partition 32h, beta_h/M_h
    at 32h+1).  Head 3 (base 96 — illegal for PE) is staged to a [2, N] tile
    by one DMA per tensor at preamble end; its blocks run last.
    """
    NB = NH * 2
    blocks = [(h, ib) for h in range(NH) for ib in range(2)]
    with tc.tile_pool(name="persist", bufs=1) as pp:
        zz = pp.tile([1, 2], F32R)
        nc.vector.memset(zz[:].bitcast(F32), 0.0)
        nc.scalar.activation(zz[:], zz[:], AF.Exp)

        id65 = pp.tile([F + 1, F + 1], F32)   # identity for postamble transposes
        id_r = pp.tile([128, 128], F32R)
        vt1 = pp.tile([F + 1, N], F32R)       # V^T rows 0..63, row 64 = ones
        ABN = 512 if LOGIT8 else N
        ab_sp = pp.tile([128, ABN], F32R, tag="ab_sp")  # 32h = alpha_h
        pm_sp = pp.tile([128, ABN], F32R, tag="pm_sp")  # 32h = P_h
        # Per-head zero-padded alpha/beta weights: K=128 logit lhsT so every
        # main-loop matmul contracts over all 128 partitions (avoids PE
        # row-group reconfig between K=2 logits and K=128 accs).  Rows
        # 32h/32h+1 hold alpha_h/-beta_h, everything else stays zero; the
        # full pm_sp rides along as rhs since zero lhsT rows null out the
        # other heads.
        BF16 = mybir.dt.bfloat16
        F16 = mybir.dt.float16
        if LOGIT8:
            # fp16 alpha/beta/P/M: 11-bit mantissa keeps exp(logit) rounding
            # at ~3e-3 overall (bf16's 8-bit would be ~2e-2) and 2-byte
            # 128-col weights make the logit matmul FWL-eligible.  Same row
            # layout as the f32r path: head h at rows 32h/32h+1, zeros
            # elsewhere in the per-head lhsT tiles.
            abp = [pp.tile([128, N], F16, name=f"abp{h}", tag=f"abp{h}")
                   for h in range(NH)]
            pm8 = pp.tile([128, N], F16, tag="pm8")
            abh16 = pp.tile([128, N], F16, tag="abh16")
            for h in range(NH):
                eng = nc.vector if h % 2 == 0 else nc.gpsimd
                eng.memset(abp[h][:], 0.0)
        else:
            abp = [pp.tile([128, N], F32R, name=f"abp{h}") for h in range(NH)]
            for h in range(NH):
                eng = nc.vector if h % 2 == 0 else nc.gpsimd
                eng.memset(abp[h][:].bitcast(F32), 0.0)
        ACDT = mybir.dt.bfloat16 if ACC_BF16 else F32R
        vp1 = pp.tile([128, NT * (F + 1)], ACDT)
        obuf = pp.tile([128, NT * NH * F], F32)
        if ACC_BF16:
            nc.vector.memset(vp1[:], 1.0)
        else:
            nc.vector.memset(vp1[:].bitcast(F32), 1.0)
        nc.vector.memset(vt1[F:F + 1, :].bitcast(F32), 1.0)

        if True:
            sp = pp  # preamble tensors live in the persistent pool: their
            # SBUF never gets recycled under the main loop's et/hsb tiles,
            # so the first exp isn't serialized behind the preamble's tail.
            xt = sp.tile([128, 2 * N], F32R)  # X^T: chunk cc at cc*N

            # One packed-param DMA (ident | vW | vb | qw | kw | qb | kb)
            # then the four X^T groups (host supplies X transposed), all FIFO
            # on the sync HWDGE queue: params land by ~2us, X owns the bus
            # right after, and each 512-node group is immediately matmul-ready
            # (no on-chip transposes).
            prm = sp.tile([128, PRM_COLS], F32)
            nc.sync.dma_start(prm[:], prm_d[:])
            # X is declared f32r in DRAM (same bits as f32), so each group
            # DMAs straight into the matmul-ready X^T tile; two HWDGE queues
            # split the 2 MB transfer.
            for g in range(4):
                dq = nc.sync if g % 2 == 0 else nc.scalar
                dq.dma_start(
                    xt[:].rearrange("p (cc n) -> p cc n", cc=2)
                        [:, :, g * 512:(g + 1) * 512],
                    X_d[:].rearrange("(cc p) n -> p cc n", p=128)
                        [:, :, g * 512:(g + 1) * 512])
            ident = prm[:, 0:128]
            vwsb = prm[:, 128:256]
            vb_t = prm[0:F, 256:257]
            qw_t = prm[0:F, 257:261]
            kw_t = prm[0:F, 261:265]
            qb_row = prm[0:1, 265:269]
            kb_row = prm[0:1, 269:273]

            vw_r = sp.tile([128, 128], F32R)
            kscr0 = sp.tile([128, 512], F32)
            kscr1 = sp.tile([128, 512], F32)
            kscr = [kscr0, kscr1]
            nc.vector.tensor_copy(id_r[:], ident[:])
            nc.gpsimd.tensor_copy(id65[:], ident[0:F + 1, 0:F + 1])
            nc.gpsimd.tensor_copy(vw_r[:], vwsb[:])

            # padded q/k lhsT: [65, 128]; rows 0..63 = w dup at {32h,32h+1},
            # row 64 = bias dup there too; zero elsewhere.
            # Padded lhsT columns: even col 32h = +w_h (+bias), odd col
            # 32h+1 = -w_h (-bias).  Odd PSUM lanes then hold -k / -q, so a
            # single full-width max() yields [alpha; -beta] / [P; -M]; the
            # rank-2 logit contraction multiplies the two odd rows together
            # and the negations cancel.
            qkw = sp.tile([F + 1, 128], F32R)
            kkw = sp.tile([F + 1, 128], F32R)
            nc.vector.memset(qkw[:].bitcast(F32), 0.0)
            nc.vector.memset(kkw[:].bitcast(F32), 0.0)
            for rr in range(2):
                sgn = 1.0 if rr == 0 else -1.0
                nc.vector.tensor_scalar_mul(
                    qkw[0:F, :].rearrange("f (h r) -> f h r", r=32)
                        [:, :, rr:rr + 1],
                    qw_t[:].unsqueeze(2), sgn)
                nc.vector.tensor_scalar_mul(
                    qkw[F:F + 1, :].rearrange("o (h r) -> o h r", r=32)
                        [:, :, rr:rr + 1],
                    qb_row[:].unsqueeze(2), sgn)
                nc.vector.tensor_scalar_mul(
                    kkw[0:F, :].rearrange("f (h r) -> f h r", r=32)
                        [:, :, rr:rr + 1],
                    kw_t[:].unsqueeze(2), sgn)
                nc.vector.tensor_scalar_mul(
                    kkw[F:F + 1, :].rearrange("o (h r) -> o h r", r=32)
                        [:, :, rr:rr + 1],
                    kb_row[:].unsqueeze(2), sgn)

            with tc.tile_pool(name="vt_ps", bufs=1, space="PSUM") as vpp, \
                 tc.tile_pool(name="qk_ps", bufs=2, space="PSUM") as qpp, \
                 tc.tile_pool(name="v_ps", bufs=1, space="PSUM") as vsp:
                for g in range(4):
                    sl = slice(g * 512, (g + 1) * 512)
                    vt_ps = vpp.tile([F, 512], F32, tag="vtps",
                                     name=f"vtps{g}")
                    for cc in range(2):
                        nc.tensor.matmul(
                            vt_ps[:],
                            vw_r[:, cc * F: cc * F + F],
                            xt[:, cc * N + g * 512: cc * N + g * 512 + 512],
                            start=(cc == 0), stop=(cc == 1))
                    nc.vector.tensor_scalar_add(vt1[0:F, sl], vt_ps[:],
                                                vb_t[:])
                    qt_ps = qpp.tile([128, 512], F32, tag="qk",
                                     name=f"qtps{g}")
                    nc.tensor.matmul(qt_ps[:], qkw[:], vt1[:, sl],
                                     start=True, stop=True)
                    kt_ps = qpp.tile([128, 512], F32, tag="qk",
                                     name=f"ktps{g}")
                    nc.tensor.matmul(kt_ps[:], kkw[:], vt1[:, sl],
                                     start=True, stop=True)
                    # LeakyReLU with one PSUM read per instruction (the
                    # HW forbids two): Act scales 0.01*k into SBUF scratch,
                    # DVE maxes it against k.  Thanks to the negated odd
                    # lanes this yields [alpha; -beta]; Relu gives [P; -M].
                    gsl = slice(0, 512) if LOGIT8 else sl
                    nc.scalar.mul(kscr[g % 2][:], kt_ps[:], 0.01)
                    nc.vector.tensor_tensor(
                        ab_sp[:, gsl], kscr[g % 2][:], kt_ps[:], ALU.max)
                    nc.scalar.activation(pm_sp[:, gsl], qt_ps[:], AF.Relu)
                    if LOGIT8:
                        nc.vector.tensor_copy(pm8[:, sl],
                                              pm_sp[:, gsl].bitcast(F32))
                        nc.gpsimd.tensor_copy(abh16[:, sl],
                                              ab_sp[:, gsl].bitcast(F32))
                        for h in range(NH):
                            dqs = nc.scalar if (g + h) % 2 else nc.sync
                            dqs.dma_start(
                                abp[h][32 * h: 32 * h + 2, sl],
                                abh16[32 * h: 32 * h + 2, sl])

                    v_ps = vsp.tile([128, 4 * F], F32R, tag="vps",
                                    name=f"vps{g}")
                    for tt in range(4):
                        t = 4 * g + tt
                        nc.tensor.transpose(
                            v_ps[:, tt * F: tt * F + F],
                            vt1[0:F, t * 128: t * 128 + 128],
                            id_r[0:F, 0:F])
                    nc.vector.tensor_copy(
                        vp1[:].rearrange("p (t c) -> p t c", c=F + 1)
                            [:, 4 * g: 4 * g + 4, 0:F],
                        v_ps[:].bitcast(F32).rearrange(
                            "p (t c) -> p t c", c=F))
            # scatter each head's alpha/-beta pair into its padded K=128 lhsT
            if LOGIT8:
                pass  # abp scatter happens per group above
            else:
                for h in range(NH):
                    nc.sync.dma_start(abp[h][32 * h: 32 * h + 2, :],
                                      ab_sp[32 * h: 32 * h + 2, :])

        # ---------- software-pipelined main loop ----------
        # A shield pool pins the 4 banks the preamble just released, so the
        # first two lt tiles claim the never-used banks 4-7 and the first
        # logits aren't serialized behind the tail of the preamble.
        # Pool creation order fixes PSUM bank assignment (first-fit from
        # bank 0): acc and ht soak up the banks the preamble just released
        # (they are needed later / tolerate the wait), so the lt tiles land
        # on the four never-touched banks and the first logits run as soon
        # as their operands are ready.
        S = NB * NT
        from contextlib import ExitStack
        with ExitStack() as mstk:
            accp = mstk.enter_context(
                tc.tile_pool(name="acc_ps", bufs=ACC_BUFS, space="PSUM"))
            ltp = mstk.enter_context(
                tc.tile_pool(name="lt_ps", bufs=LT_BUFS, space="PSUM"))
            htp = mstk.enter_context(
                tc.tile_pool(name="ht_ps", bufs=2, space="PSUM")) \
                if HT_POOL else ltp
            etp = mstk.enter_context(tc.tile_pool(name="et_sb", bufs=6))
            postp = mstk.enter_context(tc.tile_pool(name="post_sb", bufs=2))
            lts, ets, accs, hsbs = {}, {}, {}, {}

            def abpm(h):
                return abp[h][:], (pm8[:] if LOGIT8 else pm_sp[:])

            def emit_logit(s):
                b, jc = divmod(s, NT)
                h, ib = blocks[b]
                ab_h, pm_h = abpm(h)
                lt = ltp.tile([128, 1024], F32, tag="lt", name=f"lt{s}")
                for hf in range(2):
                    nc.tensor.matmul(
                        lt[:, hf * 512: hf * 512 + 512],
                        ab_h[:, jc * 128: jc * 128 + 128],
                        pm_h[:, ib * 1024 + hf * 512:
                             ib * 1024 + hf * 512 + 512],
                        start=True, stop=True)
                lts[s] = lt

            def emit_exp(s):
                et = etp.tile([128, 1024], ACDT, tag="et", name=f"et{s}")
                nc.scalar.activation(et[:], lts[s][:], AF.Exp)
                ets[s] = et

            def emit_acc(s):
                b, jc = divmod(s, NT)
                if jc == 0:
                    accs[b] = accp.tile([F + 1, 1024], F32, tag="acc",
                                        name=f"acc{b}")
                for hf in range(2):
                    nc.tensor.matmul(
                        accs[b][:, hf * 512: hf * 512 + 512],
                        vp1[:, jc * (F + 1): (jc + 1) * (F + 1)],
                        ets[s][:, hf * 512: hf * 512 + 512],
                        start=(jc == 0), stop=(jc == NT - 1))

            def emit_hsb(b):
                hsb = postp.tile([F + 1, 1024], F32, tag="hsb",
                                 name=f"hsb{b}")
                if HSB_SPLIT:
                    nc.vector.tensor_copy(hsb[:, 0:512], accs[b][:, 0:512])
                    nc.scalar.copy(hsb[:, 512:1024], accs[b][:, 512:1024])
                else:
                    nc.vector.tensor_copy(hsb[:, 0:512], accs[b][:, 0:512])
                    nc.vector.tensor_copy(hsb[:, 512:1024],
                                          accs[b][:, 512:1024])
                hsbs[b] = hsb

            def emit_trans(b, t8):
                h, ib = blocks[b]
                # ht tiles default to sharing the lt tag (one rotation covers
                # both); HT_POOL gives them their own 2-bank pool instead
                ht = htp.tile([128, F + 1], F32,
                              tag=("ht" if HT_POOL else "lt"),
                              name=f"ht{b}_{t8}")
                nc.tensor.transpose(
                    ht[:], hsbs[b][:, t8 * 128: t8 * 128 + 128],
                    id65[:])
                rcp = postp.tile([128, 1], F32, tag="rcp",
                                 name=f"rcp{b}_{t8}")
                nc.vector.reciprocal(rcp[:], ht[:, F:F + 1])
                t = ib * 8 + t8
                nc.vector.tensor_scalar_mul(
                    obuf[:, t * (NH * F) + h * F: t * (NH * F) + h * F + F],
                    ht[:, 0:F], rcp[:])
                if b == NB - 1:
                    if t8 == 3:
                        emit_outdma(b, 0, 4)
                    elif t8 == 7:
                        emit_outdma(b, 4, 8)
                elif t8 == 7:
                    emit_outdma(b, 0, 8)

            def emit_outdma(b, t0, t1):
                h, ib = blocks[b]
                nc.sync.dma_start(
                    out_d[ib * 1024 + t0 * 128: ib * 1024 + t1 * 128,
                          h * F:(h + 1) * F]
                        .rearrange("(t p) c -> p t c", p=128),
                    obuf[:].rearrange("p (t c) -> p t c", c=NH * F)
                        [:, ib * 8 + t0: ib * 8 + t1, h * F:(h + 1) * F])

            if ABLATE >= 1:
                nc.vector.memset(obuf[:], 0.0)
            if ABLATE == 17:
                # full logit+exp+acc pipeline, postamble skipped
                for s in range(S + 3):
                    if s < S:
                        emit_logit(s)
                    if 1 <= s <= S:
                        emit_exp(s - 1)
                    if s >= 3:
                        emit_acc(s - 3)
                for b in range(NB):
                    emit_outdma(b, 0, 8)
                return
            if ABLATE in (2, 3, 4):
                # pure Act throughput: one logit tile, 128 exps off it.
                # 2: PSUM f32 -> SBUF f32r (the main-loop shape)
                # 3: PSUM f32 -> SBUF bf16
                # 4: SBUF f32 -> SBUF f32r
                emit_logit(0)
                sbsrc = None
                if ABLATE == 4:
                    sbsrc = etp.tile([128, 1024], F32, tag="sbsrc",
                                     name="sbsrc")
                    nc.vector.memset(sbsrc[:], 0.0)
                for s in range(S):
                    odt = mybir.dt.bfloat16 if ABLATE == 3 else F32R
                    et = etp.tile([128, 1024], odt, tag="et", name=f"et{s}")
                    src = sbsrc if ABLATE == 4 else lts[0]
                    nc.scalar.activation(et[:], src[:], AF.Exp)
                for b in range(NB):
                    emit_outdma(b, 0, 8)
                return
            if ABLATE in (7, 8, 9):
                # PE throughput probes, no postamble:
                # 7: acc matmuls only (K=128, M=65, N=512, f32r)
                # 8: logit matmuls only (K=2, M=128, N=512, f32r)
                # 9: acc matmuls only in bf16
                if ABLATE in (7, 9):
                    dt = mybir.dt.bfloat16 if ABLATE == 9 else F32R
                    et0 = etp.tile([128, 1024], dt, tag="et", name="et0")
                    if ABLATE == 9:
                        nc.vector.memset(et0[:], 1.0)
                        vp1b = etp.tile([128, NT * (F + 1)], dt, tag="vpb",
                                        name="vp1b")
                        nc.vector.tensor_copy(vp1b[:], vp1[:].bitcast(F32))
                        vsrc = vp1b
                    else:
                        nc.vector.memset(et0[:].bitcast(F32), 1.0)
                        vsrc = vp1
                    for s in range(S):
                        b, jc = divmod(s, NT)
                        if jc == 0:
                            accs[b] = accp.tile([F + 1, 1024], F32,
                                                tag="acc", name=f"acc{b}")
                        for hf in range(2):
                            nc.tensor.matmul(
                                accs[b][:, hf * 512: hf * 512 + 512],
                                vsrc[:, jc * (F + 1): (jc + 1) * (F + 1)],
                                et0[:, hf * 512: hf * 512 + 512],
                                start=(jc == 0), stop=(jc == NT - 1))
                else:
                    for s in range(S):
                        emit_logit(s)
                for b in range(NB):
                    emit_outdma(b, 0, 8)
                return
            if ABLATE == 10:
                # preamble + output DMA only
                for b in range(NB):
                    emit_outdma(b, 0, 8)
                return
            if ABLATE == 11:
                # ABL=6 with L/A emission batched in pairs (fewer PE
                # logit<->acc switches), accs ahead of logits in the queue
                et0 = etp.tile([128, 1024], F32R, tag="et", name="et0")
                nc.vector.memset(et0[:].bitcast(F32), 1.0)
                for s in range(S + 2):
                    if s % 2 == 0:
                        for a in (s - 2, s - 1):
                            if 0 <= a < S:
                                emit_acc(a)
                                if a % NT == NT - 1:
                                    emit_hsb(a // NT)
                                b_prev = a // NT - 1
                                jc = a % NT
                                if b_prev >= 0 and jc % 2 == 1:
                                    emit_trans(b_prev, jc // 2)
                        if s < S:
                            ets[s] = et0
                            ets[s + 1] = et0
                            emit_logit(s)
                            emit_logit(s + 1)
                for t8 in range(8):
                    emit_trans(NB - 1, t8)
                return
            if ABLATE == 13:
                # acc-only but every matmul uses a different vp1 chunk
                # (forces a weight change per matmul)
                et0 = etp.tile([128, 1024], F32R, tag="et", name="et0")
                nc.vector.memset(et0[:].bitcast(F32), 1.0)
                for s in range(S):
                    b, jc = divmod(s, NT)
                    if jc == 0:
                        accs[b] = accp.tile([F + 1, 1024], F32, tag="acc",
                                            name=f"acc{b}")
                    for hf in range(2):
                        w = ((jc + 8 * hf) % NT) * (F + 1)
                        nc.tensor.matmul(
                            accs[b][:, hf * 512: hf * 512 + 512],
                            vp1[:, w: w + F + 1],
                            et0[:, hf * 512: hf * 512 + 512],
                            start=(jc == 0), stop=(jc == NT - 1))
                for b in range(NB):
                    emit_outdma(b, 0, 8)
                return
            if ABLATE == 14:
                # logit-only, ONE [128,512] matmul per step (half the work
                # of ABL=8) — isolates per-instruction overhead
                for s in range(S):
                    b, jc = divmod(s, NT)
                    h, ib = blocks[b]
                    ab_h, pm_h = abpm(h)
                    lt = ltp.tile([128, 512], F32, tag="lt", name=f"lt{s}")
                    nc.tensor.matmul(
                        lt[:], ab_h[:, jc * 128: jc * 128 + 128],
                        pm_h[:, ib * 1024: ib * 1024 + 512],
                        start=True, stop=True)
                for b in range(NB):
                    emit_outdma(b, 0, 8)
                return
            if ABLATE == 15:
                # logit-only in bf16 (tests weight-load cost by dtype)
                BF16 = mybir.dt.bfloat16
                abb = etp.tile([128, N], BF16, tag="abb", name="abb")
                pmb = etp.tile([128, N], BF16, tag="pmb", name="pmb")
                nc.vector.tensor_copy(abb[:], ab_sp[:].bitcast(F32))
                nc.vector.tensor_copy(pmb[:], pm_sp[:].bitcast(F32))
                for s in range(S):
                    b, jc = divmod(s, NT)
                    h, ib = blocks[b]
                    h2 = min(h, 2)
                    lt = ltp.tile([128, 1024], F32, tag="lt", name=f"lt{s}")
                    for hf in range(2):
                        nc.tensor.matmul(
                            lt[:, hf * 512: hf * 512 + 512],
                            abb[32 * h2: 32 * h2 + 2,
                                jc * 128: jc * 128 + 128],
                            pmb[32 * h2: 32 * h2 + 2,
                                ib * 1024 + hf * 512:
                                ib * 1024 + hf * 512 + 512],
                            start=True, stop=True)
                for b in range(NB):
                    emit_outdma(b, 0, 8)
                return
            if ABLATE == 6:
                # full pipeline minus Act: logits + acc + postamble, with a
                # constant ones tile standing in for every exp result.
                et0 = etp.tile([128, 1024], F32R, tag="et", name="et0")
                nc.vector.memset(et0[:].bitcast(F32), 1.0)
                for s in range(S + 3):
                    if s < S:
                        emit_logit(s)
                    if 1 <= s <= S:
                        ets[s - 1] = et0
                    if s >= 3:
                        a = s - 3
                        emit_acc(a)
                        if a % NT == NT - 1:
                            emit_hsb(a // NT)
                        b_prev = a // NT - 1
                        jc = a % NT
                        if b_prev >= 0 and jc % 2 == 1:
                            emit_trans(b_prev, jc // 2)
                for t8 in range(8):
                    emit_trans(NB - 1, t8)
                return
            for s in range(S + 3):
                if s < S:
                    emit_logit(s)
                if 1 <= s <= S:
                    emit_exp(s - 1)
                if s >= 3 and ABLATE != 1:
                    a = s - 3
                    emit_acc(a)
                    if a % NT == NT - 1:
                        emit_hsb(a // NT)
                    b_prev = a // NT - 1
                    jc = a % NT
                    if b_prev >= 0 and jc % 2 == 1:
                        emit_trans(b_prev, jc // 2)
            if ABLATE != 1:
                for t8 in range(8):
                    emit_trans(NB - 1, t8)
            else:
                for b in range(NB):
                    emit_outdma(b, 0, 8)



# revision 48
# speedup vs baseline: 1.7339x; 1.0033x over previous
"""Multi-head graph attention (rank-2 LeakyReLU-softmax) Trainium2 kernel.

Reference computation (per batch b, head h):
    V = X @ vW + vb                       (N, F)
    q = V @ qW[:,h] + qb[h]               (N,)   per-node scalar
    k = V @ kW[:,h] + kb[h]               (N,)
    A_ij = softmax_j( LeakyReLU(q_i * k_j) )
    out[b,i,h,:] = sum_j A_ij V_j

Key identity used here: with P = max(q,0), M = min(q,0),
alpha = LeakyReLU(k) = max(k, 0.01k), beta = min(k, 0.01k),
    LeakyReLU(q_i * k_j) == alpha_j * P_i + beta_j * M_i      (exactly)
since for each i exactly one of P_i / M_i is nonzero.  So the N x N logit
matrix is a rank-2 outer product, built on the TensorEngine as a K=2
matmul (fp32r), exponentiated on the ScalarEngine straight out of PSUM,
and contracted against [V | 1] without the N x N matrix ever leaving the
chip.  The trailing all-ones column of Vp1 yields the softmax denominator
as row 64 of the same accumulation.

Sharding: core c -> batch b = c//2, heads h0 = 4*(c%2) .. h0+3.
"""

import numpy as np

import concourse.bacc as bacc
import concourse.tile as tile
import concourse.mybir as mybir
from concourse.bass_utils import run_bass_kernel_spmd

B, N, IN, F, H = 4, 2048, 256, 64, 8
NH = H // 2          # heads per core
NT = N // 128        # 16 i-tiles / j-chunks
F32 = mybir.dt.float32
F32R = mybir.dt.float32r
AF = mybir.ActivationFunctionType
ALU = mybir.AluOpType

N_CORES = 8
# packed param tensor columns: ident(128) | vW 2 chunks(128) | vb(1) | qw(4)
# | kw(4) | qb(4) | kb(4)
PRM_COLS = 128 + 128 + 1 + 4 + 4 + 4 + 4
_CACHE = {}
XCAST_DMA = False
import os as _os
ABLATE = int(_os.environ.get("ABL", "0"))  # 1: no acc/postamble (timing probe)
ACC_BF16 = int(_os.environ.get("ACCBF", "1"))  # bf16 vp1/et for the acc matmul
HSB_GP = int(_os.environ.get("HSBGP", "0"))    # gpsimd can't read PSUM (walrus)
LT_BUFS = int(_os.environ.get("LTBUFS", "2"))  # lt PSUM rotation depth
LOGIT8 = int(_os.environ.get("LOGIT8", "1"))   # bf16 hi/lo K=8-per-head logits
ACC_BUFS = int(_os.environ.get("ACCBUFS", "1"))  # acc PSUM tiles
HT_POOL = int(_os.environ.get("HTPOOL", "1"))  # 1: ht transposes own PSUM pool
HSB_SPLIT = int(_os.environ.get("HSBSPLIT", "0"))  # 1: hsb copy DVE+Act split


def build_nc(reps=1, unroll=False, version=4):
    """Build the kernel program.

    reps > 1 wraps the whole computation in a hardware For_i loop (all-engine
    barrier between iterations) so test.py can measure per-execution HW time
    by slope: (t(R) - t(1)) / (R - 1).  The graded kernel() path uses reps=1.
    """
    nc = bacc.Bacc("TRN2", target_bir_lowering=False, debug=False,
                   num_devices=N_CORES)
    xshape = [IN, N] if version >= 4 else [N, IN]
    X_d = nc.dram_tensor("X", xshape,
                         F32R if version >= 4 else F32,
                         kind="ExternalInput")
    vW_d = nc.dram_tensor("vW", [IN, F], F32, kind="ExternalInput")
    vb_d = nc.dram_tensor("vb", [F], F32, kind="ExternalInput")
    qw_d = nc.dram_tensor("qw", [F, NH], F32, kind="ExternalInput")
    kw_d = nc.dram_tensor("kw", [F, NH], F32, kind="ExternalInput")
    qb_d = nc.dram_tensor("qb", [NH], F32, kind="ExternalInput")
    kb_d = nc.dram_tensor("kb", [NH], F32, kind="ExternalInput")
    id_d = nc.dram_tensor("ident", [128, 128], F32, kind="ExternalInput")
    prm_d = nc.dram_tensor("prm", [128, PRM_COLS], F32, kind="ExternalInput")
    out_d = nc.dram_tensor("out", [N, NH * F], F32, kind="ExternalOutput")

    body = {1: _emit_body, 2: _emit_body_v2, 3: _emit_body_v3,
            4: _emit_body_v4}[version]
    extra = {"prm_d": prm_d} if version >= 4 else {}
    with tile.TileContext(nc) as tc:
        from contextlib import ExitStack
        with ExitStack() as rep_ctx:
            if reps > 1 and not unroll:
                rep_ctx.enter_context(tc.For_i(0, reps))
            for _ in range(reps if unroll else 1):
                body(nc, tc, X_d, vW_d, vb_d, qw_d, kw_d, qb_d, kb_d,
                     id_d, out_d, **extra)
    nc.compile()
    return nc


def _emit_body_v2(nc, tc, X_d, vW_d, vb_d, qw_d, kw_d, qb_d, kb_d, id_d,
                  out_d):
    """Software-pipelined main loop.

    Per (head, i-block) "block" (NB = NH*2 of them), per j-chunk step:
      PE:  logit matmul (K=2 rank-2 outer product) -> lt PSUM [128,1024]
      Act: exp straight out of PSUM -> et SBUF (the ONLY Act work)
      PE:  acc matmul [V|1]^T @ et -> acc PSUM [65,1024] accumulated over 16 j
    Steps are emitted with a 1-step skew (logit(s) before acc(s-1)) so PE's
    in-order queue never parks an exp-dependent acc in front of independent
    logit work.  Postamble (PE transpose + DVE normalize into an SBUF staging
    buffer) is interleaved into the following block's steps; output leaves the
    chip in one final DMA.
    """
    NB = NH * 2
    blocks = [(h, ib) for h in range(NH) for ib in range(2)]
    with tc.tile_pool(name="persist", bufs=1) as pp:
        ident = pp.tile([128, 128], F32)
        nc.sync.dma_start(ident[:], id_d[:])
        id_r = pp.tile([128, 128], F32R)
        nc.vector.tensor_copy(id_r[:], ident[:])
        vt_sb = pp.tile([F, N], F32R)         # V^T, bias folded
        qt = pp.tile([NH, N], F32)
        kt = pp.tile([NH, N], F32)
        ab_all = pp.tile([2, NH * N], F32R)   # row0 alpha, row1 beta; head h at cols h*N
        pm_all = pp.tile([2, NH * N], F32R)   # row0 P, row1 M
        vp1 = pp.tile([128, NT * (F + 1)], F32R)   # [V | 1] per j-tile
        obuf = pp.tile([128, NT * NH * F], F32)    # staged output

        # ---------- preamble: X^T, V^T, q/k ----------
        with tc.tile_pool(name="pre_sb", bufs=1) as sp:
            xsb = sp.tile([128, NT * IN], F32)
            nc.sync.dma_start(
                xsb[:].rearrange("p (t c) -> p t c", t=NT),
                X_d[:].rearrange("(t p) c -> p t c", p=128))
            vwsb = sp.tile([128, 128], F32)
            nc.sync.dma_start(
                vwsb[:].rearrange("p (t f) -> p t f", t=2),
                vW_d[:].rearrange("(t p) f -> p t f", p=128))
            vb_t = sp.tile([F, 1], F32)
            nc.sync.dma_start(vb_t[:], vb_d[:].unsqueeze(1))
            qw_t = sp.tile([F, NH], F32)
            nc.sync.dma_start(qw_t[:], qw_d[:])
            kw_t = sp.tile([F, NH], F32)
            nc.sync.dma_start(kw_t[:], kw_d[:])
            qb_t = sp.tile([NH, 1], F32)
            nc.sync.dma_start(qb_t[:], qb_d[:].unsqueeze(1))
            kb_t = sp.tile([NH, 1], F32)
            nc.sync.dma_start(kb_t[:], kb_d[:].unsqueeze(1))

            xt = sp.tile([128, 2 * N], F32R)  # X^T: chunk cc at cc*N
            vw_r = sp.tile([128, 128], F32R)
            nc.vector.tensor_copy(vw_r[:], vwsb[:])
            qw_r = sp.tile([F, NH], F32R)
            nc.vector.tensor_copy(qw_r[:], qw_t[:])
            kw_r = sp.tile([F, NH], F32R)
            nc.vector.tensor_copy(kw_r[:], kw_t[:])

            with tc.tile_pool(name="pre_ps", bufs=2, space="PSUM") as xp:
                for t in range(NT):
                    for cc in range(2):
                        tp = xp.tile([128, 128], F32)
                        nc.tensor.transpose(
                            tp[:], xsb[:, t * IN + cc * 128:
                                       t * IN + cc * 128 + 128], ident[:])
                        nc.vector.tensor_copy(
                            xt[:, cc * N + t * 128: cc * N + t * 128 + 128],
                            tp[:])

            with tc.tile_pool(name="vt_ps", bufs=1, space="PSUM") as vpp:
                vt_ps = vpp.tile([F, N], F32)
                for nb in range(4):
                    for cc in range(2):
                        nc.tensor.matmul(
                            vt_ps[:, nb * 512: nb * 512 + 512],
                            vw_r[:, cc * F: cc * F + F],
                            xt[:, cc * N + nb * 512: cc * N + nb * 512 + 512],
                            start=(cc == 0), stop=(cc == 1))
                nc.vector.tensor_scalar_add(vt_sb[:], vt_ps[:], vb_t[:])

            with tc.tile_pool(name="qk_ps", bufs=1, space="PSUM") as qpp:
                qt_ps = qpp.tile([NH, N], F32)
                kt_ps = qpp.tile([NH, N], F32)
                for nb in range(4):
                    nc.tensor.matmul(
                        qt_ps[:, nb * 512: nb * 512 + 512], qw_r[:],
                        vt_sb[:, nb * 512: nb * 512 + 512],
                        start=True, stop=True)
                    nc.tensor.matmul(
                        kt_ps[:, nb * 512: nb * 512 + 512], kw_r[:],
                        vt_sb[:, nb * 512: nb * 512 + 512],
                        start=True, stop=True)
                nc.vector.tensor_scalar_add(qt[:], qt_ps[:], qb_t[:])
                nc.vector.tensor_scalar_add(kt[:], kt_ps[:], kb_t[:])

            # per-head vectors, written [alpha0..3 | beta0..3] then paired
            abq = sp.tile([2 * NH, N], F32R)
            pmq = sp.tile([2 * NH, N], F32R)
            nc.vector.scalar_tensor_tensor(abq[0:NH, :], kt[:], 0.01, kt[:],
                                           ALU.mult, ALU.max)
            nc.vector.scalar_tensor_tensor(abq[NH:2 * NH, :], kt[:], 0.01,
                                           kt[:], ALU.mult, ALU.min)
            nc.vector.tensor_scalar_max(pmq[0:NH, :], qt[:], 0.0)
            nc.vector.tensor_scalar_min(pmq[NH:2 * NH, :], qt[:], 0.0)
            nc.sync.dma_start(
                ab_all[0:1, :].rearrange("o (h n) -> o h n", h=NH),
                abq[0:NH, :].unsqueeze(0))
            nc.sync.dma_start(
                ab_all[1:2, :].rearrange("o (h n) -> o h n", h=NH),
                abq[NH:2 * NH, :].unsqueeze(0))
            nc.sync.dma_start(
                pm_all[0:1, :].rearrange("o (h n) -> o h n", h=NH),
                pmq[0:NH, :].unsqueeze(0))
            nc.sync.dma_start(
                pm_all[1:2, :].rearrange("o (h n) -> o h n", h=NH),
                pmq[NH:2 * NH, :].unsqueeze(0))

        # ---------- Vp1 = [V | 1] per j-tile ----------
        nc.vector.memset(vp1[:].bitcast(F32), 1.0)
        with tc.tile_pool(name="v_ps", bufs=2, space="PSUM") as vp:
            for t in range(NT):
                v_ps = vp.tile([128, F], F32R)
                nc.tensor.transpose(
                    v_ps[:], vt_sb[:, t * 128: t * 128 + 128],
                    id_r[0:F, 0:F])
                nc.vector.tensor_copy(
                    vp1[:, t * (F + 1): t * (F + 1) + F], v_ps[:])

        # ---------- software-pipelined main loop ----------
        S = NB * NT  # 128 steps
        with tc.tile_pool(name="lt_ps", bufs=2, space="PSUM") as ltp, \
             tc.tile_pool(name="acc_ps", bufs=1, space="PSUM") as accp, \
             tc.tile_pool(name="ht_ps", bufs=2, space="PSUM") as htp, \
             tc.tile_pool(name="et_sb", bufs=4) as etp, \
             tc.tile_pool(name="post_sb", bufs=2) as postp:
            lts, ets, accs, hsbs = {}, {}, {}, {}

            def emit_logit(s):
                b, jc = divmod(s, NT)
                h, ib = blocks[b]
                lt = ltp.tile([128, 1024], F32, tag="lt", name=f"lt{s}")
                for hf in range(2):
                    nc.tensor.matmul(
                        lt[:, hf * 512: hf * 512 + 512],
                        ab_all[:, h * N + jc * 128: h * N + jc * 128 + 128],
                        pm_all[:, h * N + ib * 1024 + hf * 512:
                               h * N + ib * 1024 + hf * 512 + 512],
                        start=True, stop=True)
                lts[s] = lt

            def emit_exp(s):
                et = etp.tile([128, 1024], F32R, tag="et", name=f"et{s}")
                nc.scalar.activation(et[:], lts[s][:], AF.Exp)
                ets[s] = et

            def emit_acc(s):
                b, jc = divmod(s, NT)
                if jc == 0:
                    accs[b] = accp.tile([F + 1, 1024], F32, tag="acc", name=f"acc{b}")
                for hf in range(2):
                    nc.tensor.matmul(
                        accs[b][:, hf * 512: hf * 512 + 512],
                        vp1[:, jc * (F + 1): (jc + 1) * (F + 1)],
                        ets[s][:, hf * 512: hf * 512 + 512],
                        start=(jc == 0), stop=(jc == NT - 1))

            def emit_hsb(b):
                hsb = postp.tile([F + 1, 1024], F32, tag="hsb", name=f"hsb{b}")
                nc.vector.tensor_copy(hsb[:], accs[b][:])
                hsbs[b] = hsb

            def emit_trans(b, t8):
                h, ib = blocks[b]
                ht = htp.tile([128, F + 1], F32, tag="ht", name=f"ht{b}_{t8}")
                nc.tensor.transpose(
                    ht[:], hsbs[b][:, t8 * 128: t8 * 128 + 128],
                    id65[:])
                rcp = postp.tile([128, 1], F32, tag="rcp", name=f"rcp{b}_{t8}")
                nc.vector.reciprocal(rcp[:], ht[:, F:F + 1])
                t = ib * 8 + t8
                nc.vector.tensor_scalar_mul(
                    obuf[:, t * (NH * F) + h * F: t * (NH * F) + h * F + F],
                    ht[:, 0:F], rcp[:])

            for s in range(S + 1):
                if s < S:
                    emit_logit(s)
                if s >= 1:
                    emit_exp(s - 1)
                    emit_acc(s - 1)
                    if (s - 1) % NT == NT - 1:
                        emit_hsb((s - 1) // NT)
                    # spread previous block's 8 transposes over this block
                    b_prev = s // NT - 1
                    jc = s % NT
                    if b_prev >= 0 and s < S and jc % 2 == 1:
                        emit_trans(b_prev, jc // 2)
            for t8 in range(8):  # drain last block
                emit_trans(NB - 1, t8)

        nc.sync.dma_start(
            out_d[:].rearrange("(t p) c -> p t c", p=128),
            obuf[:].rearrange("p (t c) -> p t c", t=NT))


def _emit_body(nc, tc, X_d, vW_d, vb_d, qw_d, kw_d, qb_d, kb_d, id_d, out_d):
    if True:
        with tc.tile_pool(name="persist", bufs=1) as pp:
            ident = pp.tile([128, 128], F32)
            nc.sync.dma_start(ident[:], id_d[:])
            id_r = pp.tile([128, 128], F32R)
            nc.vector.tensor_copy(id_r[:], ident[:])
            vt_sb = pp.tile([F, N], F32R)         # V^T, bias folded
            qt = pp.tile([NH, N], F32)
            kt = pp.tile([NH, N], F32)
            ab_hs = [pp.tile([2, N], F32R, name=f"abh{h}", tag=f"ab{h}") for h in range(NH)]
            pm_hs = [pp.tile([2, N], F32R, name=f"pmh{h}", tag=f"pm{h}") for h in range(NH)]
            vp1 = pp.tile([128, NT * (F + 1)], F32R)   # [V | 1] per j-tile

            # ---------- preamble: X^T, V^T, q/k ----------
            with tc.tile_pool(name="pre_sb", bufs=1) as sp:
                xsb = sp.tile([128, NT * IN], F32)
                nc.sync.dma_start(
                    xsb[:].rearrange("p (t c) -> p t c", t=NT),
                    X_d[:].rearrange("(t p) c -> p t c", p=128))
                vwsb = sp.tile([128, 128], F32)
                nc.sync.dma_start(
                    vwsb[:].rearrange("p (t f) -> p t f", t=2),
                    vW_d[:].rearrange("(t p) f -> p t f", p=128))
                vb_t = sp.tile([F, 1], F32)
                nc.sync.dma_start(vb_t[:], vb_d[:].unsqueeze(1))
                qw_t = sp.tile([F, NH], F32)
                nc.sync.dma_start(qw_t[:], qw_d[:])
                kw_t = sp.tile([F, NH], F32)
                nc.sync.dma_start(kw_t[:], kw_d[:])
                qb_t = sp.tile([NH, 1], F32)
                nc.sync.dma_start(qb_t[:], qb_d[:].unsqueeze(1))
                kb_t = sp.tile([NH, 1], F32)
                nc.sync.dma_start(kb_t[:], kb_d[:].unsqueeze(1))

                xt = sp.tile([128, 2 * N], F32R)  # X^T: chunk cc at cc*N
                vw_r = sp.tile([128, 128], F32R)
                nc.vector.tensor_copy(vw_r[:], vwsb[:])
                qw_r = sp.tile([F, NH], F32R)
                nc.vector.tensor_copy(qw_r[:], qw_t[:])
                kw_r = sp.tile([F, NH], F32R)
                nc.vector.tensor_copy(kw_r[:], kw_t[:])

                with tc.tile_pool(name="pre_ps", bufs=2, space="PSUM") as xp:
                    for t in range(NT):
                        for cc in range(2):
                            tp = xp.tile([128, 128], F32)
                            nc.tensor.transpose(
                                tp[:], xsb[:, t * IN + cc * 128:
                                           t * IN + cc * 128 + 128], ident[:])
                            nc.vector.tensor_copy(
                                xt[:, cc * N + t * 128: cc * N + t * 128 + 128],
                                tp[:])

                with tc.tile_pool(name="vt_ps", bufs=1, space="PSUM") as vpp:
                    vt_ps = vpp.tile([F, N], F32)
                    for nb in range(4):
                        for cc in range(2):
                            nc.tensor.matmul(
                                vt_ps[:, nb * 512: nb * 512 + 512],
                                vw_r[:, cc * F: cc * F + F],
                                xt[:, cc * N + nb * 512: cc * N + nb * 512 + 512],
                                start=(cc == 0), stop=(cc == 1))
                    nc.vector.tensor_scalar_add(vt_sb[:], vt_ps[:], vb_t[:])

                with tc.tile_pool(name="qk_ps", bufs=1, space="PSUM") as qpp:
                    qt_ps = qpp.tile([NH, N], F32)
                    kt_ps = qpp.tile([NH, N], F32)
                    for nb in range(4):
                        nc.tensor.matmul(
                            qt_ps[:, nb * 512: nb * 512 + 512], qw_r[:],
                            vt_sb[:, nb * 512: nb * 512 + 512],
                            start=True, stop=True)
                        nc.tensor.matmul(
                            kt_ps[:, nb * 512: nb * 512 + 512], kw_r[:],
                            vt_sb[:, nb * 512: nb * 512 + 512],
                            start=True, stop=True)
                    nc.vector.tensor_scalar_add(qt[:], qt_ps[:], qb_t[:])
                    nc.vector.tensor_scalar_add(kt[:], kt_ps[:], kb_t[:])

            # ---------- per-head vectors (fp32r) ----------
            with tc.tile_pool(name="vec_sb", bufs=1) as vs:
                a4 = vs.tile([NH, N], F32R)
                b4 = vs.tile([NH, N], F32R)
                p4 = vs.tile([NH, N], F32R)
                m4 = vs.tile([NH, N], F32R)
                nc.vector.scalar_tensor_tensor(a4[:], kt[:], 0.01, kt[:],
                                               ALU.mult, ALU.max)
                nc.vector.scalar_tensor_tensor(b4[:], kt[:], 0.01, kt[:],
                                               ALU.mult, ALU.min)
                nc.vector.tensor_scalar_max(p4[:], qt[:], 0.0)
                nc.vector.tensor_scalar_min(m4[:], qt[:], 0.0)
                for h in range(NH):
                    nc.sync.dma_start(ab_hs[h][0:1, :], a4[h:h + 1, :])
                    nc.sync.dma_start(ab_hs[h][1:2, :], b4[h:h + 1, :])
                    nc.sync.dma_start(pm_hs[h][0:1, :], p4[h:h + 1, :])
                    nc.sync.dma_start(pm_hs[h][1:2, :], m4[h:h + 1, :])

            # ---------- Vp1 = [V | 1] per j-tile ----------
            nc.vector.memset(vp1[:].bitcast(F32), 1.0)
            with tc.tile_pool(name="v_ps", bufs=2, space="PSUM") as vp:
                for t in range(NT):
                    v_ps = vp.tile([128, F], F32R)
                    nc.tensor.transpose(
                        v_ps[:], vt_sb[:, t * 128: t * 128 + 128],
                        id_r[0:F, 0:F])
                    nc.vector.tensor_copy(
                        vp1[:, t * (F + 1): t * (F + 1) + F], v_ps[:])

            # ---------- main loop ----------
            hsbs = {}
            with tc.tile_pool(name="lt_ps", bufs=3, space="PSUM") as ltp, \
                 tc.tile_pool(name="acc_ps", bufs=1, space="PSUM") as accp, \
                 tc.tile_pool(name="et_sb", bufs=3) as etp:
                for h in range(NH):
                    ab_h = ab_hs[h][:]
                    pm_h = pm_hs[h][:]
                    for ib in range(2):
                        acc = accp.tile([F + 1, 1024], F32, tag="acc")
                        for jc in range(NT):
                            lt = ltp.tile([128, 1024], F32, tag="lt", name=f"lt{s}")
                            for hf in range(2):
                                nc.tensor.matmul(
                                    lt[:, hf * 512: hf * 512 + 512],
                                    ab_h[:, jc * 128: jc * 128 + 128],
                                    pm_h[:, ib * 1024 + hf * 512:
                                         ib * 1024 + hf * 512 + 512],
                                    start=True, stop=True)
                            et = etp.tile([128, 1024], F32R, tag="et", name=f"et{s}")
                            nc.scalar.activation(et[:], lt[:], AF.Exp)
                            for hf in range(2):
                                nc.tensor.matmul(
                                    acc[:, hf * 512: hf * 512 + 512],
                                    vp1[:, jc * (F + 1): (jc + 1) * (F + 1)],
                                    et[:, hf * 512: hf * 512 + 512],
                                    start=(jc == 0), stop=(jc == NT - 1))
                        hsb = pp.tile([F + 1, 1024], F32, name=f"hsb{h}_{ib}",
                                      tag=f"hsb{h}_{ib}")
                        nc.vector.tensor_copy(hsb[:], acc[:])
                        hsbs[(h, ib)] = hsb

            # ---------- postamble: transpose + normalize + store ----------
            with tc.tile_pool(name="ht_ps", bufs=4, space="PSUM") as htp, \
                 tc.tile_pool(name="post_sb", bufs=4) as postp:
                for h in range(NH):
                    for ib in range(2):
                        hsb = hsbs[(h, ib)]
                        for t8 in range(8):
                            ht = htp.tile([128, F + 1], F32, tag="ht")
                            nc.tensor.transpose(
                                ht[:], hsb[:, t8 * 128: t8 * 128 + 128],
                                ident[0:F + 1, 0:F + 1])
                            rcp = postp.tile([128, 1], F32, tag="rcp", name=f"rcp{b}_{t8}")
                            nc.vector.reciprocal(rcp[:], ht[:, F:F + 1])
                            ob = postp.tile([128, F], F32, tag="ob")
                            nc.vector.tensor_scalar_mul(ob[:], ht[:, 0:F], rcp[:])
                            r0 = ib * 1024 + t8 * 128
                            nc.sync.dma_start(
                                out_d[r0:r0 + 128, h * F: h * F + F], ob[:])


def _get_nc():
    if "nc" not in _CACHE:
        _CACHE["nc"] = build_nc()
    return _CACHE["nc"]


def make_in_maps(X, vW, vb, qW, qb, kW, kb):
    ident = np.eye(128, dtype=np.float32)
    in_maps = []
    for c in range(N_CORES):
        b, h0 = c // 2, NH * (c % 2)
        qwc = np.ascontiguousarray(qW[:, h0:h0 + NH])
        kwc = np.ascontiguousarray(kW[:, h0:h0 + NH])
        qbc = np.ascontiguousarray(qb[h0:h0 + NH])
        kbc = np.ascontiguousarray(kb[h0:h0 + NH])
        prm = np.zeros((128, PRM_COLS), dtype=np.float32)
        prm[:, 0:128] = ident
        prm[:, 128:256] = vW.reshape(2, 128, F).transpose(1, 0, 2).reshape(128, 128)
        prm[0:F, 256] = vb
        prm[0:F, 257:261] = qwc
        prm[0:F, 261:265] = kwc
        prm[0:1, 265:269] = qbc
        prm[0:1, 269:273] = kbc
        in_maps.append({
            "X": np.ascontiguousarray(X[b].T),
            "vW": np.ascontiguousarray(vW),
            "vb": np.ascontiguousarray(vb),
            "qw": qwc,
            "kw": kwc,
            "qb": qbc,
            "kb": kbc,
            "ident": ident,
            "prm": prm,
        })
    return in_maps


def assemble(results):
    full = np.empty((B, N, H * F), dtype=np.float32)
    for c in range(N_CORES):
        b, h0 = c // 2, NH * (c % 2)
        full[b][:, h0 * F:(h0 + NH) * F] = results[c]["out"]
    return full


def kernel(X, vW, vb, qW, qb, kW, kb):
    X, vW, vb = np.asarray(X), np.asarray(vW), np.asarray(vb)
    qW, qb, kW, kb = np.asarray(qW), np.asarray(qb), np.asarray(kW), np.asarray(kb)
    nc = _get_nc()
    res = run_bass_kernel_spmd(nc, make_in_maps(X, vW, vb, qW, qb, kW, kb),
                               list(range(N_CORES)))
    return assemble(res.results)


def _emit_body_v3(nc, tc, X_d, vW_d, vb_d, qw_d, kw_d, qb_d, kb_d, id_d,
                  out_d):
    """v2 main loop + pipelined preamble and per-block output DMAs.

    Preamble works in 4 node-groups of 512: X DMA group g -> 8 PE transposes
    into a [128,512] PSUM tile -> 2 wide copies (DVE/Pool) -> V^T matmul ->
    bias-add -> q/k matmul -> bias-add -> alpha/beta/P/M chunk -> pack DMA.
    First exp can start after group 0's chain (~7us) instead of after the
    whole preamble.  Act engine does exp ONLY (table preloaded at t=0).
    """
    NB = NH * 2
    blocks = [(h, ib) for h in range(NH) for ib in range(2)]
    with tc.tile_pool(name="persist", bufs=1) as pp:
        # Exp activation-table preload, before anything else on Act.
        zz = pp.tile([1, 2], F32R)
        nc.vector.memset(zz[:].bitcast(F32), 0.0)
        nc.scalar.activation(zz[:], zz[:], AF.Exp)

        id65 = pp.tile([F + 1, F + 1], F32)   # identity for postamble transposes
        id_r = pp.tile([128, 128], F32R)
        vt_sb = pp.tile([F, N], F32R)         # V^T, bias folded
        qt = pp.tile([NH, N], F32)
        kt = pp.tile([NH, N], F32)
        ab_all = pp.tile([2, NH * N], F32R)   # row0 alpha, row1 beta
        pm_all = pp.tile([2, NH * N], F32R)   # row0 P, row1 M
        vp1 = pp.tile([128, NT * (F + 1)], F32R)
        obuf = pp.tile([128, NT * NH * F], F32)
        nc.vector.memset(vp1[:].bitcast(F32), 1.0)

        with tc.tile_pool(name="pre_sb", bufs=1) as sp:
            xsb = sp.tile([128, NT * IN], F32)
            vwsb = sp.tile([128, 128], F32)
            vb_t = sp.tile([F, 1], F32)
            qw_t = sp.tile([F, NH], F32)
            kw_t = sp.tile([F, NH], F32)
            qb_t = sp.tile([NH, 1], F32)
            kb_t = sp.tile([NH, 1], F32)
            abq = sp.tile([2 * NH, N], F32R)  # rows 0-3 alpha, 4-7 beta
            pmq = sp.tile([2 * NH, N], F32R)  # rows 0-3 P, 4-7 M

            # input DMAs: ident+vW first (needed by transposes / V^T), then
            # X in 4 groups; small params via other queues.
            nc.sync.dma_start(ident[:], id_d[:])
            nc.sync.dma_start(
                vwsb[:].rearrange("p (t f) -> p t f", t=2),
                vW_d[:].rearrange("(t p) f -> p t f", p=128))
            for g in range(4):
                nc.sync.dma_start(
                    xsb[:, g * 4 * IN:(g + 1) * 4 * IN]
                        .rearrange("p (t c) -> p t c", t=4),
                    X_d[g * 512:(g + 1) * 512, :]
                        .rearrange("(t p) c -> p t c", p=128))
            nc.scalar.dma_start(vb_t[:], vb_d[:].unsqueeze(1))
            nc.scalar.dma_start(qw_t[:], qw_d[:])
            nc.scalar.dma_start(kw_t[:], kw_d[:])
            nc.gpsimd.dma_start(qb_t[:], qb_d[:].unsqueeze(1))
            nc.gpsimd.dma_start(kb_t[:], kb_d[:].unsqueeze(1))

            xt = sp.tile([128, 2 * N], F32R)  # X^T: chunk cc at cc*N
            vw_r = sp.tile([128, 128], F32R)
            nc.vector.tensor_copy(id_r[:], ident[:])
            nc.gpsimd.tensor_copy(vw_r[:], vwsb[:])
            qw_r = sp.tile([F, NH], F32R)
            nc.vector.tensor_copy(qw_r[:], qw_t[:])
            kw_r = sp.tile([F, NH], F32R)
            nc.vector.tensor_copy(kw_r[:], kw_t[:])

            with tc.tile_pool(name="tp_ps", bufs=2, space="PSUM") as xp, \
                 tc.tile_pool(name="vt_ps", bufs=1, space="PSUM") as vpp, \
                 tc.tile_pool(name="qk_ps", bufs=2, space="PSUM") as qpp, \
                 tc.tile_pool(name="v_ps", bufs=1, space="PSUM") as vsp:
                for g in range(4):
                    # X^T for this group's 4 node-tiles (both 128-col chunks)
                    for cc in range(2):
                        tp = xp.tile([128, 512], F32, tag="tp",
                                     name=f"tp{g}_{cc}")
                        for tt in range(4):
                            t = 4 * g + tt
                            nc.tensor.transpose(
                                tp[:, tt * 128: tt * 128 + 128],
                                xsb[:, t * IN + cc * 128:
                                    t * IN + cc * 128 + 128], ident[:])
                        nc.scalar.copy(
                            xt[:, cc * N + g * 512: cc * N + g * 512 + 512],
                            tp[:])
                    # V^T chunk
                    vt_ps = vpp.tile([F, 512], F32, tag="vtps",
                                     name=f"vtps{g}")
                    for cc in range(2):
                        nc.tensor.matmul(
                            vt_ps[:],
                            vw_r[:, cc * F: cc * F + F],
                            xt[:, cc * N + g * 512: cc * N + g * 512 + 512],
                            start=(cc == 0), stop=(cc == 1))
                    nc.vector.tensor_scalar_add(
                        vt_sb[:, g * 512:(g + 1) * 512], vt_ps[:], vb_t[:])
                    # q / k chunks
                    qt_ps = qpp.tile([NH, 512], F32, tag="qk",
                                     name=f"qtps{g}")
                    nc.tensor.matmul(qt_ps[:], qw_r[:],
                                     vt_sb[:, g * 512: g * 512 + 512],
                                     start=True, stop=True)
                    kt_ps = qpp.tile([NH, 512], F32, tag="qk",
                                     name=f"ktps{g}")
                    nc.tensor.matmul(kt_ps[:], kw_r[:],
                                     vt_sb[:, g * 512: g * 512 + 512],
                                     start=True, stop=True)
                    sl = slice(g * 512, (g + 1) * 512)
                    nc.vector.tensor_scalar_add(qt[:, sl], qt_ps[:], qb_t[:])
                    nc.gpsimd.tensor_scalar_add(kt[:, sl], kt_ps[:], kb_t[:])
                    # alpha/beta (from k), P/M (from q) for this chunk
                    nc.vector.scalar_tensor_tensor(
                        abq[0:NH, sl], kt[:, sl], 0.01, kt[:, sl],
                        ALU.mult, ALU.max)
                    nc.gpsimd.scalar_tensor_tensor(
                        abq[NH:2 * NH, sl], kt[:, sl], 0.01, kt[:, sl],
                        ALU.mult, ALU.min)
                    nc.vector.tensor_scalar_max(pmq[0:NH, sl], qt[:, sl], 0.0)
                    nc.gpsimd.tensor_scalar_min(pmq[NH:2 * NH, sl],
                                                qt[:, sl], 0.0)
                    # pack into [2, NH*N] layout (head-major columns)
                    for row in range(2):
                        nc.gpsimd.dma_start(
                            ab_all[row:row + 1, :]
                                .rearrange("o (h n) -> o h n", h=NH)
                                [:, :, g * 512:(g + 1) * 512],
                            abq[row * NH:(row + 1) * NH, sl].unsqueeze(0))
                        nc.gpsimd.dma_start(
                            pm_all[row:row + 1, :]
                                .rearrange("o (h n) -> o h n", h=NH)
                                [:, :, g * 512:(g + 1) * 512],
                            pmq[row * NH:(row + 1) * NH, sl].unsqueeze(0))
                    # Vp1 tiles for this group
                    v_ps = vsp.tile([128, 4 * F], F32R, tag="vps",
                                    name=f"vps{g}")
                    for tt in range(4):
                        t = 4 * g + tt
                        nc.tensor.transpose(
                            v_ps[:, tt * F: tt * F + F],
                            vt_sb[:, t * 128: t * 128 + 128],
                            id_r[0:F, 0:F])
                    eng = nc.vector if g % 2 == 0 else nc.gpsimd
                    eng.tensor_copy(
                        vp1[:].rearrange("p (t c) -> p t c", c=F + 1)
                            [:, 4 * g: 4 * g + 4, 0:F],
                        v_ps[:].rearrange("p (t c) -> p t c", c=F))

        # ---------- software-pipelined main loop ----------
        S = NB * NT  # 128 steps
        with tc.tile_pool(name="lt_ps", bufs=2, space="PSUM") as ltp, \
             tc.tile_pool(name="acc_ps", bufs=1, space="PSUM") as accp, \
             tc.tile_pool(name="ht_ps", bufs=2, space="PSUM") as htp, \
             tc.tile_pool(name="et_sb", bufs=4) as etp, \
             tc.tile_pool(name="post_sb", bufs=2) as postp:
            lts, ets, accs, hsbs = {}, {}, {}, {}

            def emit_logit(s):
                b, jc = divmod(s, NT)
                h, ib = blocks[b]
                lt = ltp.tile([128, 1024], F32, tag="lt", name=f"lt{s}")
                for hf in range(2):
                    nc.tensor.matmul(
                        lt[:, hf * 512: hf * 512 + 512],
                        ab_all[:, h * N + jc * 128: h * N + jc * 128 + 128],
                        pm_all[:, h * N + ib * 1024 + hf * 512:
                               h * N + ib * 1024 + hf * 512 + 512],
                        start=True, stop=True)
                lts[s] = lt

            def emit_exp(s):
                et = etp.tile([128, 1024], F32R, tag="et", name=f"et{s}")
                nc.scalar.activation(et[:], lts[s][:], AF.Exp)
                ets[s] = et

            def emit_acc(s):
                b, jc = divmod(s, NT)
                if jc == 0:
                    accs[b] = accp.tile([F + 1, 1024], F32, tag="acc",
                                        name=f"acc{b}")
                for hf in range(2):
                    nc.tensor.matmul(
                        accs[b][:, hf * 512: hf * 512 + 512],
                        vp1[:, jc * (F + 1): (jc + 1) * (F + 1)],
                        ets[s][:, hf * 512: hf * 512 + 512],
                        start=(jc == 0), stop=(jc == NT - 1))

            def emit_hsb(b):
                hsb = postp.tile([F + 1, 1024], F32, tag="hsb",
                                 name=f"hsb{b}")
                nc.vector.tensor_copy(hsb[:, 0:512], accs[b][:, 0:512])
                nc.gpsimd.tensor_copy(hsb[:, 512:1024], accs[b][:, 512:1024])
                hsbs[b] = hsb

            def emit_trans(b, t8):
                h, ib = blocks[b]
                ht = htp.tile([128, F + 1], F32, tag="ht", name=f"ht{b}_{t8}")
                nc.tensor.transpose(
                    ht[:], hsbs[b][:, t8 * 128: t8 * 128 + 128],
                    id65[:])
                rcp = postp.tile([128, 1], F32, tag="rcp",
                                 name=f"rcp{b}_{t8}")
                nc.vector.reciprocal(rcp[:], ht[:, F:F + 1])
                t = ib * 8 + t8
                nc.vector.tensor_scalar_mul(
                    obuf[:, t * (NH * F) + h * F: t * (NH * F) + h * F + F],
                    ht[:, 0:F], rcp[:])
                if t8 == 7:
                    emit_outdma(b)

            def emit_outdma(b):
                h, ib = blocks[b]
                nc.sync.dma_start(
                    out_d[ib * 1024:(ib + 1) * 1024, h * F:(h + 1) * F]
                        .rearrange("(t p) c -> p t c", p=128),
                    obuf[:].rearrange("p (t c) -> p t c", c=NH * F)
                        [:, ib * 8:(ib + 1) * 8, h * F:(h + 1) * F])

            for s in range(S + 1):
                if s < S:
                    emit_logit(s)
                if s >= 1:
                    emit_exp(s - 1)
                    emit_acc(s - 1)
                    if (s - 1) % NT == NT - 1:
                        emit_hsb((s - 1) // NT)
                    b_prev = s // NT - 1
                    jc = s % NT
                    if b_prev >= 0 and s < S and jc % 2 == 1:
                        emit_trans(b_prev, jc // 2)
            for t8 in range(8):  # drain last block
                emit_trans(NB - 1, t8)


def _emit_body_v4(nc, tc, X_d, vW_d, vb_d, qw_d, kw_d, qb_d, kb_d, id_d,
                  out_d, prm_d=None):
    """v3 + lane-aligned preamble, no per-chunk pack DMAs.

    q/k are produced by matmuls whose lhsT is zero-padded so head h's scalar
    lands (duplicated) on partitions {32h, 32h+1}; q/k biases ride a ones row
    appended to V^T (so the q/k matmul adds them via K=65).  alpha/beta/P/M
    are then single strided DVE/Pool ops straight out of PSUM into the
    matmul-legal [128, N] layouts (alpha_h/P_h at partition 32h, beta_h/M_h
    at 32h+1).  Head 3 (base 96 — illegal for PE) is staged to a [2, N] tile
    by one DMA per tensor at preamble end; its blocks run last.
    """
    NB = NH * 2
    blocks = [(h, ib) for h in range(NH) for ib in range(2)]
    with tc.tile_pool(name="persist", bufs=1) as pp:
        zz = pp.tile([1, 2], F32R)
        nc.vector.memset(zz[:].bitcast(F32), 0.0)
        nc.scalar.activation(zz[:], zz[:], AF.Exp)

        id65 = pp.tile([F + 1, F + 1], F32)   # identity for postamble transposes
        id_r = pp.tile([128, 128], F32R)
        vt1 = pp.tile([F + 1, N], F32R)       # V^T rows 0..63, row 64 = ones
        ABN = 512 if LOGIT8 else N
        ab_sp = pp.tile([128, ABN], F32R, tag="ab_sp")  # 32h = alpha_h
        pm_sp = pp.tile([128, ABN], F32R, tag="pm_sp")  # 32h = P_h
        # Per-head zero-padded alpha/beta weights: K=128 logit lhsT so every
        # main-loop matmul contracts over all 128 partitions (avoids PE
        # row-group reconfig between K=2 logits and K=128 accs).  Rows
        # 32h/32h+1 hold alpha_h/-beta_h, everything else stays zero; the
        # full pm_sp rides along as rhs since zero lhsT rows null out the
        # other heads.
        BF16 = mybir.dt.bfloat16
        F16 = mybir.dt.float16
        if LOGIT8:
            # fp16 alpha/beta/P/M: 11-bit mantissa keeps exp(logit) rounding
            # at ~3e-3 overall (bf16's 8-bit would be ~2e-2) and 2-byte
            # 128-col weights make the logit matmul FWL-eligible.  Same row
            # layout as the f32r path: head h at rows 32h/32h+1, zeros
            # elsewhere in the per-head lhsT tiles.
            abp = [pp.tile([128, N], F16, name=f"abp{h}", tag=f"abp{h}")
                   for h in range(NH)]
            pm8 = pp.tile([128, N], F16, tag="pm8")
            abh16 = pp.tile([128, N], F16, tag="abh16")
            for h in range(NH):
                eng = nc.vector if h % 2 == 0 else nc.gpsimd
                eng.memset(abp[h][:], 0.0)
        else:
            abp = [pp.tile([128, N], F32R, name=f"abp{h}") for h in range(NH)]
            for h in range(NH):
                eng = nc.vector if h % 2 == 0 else nc.gpsimd
                eng.memset(abp[h][:].bitcast(F32), 0.0)
        ACDT = mybir.dt.bfloat16 if ACC_BF16 else F32R
        vp1 = pp.tile([128, NT * (F + 1)], ACDT)
        obuf = pp.tile([128, NT * NH * F], F32)
        if ACC_BF16:
            nc.vector.memset(vp1[:], 1.0)
        else:
            nc.vector.memset(vp1[:].bitcast(F32), 1.0)
        nc.vector.memset(vt1[F:F + 1, :].bitcast(F32), 1.0)

        if True:
            sp = pp  # preamble tensors live in the persistent pool: their
            # SBUF never gets recycled under the main loop's et/hsb tiles,
            # so the first exp isn't serialized behind the preamble's tail.
            xt = sp.tile([128, 2 * N], F32R)  # X^T: chunk cc at cc*N

            # One packed-param DMA (ident | vW | vb | qw | kw | qb | kb)
            # then the four X^T groups (host supplies X transposed), all FIFO
            # on the sync HWDGE queue: params land by ~2us, X owns the bus
            # right after, and each 512-node group is immediately matmul-ready
            # (no on-chip transposes).
            prm = sp.tile([128, PRM_COLS], F32)
            nc.sync.dma_start(prm[:], prm_d[:])
            # X is declared f32r in DRAM (same bits as f32), so each group
            # DMAs straight into the matmul-ready X^T tile; two HWDGE queues
            # split the 2 MB transfer.
            for g in range(4):
                dq = nc.sync if g % 2 == 0 else nc.scalar
                dq.dma_start(
                    xt[:].rearrange("p (cc n) -> p cc n", cc=2)
                        [:, :, g * 512:(g + 1) * 512],
                    X_d[:].rearrange("(cc p) n -> p cc n", p=128)
                        [:, :, g * 512:(g + 1) * 512])
            ident = prm[:, 0:128]
            vwsb = prm[:, 128:256]
            vb_t = prm[0:F, 256:257]
            qw_t = prm[0:F, 257:261]
            kw_t = prm[0:F, 261:265]
            qb_row = prm[0:1, 265:269]
            kb_row = prm[0:1, 269:273]

            vw_r = sp.tile([128, 128], F32R)
            kscr0 = sp.tile([128, 512], F32)
            kscr1 = sp.tile([128, 512], F32)
            kscr = [kscr0, kscr1]
            nc.vector.tensor_copy(id_r[:], ident[:])
            nc.gpsimd.tensor_copy(id65[:], ident[0:F + 1, 0:F + 1])
            nc.gpsimd.tensor_copy(vw_r[:], vwsb[:])

            # padded q/k lhsT: [65, 128]; rows 0..63 = w dup at {32h,32h+1},
            # row 64 = bias dup there too; zero elsewhere.
            # Padded lhsT columns: even col 32h = +w_h (+bias), odd col
            # 32h+1 = -w_h (-bias).  Odd PSUM lanes then hold -k / -q, so a
            # single full-width max() yields [alpha; -beta] / [P; -M]; the
            # rank-2 logit contraction multiplies the two odd rows together
            # and the negations cancel.
            qkw = sp.tile([F + 1, 128], F32R)
            kkw = sp.tile([F + 1, 128], F32R)
            nc.vector.memset(qkw[:].bitcast(F32), 0.0)
            nc.vector.memset(kkw[:].bitcast(F32), 0.0)
            for rr in range(2):
                sgn = 1.0 if rr == 0 else -1.0
                nc.vector.tensor_scalar_mul(
                    qkw[0:F, :].rearrange("f (h r) -> f h r", r=32)
                        [:, :, rr:rr + 1],
                    qw_t[:].unsqueeze(2), sgn)
                nc.vector.tensor_scalar_mul(
                    qkw[F:F + 1, :].rearrange("o (h r) -> o h r", r=32)
                        [:, :, rr:rr + 1],
                    qb_row[:].unsqueeze(2), sgn)
                nc.vector.tensor_scalar_mul(
                    kkw[0:F, :].rearrange("f (h r) -> f h r", r=32)
                        [:, :, rr:rr + 1],
                    kw_t[:].unsqueeze(2), sgn)
                nc.vector.tensor_scalar_mul(
                    kkw[F:F + 1, :].rearrange("o (h r) -> o h r", r=32)
                        [:, :, rr:rr + 1],
                    kb_row[:].unsqueeze(2), sgn)

            with tc.tile_pool(name="vt_ps", bufs=1, space="PSUM") as vpp, \
                 tc.tile_pool(name="qk_ps", bufs=2, space="PSUM") as qpp, \
                 tc.tile_pool(name="v_ps", bufs=1, space="PSUM") as vsp:
                for g in range(4):
                    sl = slice(g * 512, (g + 1) * 512)
                    vt_ps = vpp.tile([F, 512], F32, tag="vtps",
                                     name=f"vtps{g}")
                    for cc in range(2):
                        nc.tensor.matmul(
                            vt_ps[:],
                            vw_r[:, cc * F: cc * F + F],
                            xt[:, cc * N + g * 512: cc * N + g * 512 + 512],
                            start=(cc == 0), stop=(cc == 1))
                    nc.vector.tensor_scalar_add(vt1[0:F, sl], vt_ps[:],
                                                vb_t[:])
                    qt_ps = qpp.tile([128, 512], F32, tag="qk",
                                     name=f"qtps{g}")
                    nc.tensor.matmul(qt_ps[:], qkw[:], vt1[:, sl],
                                     start=True, stop=True)
                    kt_ps = qpp.tile([128, 512], F32, tag="qk",
                                     name=f"ktps{g}")
                    nc.tensor.matmul(kt_ps[:], kkw[:], vt1[:, sl],
                                     start=True, stop=True)
                    # LeakyReLU with one PSUM read per instruction (the
                    # HW forbids two): Act scales 0.01*k into SBUF scratch,
                    # DVE maxes it against k.  Thanks to the negated odd
                    # lanes this yields [alpha; -beta]; Relu gives [P; -M].
                    gsl = slice(0, 512) if LOGIT8 else sl
                    nc.scalar.mul(kscr[g % 2][:], kt_ps[:], 0.01)
                    nc.vector.tensor_tensor(
                        ab_sp[:, gsl], kscr[g % 2][:], kt_ps[:], ALU.max)
                    nc.scalar.activation(pm_sp[:, gsl], qt_ps[:], AF.Relu)
                    if LOGIT8:
                        nc.vector.tensor_copy(pm8[:, sl],
                                              pm_sp[:, gsl].bitcast(F32))
                        nc.gpsimd.tensor_copy(abh16[:, sl],
                                              ab_sp[:, gsl].bitcast(F32))
                        for h in range(NH):
                            dqs = nc.scalar if (g + h) % 2 else nc.sync
                            dqs.dma_start(
                                abp[h][32 * h: 32 * h + 2, sl],
                                abh16[32 * h: 32 * h + 2, sl])

                    v_ps = vsp.tile([128, 4 * F], F32R, tag="vps",
                                    name=f"vps{g}")
                    for tt in range(4):
                        t = 4 * g + tt
                        nc.tensor.transpose(
                            v_ps[:, tt * F: tt * F + F],
                            vt1[0:F, t * 128: t * 128 + 128],
                            id_r[0:F, 0:F])
                    nc.vector.tensor_copy(
                        vp1[:].rearrange("p (t c) -> p t c", c=F + 1)
                            [:, 4 * g: 4 * g + 4, 0:F],
                        v_ps[:].bitcast(F32).rearrange(
                            "p (t c) -> p t c", c=F))
            # scatter each head's alpha/-beta pair into its padded K=128 lhsT
            if LOGIT8:
                pass  # abp scatter happens per group above
            else:
                for h in range(NH):
                    nc.sync.dma_start(abp[h][32 * h: 32 * h + 2, :],
                                      ab_sp[32 * h: 32 * h + 2, :])

        # ---------- software-pipelined main loop ----------
        # A shield pool pins the 4 banks the preamble just released, so the
        # first two lt tiles claim the never-used banks 4-7 and the first
        # logits aren't serialized behind the tail of the preamble.
        # Pool creation order fixes PSUM bank assignment (first-fit from
        # bank 0): acc and ht soak up the banks the preamble just released
        # (they are needed later / tolerate the wait), so the lt tiles land
        # on the four never-touched banks and the first logits run as soon
        # as their operands are ready.
        S = NB * NT
        from contextlib import ExitStack
        with ExitStack() as mstk:
            accp = mstk.enter_context(
                tc.tile_pool(name="acc_ps", bufs=ACC_BUFS, space="PSUM"))
            ltp = mstk.enter_context(
                tc.tile_pool(name="lt_ps", bufs=LT_BUFS, space="PSUM"))
            htp = mstk.enter_context(
                tc.tile_pool(name="ht_ps", bufs=2, space="PSUM")) \
                if HT_POOL else ltp
            etp = mstk.enter_context(tc.tile_pool(name="et_sb", bufs=8))
            postp = mstk.enter_context(tc.tile_pool(name="post_sb", bufs=2))
            lts, ets, accs, hsbs = {}, {}, {}, {}

            def abpm(h):
                return abp[h][:], (pm8[:] if LOGIT8 else pm_sp[:])

            def emit_logit(s):
                b, jc = divmod(s, NT)
                h, ib = blocks[b]
                ab_h, pm_h = abpm(h)
                lt = ltp.tile([128, 1024], F32, tag="lt", name=f"lt{s}")
                for hf in range(2):
                    nc.tensor.matmul(
                        lt[:, hf * 512: hf * 512 + 512],
                        ab_h[:, jc * 128: jc * 128 + 128],
                        pm_h[:, ib * 1024 + hf * 512:
                             ib * 1024 + hf * 512 + 512],
                        start=True, stop=True)
                lts[s] = lt

            def emit_exp(s):
                et = etp.tile([128, 1024], ACDT, tag="et", name=f"et{s}")
                nc.scalar.activation(et[:], lts[s][:], AF.Exp)
                ets[s] = et

            def emit_acc(s):
                b, jc = divmod(s, NT)
                if jc == 0:
                    accs[b] = accp.tile([F + 1, 1024], F32, tag="acc",
                                        name=f"acc{b}")
                for hf in range(2):
                    nc.tensor.matmul(
                        accs[b][:, hf * 512: hf * 512 + 512],
                        vp1[:, jc * (F + 1): (jc + 1) * (F + 1)],
                        ets[s][:, hf * 512: hf * 512 + 512],
                        start=(jc == 0), stop=(jc == NT - 1))

            def emit_hsb(b):
                hsb = postp.tile([F + 1, 1024], F32, tag="hsb",
                                 name=f"hsb{b}")
                if HSB_SPLIT:
                    nc.vector.tensor_copy(hsb[:, 0:512], accs[b][:, 0:512])
                    nc.scalar.copy(hsb[:, 512:1024], accs[b][:, 512:1024])
                else:
                    nc.vector.tensor_copy(hsb[:, 0:512], accs[b][:, 0:512])
                    nc.vector.tensor_copy(hsb[:, 512:1024],
                                          accs[b][:, 512:1024])
                hsbs[b] = hsb

            def emit_trans(b, t8):
                h, ib = blocks[b]
                # ht tiles default to sharing the lt tag (one rotation covers
                # both); HT_POOL gives them their own 2-bank pool instead
                ht = htp.tile([128, F + 1], F32,
                              tag=("ht" if HT_POOL else "lt"),
                              name=f"ht{b}_{t8}")
                nc.tensor.transpose(
                    ht[:], hsbs[b][:, t8 * 128: t8 * 128 + 128],
                    id65[:])
                rcp = postp.tile([128, 1], F32, tag="rcp",
                                 name=f"rcp{b}_{t8}")
                nc.vector.reciprocal(rcp[:], ht[:, F:F + 1])
                t = ib * 8 + t8
                nc.vector.tensor_scalar_mul(
                    obuf[:, t * (NH * F) + h * F: t * (NH * F) + h * F + F],
                    ht[:, 0:F], rcp[:])
                if b == NB - 1:
                    if t8 == 3:
                        emit_outdma(b, 0, 4)
                    elif t8 == 7:
                        emit_outdma(b, 4, 8)
                elif t8 == 7:
                    emit_outdma(b, 0, 8)

            def emit_outdma(b, t0, t1):
                h, ib = blocks[b]
                nc.sync.dma_start(
                    out_d[ib * 1024 + t0 * 128: ib * 1024 + t1 * 128,
                          h * F:(h + 1) * F]
                        .rearrange("(t p) c -> p t c", p=128),
                    obuf[:].rearrange("p (t c) -> p t c", c=NH * F)
                        [:, ib * 8 + t0: ib * 8 + t1, h * F:(h + 1) * F])

            if ABLATE >= 1:
                nc.vector.memset(obuf[:], 0.0)
            if ABLATE == 17:
                # full logit+exp+acc pipeline, postamble skipped
                for s in range(S + 3):
                    if s < S:
                        emit_logit(s)
                    if 1 <= s <= S:
                        emit_exp(s - 1)
                    if s >= 3:
                        emit_acc(s - 3)
                for b in range(NB):
                    emit_outdma(b, 0, 8)
                return
            if ABLATE in (2, 3, 4):
                # pure Act throughput: one logit tile, 128 exps off it.
                # 2: PSUM f32 -> SBUF f32r (the main-loop shape)
                # 3: PSUM f32 -> SBUF bf16
                # 4: SBUF f32 -> SBUF f32r
                emit_logit(0)
                sbsrc = None
                if ABLATE == 4:
                    sbsrc = etp.tile([128, 1024], F32, tag="sbsrc",
                                     name="sbsrc")
                    nc.vector.memset(sbsrc[:], 0.0)
                for s in range(S):
                    odt = mybir.dt.bfloat16 if ABLATE == 3 else F32R
                    et = etp.tile([128, 1024], odt, tag="et", name=f"et{s}")
                    src = sbsrc if ABLATE == 4 else lts[0]
                    nc.scalar.activation(et[:], src[:], AF.Exp)
                for b in range(NB):
                    emit_outdma(b, 0, 8)
                return
            if ABLATE in (7, 8, 9):
                # PE throughput probes, no postamble:
                # 7: acc matmuls only (K=128, M=65, N=512, f32r)
                # 8: logit matmuls only (K=2, M=128, N=512, f32r)
                # 9: acc matmuls only in bf16
                if ABLATE in (7, 9):
                    dt = mybir.dt.bfloat16 if ABLATE == 9 else F32R
                    et0 = etp.tile([128, 1024], dt, tag="et", name="et0")
                    if ABLATE == 9:
                        nc.vector.memset(et0[:], 1.0)
                        vp1b = etp.tile([128, NT * (F + 1)], dt, tag="vpb",
                                        name="vp1b")
                        nc.vector.tensor_copy(vp1b[:], vp1[:].bitcast(F32))
                        vsrc = vp1b
                    else:
                        nc.vector.memset(et0[:].bitcast(F32), 1.0)
                        vsrc = vp1
                    for s in range(S):
                        b, jc = divmod(s, NT)
                        if jc == 0:
                            accs[b] = accp.tile([F + 1, 1024], F32,
                                                tag="acc", name=f"acc{b}")
                        for hf in range(2):
                            nc.tensor.matmul(
                                accs[b][:, hf * 512: hf * 512 + 512],
                                vsrc[:, jc * (F + 1): (jc + 1) * (F + 1)],
                                et0[:, hf * 512: hf * 512 + 512],
                                start=(jc == 0), stop=(jc == NT - 1))
                else:
                    for s in range(S):
                        emit_logit(s)
                for b in range(NB):
                    emit_outdma(b, 0, 8)
                return
            if ABLATE == 10:
                # preamble + output DMA only
                for b in range(NB):
                    emit_outdma(b, 0, 8)
                return
            if ABLATE == 11:
                # ABL=6 with L/A emission batched in pairs (fewer PE
                # logit<->acc switches), accs ahead of logits in the queue
                et0 = etp.tile([128, 1024], F32R, tag="et", name="et0")
                nc.vector.memset(et0[:].bitcast(F32), 1.0)
                for s in range(S + 2):
                    if s % 2 == 0:
                        for a in (s - 2, s - 1):
                            if 0 <= a < S:
                                emit_acc(a)
                                if a % NT == NT - 1:
                                    emit_hsb(a // NT)
                                b_prev = a // NT - 1
                                jc = a % NT
                                if b_prev >= 0 and jc % 2 == 1:
                                    emit_trans(b_prev, jc // 2)
                        if s < S:
                            ets[s] = et0
                            ets[s + 1] = et0
                            emit_logit(s)
                            emit_logit(s + 1)
                for t8 in range(8):
                    emit_trans(NB - 1, t8)
                return
            if ABLATE == 13:
                # acc-only but every matmul uses a different vp1 chunk
                # (forces a weight change per matmul)
                et0 = etp.tile([128, 1024], F32R, tag="et", name="et0")
                nc.vector.memset(et0[:].bitcast(F32), 1.0)
                for s in range(S):
                    b, jc = divmod(s, NT)
                    if jc == 0:
                        accs[b] = accp.tile([F + 1, 1024], F32, tag="acc",
                                            name=f"acc{b}")
                    for hf in range(2):
                        w = ((jc + 8 * hf) % NT) * (F + 1)
                        nc.tensor.matmul(
                            accs[b][:, hf * 512: hf * 512 + 512],
                            vp1[:, w: w + F + 1],
                            et0[:, hf * 512: hf * 512 + 512],
                            start=(jc == 0), stop=(jc == NT - 1))
                for b in range(NB):
                    emit_outdma(b, 0, 8)
                return
            if ABLATE == 14:
                # logit-only, ONE [128,512] matmul per step (half the work
                # of ABL=8) — isolates per-instruction overhead
                for s in range(S):
                    b, jc = divmod(s, NT)
                    h, ib = blocks[b]
                    ab_h, pm_h = abpm(h)
                    lt = ltp.tile([128, 512], F32, tag="lt", name=f"lt{s}")
                    nc.tensor.matmul(
                        lt[:], ab_h[:, jc * 128: jc * 128 + 128],
                        pm_h[:, ib * 1024: ib * 1024 + 512],
                        start=True, stop=True)
                for b in range(NB):
                    emit_outdma(b, 0, 8)
                return
            if ABLATE == 15:
                # logit-only in bf16 (tests weight-load cost by dtype)
                BF16 = mybir.dt.bfloat16
                abb = etp.tile([128, N], BF16, tag="abb", name="abb")
                pmb = etp.tile([128, N], BF16, tag="pmb", name="pmb")
                nc.vector.tensor_copy(abb[:], ab_sp[:].bitcast(F32))
                nc.vector.tensor_copy(pmb[:], pm_sp[:].bitcast(F32))
                for s in range(S):
                    b, jc = divmod(s, NT)
                    h, ib = blocks[b]
                    h2 = min(h, 2)
                    lt = ltp.tile([128, 1024], F32, tag="lt", name=f"lt{s}")
                    for hf in range(2):
                        nc.tensor.matmul(
                            lt[:, hf * 512: hf * 512 + 512],
                            abb[32 * h2: 32 * h2 + 2,
                                jc * 128: jc * 128 + 128],
                            pmb[32 * h2: 32 * h2 + 2,
                                ib * 1024 + hf * 512:
                                ib * 1024 + hf * 512 + 512],
                            start=True, stop=True)
                for b in range(NB):
                    emit_outdma(b, 0, 8)
                return
            if ABLATE == 6:
                # full pipeline minus Act: logits + acc + postamble, with a
                # constant ones tile standing in for every exp result.
                et0 = etp.tile([128, 1024], F32R, tag="et", name="et0")
                nc.vector.memset(et0[:].bitcast(F32), 1.0)
                for s in range(S + 3):
                    if s < S:
                        emit_logit(s)
                    if 1 <= s <= S:
                        ets[s - 1] = et0
                    if s >= 3:
                        a = s - 3
                        emit_acc(a)
                        if a % NT == NT - 1:
                            emit_hsb(a // NT)
                        b_prev = a // NT - 1
                        jc = a % NT
                        if b_prev >= 0 and jc % 2 == 1:
                            emit_trans(b_prev, jc // 2)
                for t8 in range(8):
                    emit_trans(NB - 1, t8)
                return
            # acc(a) is emitted at step a+3, except each block's first two
            # chunks go 2 steps later (both at a+5): the intervening logit
            # work covers the DVE drain of the previous block's acc tile, so
            # the PE never stalls on the ACC_BUFS=1 WAR at block boundaries.
            def acc_step(a):
                return a + 3 + max(0, 2 - (a % NT))
            for s in range(S + 5):
                if s < S:
                    emit_logit(s)
                if 1 <= s <= S:
                    emit_exp(s - 1)
                if ABLATE != 1:
                    for a in range(max(0, s - 5), s - 2):
                        if acc_step(a) != s:
                            continue
                        emit_acc(a)
                        if a % NT == NT - 1:
                            emit_hsb(a // NT)
                        b_prev = a // NT - 1
                        jc = a % NT
                        if b_prev >= 0 and jc % 2 == 1:
                            emit_trans(b_prev, jc // 2)
            if ABLATE != 1:
                for t8 in range(8):
                    emit_trans(NB - 1, t8)
            else:
                for b in range(NB):
                    emit_outdma(b, 0, 8)

